# revision 31
# baseline (speedup 1.0000x reference)
"""Trainium2 Bass kernel for nn_ConvLinearLayer (KAN-style conv-linear block).

Strategy
--------
Data-parallel over batch: 16 images -> 8 cores x 2 images. All activations
live on-chip in transposed layout [channels(partitions), pixels(free)], so
GEMMs (PE, fp16), depthwise 3x3 convs (8 PE diag-matmul taps + 1 fused DVE
tap), BN stats (accumulator outputs) and BN-apply+ReLU (ACT, per-partition
scale/bias) all hit their natural axes. Train-mode BN needs global batch
stats -> three tiny AllReduces (per-channel sum/sumsq), each overlapped with
surrounding compute (fc2 is interleaved into conv2's slab loop).

All matmul operands are fp16 (fp32 PSUM accumulation, fp32 BN statistics,
fp32 output): fp32 moving operands stream at ~half rate through the PE
array, fp16 streams at full rate with ample mantissa for this tolerance.
All conv outputs stay SBUF-resident; weights are host-pre-tiled so every
DMA is a single contiguous 2D transfer.

Host-side precompute: input/weight transposes + fp16 casts, spline-weight
sum (sum_k sw[:,:,k]/K == one GEMM), channel_scale folded into fus_w1,
fus_w2+b2 folded into fc3 (W3_eff = W3 @ W2, b3_eff = W3 @ b2), conv-bias
folded into the BN affine.
"""

import numpy as np

F16 = np.dtype(np.float16)

K_SPLINE = 10
EPS = 1e-5
HH = 64
PW = 66           # padded row stride (64 + 2 zero border)
PAREA = PW * PW   # 4356
NPIX = HH * HH    # 4096 pixels per image
R = 2 * NPIX      # rows per core (2 images)
CIN = 512
LOW = 128
FULL = 256
CAT = 384
FUSH = 192
COUT = 512

TAPS = [(di, dj) for di in (-1, 0, 1) for dj in (-1, 0, 1)]
DVE_TAP = 0                           # fused into the PSUM-combine stt
PE_TAPS = [t for t in range(9) if t != DVE_TAP]
NBLKS = [1, 2, 4]

_CACHE = {}


def _smalls_layout():
    """Column layout of the packed [128, N] fp32 constants tensor."""
    col = 0
    lay = {}
    lay["rs"] = col; col += 1
    for ci, nblk in enumerate(NBLKS):
        for b in range(nblk):
            for nm in ("g", "be", "bb"):
                lay[f"bn{ci}{nm}{b}"] = col; col += 1
    for ci, nblk in enumerate(NBLKS):
        for b in range(nblk):
            lay[f"wv{ci}{b}"] = col; col += 9
    lay["bf1a"] = col; col += 1
    lay["bf1b"] = col; col += 1
    for m in range(4):
        lay[f"b3b{m}"] = col; col += 1
    for m in range(4):
        lay[f"b3s{m}"] = col; col += 1
    return lay, col


# ---------------------------------------------------------------- host prep

def _prep_shared(inp):
    """All non-x device tensors (replicated across cores), as numpy 2D."""
    f = lambda a: np.ascontiguousarray(np.asarray(a, dtype=np.float32))
    h = lambda a: np.ascontiguousarray(
        np.asarray(a, dtype=np.float32).astype(F16))
    sws = lambda sw: np.asarray(sw, np.float64).sum(-1) / K_SPLINE

    fc1_low_bw = f(inp["fc1_low_bw"]); s1l = f(sws(inp["fc1_low_sw"]))
    fc1_full_bw = f(inp["fc1_full_bw"]); s1f = f(sws(inp["fc1_full_sw"]))
    fc2_bw = f(inp["fc2_low_bw"]); s2 = f(sws(inp["fc2_low_sw"]))
    fc3_bw = f(inp["fc3_bw"]); s3 = f(sws(inp["fc3_sw"]))
    w1 = f(inp["fus_w1"]); b1 = f(inp["fus_b1"])
    w2 = f(inp["fus_w2"]); b2 = f(inp["fus_b2"])
    cs = f(inp["channel_scale"])

    d = {}
    # stage A lhsT tiles (k,m) of [512, 768] packed as [128, 24*128]
    # m-blocks: [lowb, lows, fullb0, fullb1, fulls0, fulls1]
    wA = np.concatenate([fc1_low_bw.T, s1l.T, fc1_full_bw.T, s1f.T], axis=1)
    wAt = np.empty((128, 24 * 128), np.float32)
    for k in range(4):
        for m in range(6):
            wAt[:, (k * 6 + m) * 128:(k * 6 + m + 1) * 128] = \
                wA[k * 128:(k + 1) * 128, m * 128:(m + 1) * 128]
    d["wA"] = h(wAt)
    d["wfc2"] = h(np.concatenate([fc2_bw.T, s2.T], axis=1))      # [128, 256]
    wfus1 = (w1 * cs[None, :]).T                                 # [384, 192]
    wf1t = np.empty((128, 3 * FUSH), np.float32)
    for k in range(3):
        wf1t[:, k * FUSH:(k + 1) * FUSH] = wfus1[k * 128:(k + 1) * 128, :]
    d["wfus1"] = h(wf1t)                                         # [128, 576]
    w3b = fc3_bw @ w2                                            # [512, 192]
    w3s = s3 @ w2
    d["wfc3"] = h(np.concatenate([w3b.T, w3s.T], axis=1))        # [192, 1024]
    b3b = (fc3_bw @ b2).reshape(-1)
    b3s = (s3 @ b2).reshape(-1)

    lay, ncols = _smalls_layout()
    sm = np.zeros((128, ncols), np.float32)
    sm[:, lay["rs"]] = float(np.asarray(inp["res_scale"]).reshape(-1)[0])
    sm[0:128, lay["bf1a"]] = b1[0:128]
    sm[0:64, lay["bf1b"]] = b1[128:192]
    for m in range(4):
        sm[:, lay[f"b3b{m}"]] = b3b[m * 128:(m + 1) * 128]
        sm[:, lay[f"b3s{m}"]] = b3s[m * 128:(m + 1) * 128]

    # depthwise convs: diag tiles (b,t) packed as [128, nblk*9*128]
    for ci, (wname, gname, bname, bbname, Cc) in enumerate([
            ("dw1_w", "dw1_g", "dw1_beta", "dw1_b", LOW),
            ("dw2_w", "dw2_g", "dw2_beta", "dw2_b", FULL),
            ("dw3_w", "dw3_g", "dw3_beta", "dw3_b", COUT)]):
        w = f(inp[wname]).reshape(Cc, 9)                          # [C, taps]
        g = f(inp[gname]).reshape(-1)
        be = f(inp[bname]).reshape(-1)
        bb = f(inp[bbname]).reshape(-1)
        nblk = Cc // 128
        diag = np.zeros((128, nblk * 9 * 128), np.float32)
        for b in range(nblk):
            rows = slice(b * 128, (b + 1) * 128)
            for t in range(9):
                c0 = (b * 9 + t) * 128
                diag[:, c0:c0 + 128] = np.diag(w[rows, t])
            c = lay[f"wv{ci}{b}"]
            sm[:, c:c + 9] = w[rows]
            sm[:, lay[f"bn{ci}g{b}"]] = g[rows]
            sm[:, lay[f"bn{ci}be{b}"]] = be[rows]
            sm[:, lay[f"bn{ci}bb{b}"]] = bb[rows]
        d[f"diag{ci+1}"] = h(diag)
    d["smalls"] = np.ascontiguousarray(sm)
    return d


def _prep_x(x, n_cores, scale=None):
    """Per-core transposed fp16 shards [512, 8192]."""
    x = np.asarray(x, np.float32)
    if scale is not None:
        x = x * scale
    x = x.astype(F16)
    B = x.shape[0]
    per = B // n_cores
    return [np.ascontiguousarray(
        x[c * per:(c + 1) * per].reshape(per * NPIX, CIN).T)
        for c in range(n_cores)]


# ---------------------------------------------------------------- builder

def _build(n_cores):
    import concourse.bacc as bacc
    import concourse.mybir as mybir
    import concourse.tile as tile

    f32 = mybir.dt.float32
    f16 = mybir.dt.float16

    nc = bacc.Bacc("TRN2", target_bir_lowering=False, debug=False,
                   num_devices=n_cores)

    def din(name, shape, dt=f16):
        return nc.dram_tensor(name, list(shape), dt, kind="ExternalInput").ap()

    x_d = din("x_t", (CIN, R))
    xs_d = din("xs_t", (CIN, R))
    wA_d = din("wA", (128, 24 * 128))
    wfc2_d = din("wfc2", (128, 256))
    wfus1_d = din("wfus1", (128, 3 * FUSH))
    wfc3_d = din("wfc3", (FUSH, 1024))
    lay, ncols = _smalls_layout()
    smalls_d = din("smalls", (128, ncols), f32)
    conv_d = []
    for ci, Cc in [(1, LOW), (2, FULL), (3, COUT)]:
        nblk = Cc // 128
        conv_d.append(dict(
            diag=din(f"diag{ci}", (128, nblk * 9 * 128)),
            nblk=nblk))
    out_d = nc.dram_tensor("out_t", [COUT, R], f16, kind="ExternalOutput").ap()

    with tile.TileContext(nc) as tc:
        _emit(nc, tc, mybir, n_cores, x_d, xs_d, wA_d, wfc2_d, wfus1_d,
              wfc3_d, conv_d, smalls_d, lay, ncols, out_d)
    nc.compile()
    return nc


def _emit(nc, tc, mybir, n_cores, x_d, xs_d, wA_d, wfc2_d, wfus1_d,
          wfc3_d, conv_d, smalls_d, lay, ncols, out_d):
    f32 = mybir.dt.float32
    f16 = mybir.dt.float16
    AL = mybir.AluOpType
    AF = mybir.ActivationFunctionType
    inv_n = 1.0 / (n_cores * R)

    class _Pools:
        def __init__(self, tc):
            self.tc = tc
            self.cms = {}
            self.order = []
        def open(self, name, **kw):
            cm = self.tc.tile_pool(name=name, **kw)
            pool = cm.__enter__()
            self.cms[name] = cm
            self.order.append(name)
            return pool
        def close(self, *names):
            names = sorted(names, key=self.order.index, reverse=True)
            for n in names:
                assert n == self.order[-1], (n, self.order)
                self.order.pop()
                self.cms.pop(n).__exit__(None, None, None)
        def close_all(self):
            self.close(*self.order)

    pools = _Pools(tc)

    def pad3(t):
        return t[:].rearrange("p (a b) -> p a b", a=PW)

    # ---------------- persistent small tiles ----------------
    P_pers = pools.open("pers", bufs=1)
    P_tmpv = pools.open("tmpv", bufs=4)
    P_dram = pools.open("dramp", bufs=1, space="DRAM")

    # one DMA for every small constant; everything below is a column slice
    smalls = P_pers.tile([128, ncols], f32, name="smalls", tag="smalls")
    sc = lambda key: smalls[:, lay[key]:lay[key] + 1]

    rs_t = sc("rs")
    bf1a = sc("bf1a")
    bf1b = smalls[0:64, lay["bf1b"]:lay["bf1b"] + 1]
    b3bt = [sc(f"b3b{m}") for m in range(4)]
    b3st = [sc(f"b3s{m}") for m in range(4)]

    dummy_w = P_pers.tile([128, 128], f16, name="dummy_w", tag="dummy_w")
    dummy_rhs = P_pers.tile([128, 512], f16, name="dummy_rhs", tag="dummy_rhs")
    nc.gpsimd.memset(dummy_w[:], 0.0)
    nc.gpsimd.memset(dummy_rhs[:], 0.0)

    bn = []  # bn[ci][blk] = dict(g, be, bb(slices), a, b(tiles))
    for ci in range(3):
        blks = []
        for b in range(conv_d[ci]["nblk"]):
            e = {nm: sc(f"bn{ci}{nm}{b}") for nm in ("g", "be", "bb")}
            e["a"] = P_pers.tile([128, 1], f32, name=f"bn{ci}a{b}", tag=f"bn{ci}a{b}")
            e["b"] = P_pers.tile([128, 1], f32, name=f"bn{ci}b{b}", tag=f"bn{ci}b{b}")
            blks.append(e)
        bn.append(blks)

    wv_t = [[smalls[:, lay[f"wv{ci}{b}"]:lay[f"wv{ci}{b}"] + 9]
             for b in range(conv_d[ci]["nblk"])] for ci in range(3)]

    SLAB = 1024                      # conv slab (PSUM-resident px per step)
    NSLAB = NPIX // SLAB             # 4 slabs per image
    Sp, Qp = [], []
    for ci in range(3):
        Sp.append([P_pers.tile([128, 2 * NSLAB], f32, name=f"Sp{ci}{b}",
                               tag=f"Sp{ci}{b}") for b in range(conv_d[ci]["nblk"])])
        Qp.append([P_pers.tile([128, 2 * NSLAB], f32, name=f"Qp{ci}{b}",
                               tag=f"Qp{ci}{b}") for b in range(conv_d[ci]["nblk"])])
    pack1 = P_pers.tile([128, 2], f32, name="pack1", tag="pack1")
    pack2 = P_pers.tile([128, 4], f32, name="pack2", tag="pack2")
    pack3a = P_pers.tile([128, 4], f32, name="pack3a", tag="pack3a")
    pack3b = P_pers.tile([128, 4], f32, name="pack3b", tag="pack3b")
    gst1 = P_pers.tile([128, 2], f32, name="gst1", tag="gst1")
    gst2 = P_pers.tile([128, 4], f32, name="gst2", tag="gst2")
    gst3a = P_pers.tile([128, 4], f32, name="gst3a", tag="gst3a")
    gst3b = P_pers.tile([128, 4], f32, name="gst3b", tag="gst3b")

    # --------- conv emitter: 8 PE taps + fused DVE tap/combine/stats -----
    # zdst[b] = persistent SBUF tile [128, R]; slab written at
    # [:, img*NPIX + s*SLAB : +SLAB]. After each slab, one queued
    # interleave callback is drained (used to overlap fc2 with conv2).
    FUSED_STT = True     # fuse DVE tap + PSUM-combine + Sp into one stt
    ACT_SQUARE = True     # Qp via ACT Square (v1) vs DVE tensor_tensor_reduce

    def emit_conv(ci, pads, P_cps, P_cacc, P_csq, P_diag, zdst, imgs=(0, 1),
                  interleave=None, blocks=None):
        nblk = conv_d[ci]["nblk"]
        rows = SLAB // HH
        diag_dram = conv_d[ci]["diag"]
        for b in (range(nblk) if blocks is None else blocks):
            dg = P_diag.tile([128, 9 * 128], f16, name="dg", tag="dg")
            nc.sync.dma_start(
                dg[:], diag_dram[:, b * 9 * 128:(b + 1) * 9 * 128])
            for img in imgs:
                p3 = pad3(pads[b][img])
                for s in range(NSLAB):
                    r0 = s * rows
                    ps = P_cps.tile([128, SLAB], f32, name=f"cps{ci}", tag=f"cps{ci}")
                    for ti, t in enumerate(PE_TAPS):
                        di, dj = TAPS[t]
                        rhs = p3[:, 1 + di + r0:1 + di + r0 + rows,
                                 1 + dj:1 + dj + HH]
                        for nn in range(SLAB // 512):
                            rr = nn * (512 // HH)
                            nc.tensor.matmul(
                                ps[:, nn * 512:(nn + 1) * 512],
                                dg[:, t * 128:(t + 1) * 128],
                                rhs[:, rr:rr + (512 // HH), :],
                                start=(ti == 0), stop=(ti == len(PE_TAPS) - 1))
                    slot = img * NSLAB + s
                    col = img * NPIX + s * SLAB
                    zsl = zdst[b][:, col:col + SLAB]
                    di, dj = TAPS[DVE_TAP]
                    tap_ap = p3[:, 1 + di + r0:1 + di + r0 + rows,
                                1 + dj:1 + dj + HH]
                    if FUSED_STT:
                        nc.vector.scalar_tensor_tensor(
                            zsl.rearrange("p (a b) -> p a b", a=rows),
                            tap_ap,
                            wv_t[ci][b][:, DVE_TAP:DVE_TAP + 1],
                            ps[:].rearrange("p (a b) -> p a b", a=rows),
                            op0=AL.mult, op1=AL.add,
                            accum_out=Sp[ci][b][:, slot:slot + 1])
                    else:
                        acc = P_cacc.tile([128, SLAB], f16, name="cacc", tag="cacc")
                        nc.vector.tensor_scalar(
                            acc[:].rearrange("p (a b) -> p a b", a=rows),
                            tap_ap, wv_t[ci][b][:, DVE_TAP:DVE_TAP + 1], None,
                            op0=AL.mult)
                        nc.vector.scalar_tensor_tensor(
                            zsl, acc[:], 0.0, ps[:], op0=AL.bypass, op1=AL.add,
                            accum_out=Sp[ci][b][:, slot:slot + 1])
                    sq = P_csq.tile([128, SLAB], f16, name="sqs", tag="sqs")
                    if ACT_SQUARE:
                        nc.scalar.activation(sq[:], zsl, AF.Square,
                                             accum_out=Qp[ci][b][:, slot:slot + 1])
                    else:
                        nc.vector.tensor_tensor_reduce(
                            sq[:], zsl, zsl, 1.0, 0.0, op0=AL.mult, op1=AL.add,
                            accum_out=Qp[ci][b][:, slot:slot + 1])
                    if interleave:
                        interleave.pop(0)()

    def open_conv_pools(sfx):
        return (pools.open(f"cps{sfx}", bufs=2, space="PSUM"),
                pools.open(f"cacc{sfx}", bufs=2),
                pools.open(f"csq{sfx}", bufs=2),
                pools.open(f"diag{sfx}", bufs=2))

    def close_conv_pools(sfx):
        pools.close(f"diag{sfx}", f"csq{sfx}", f"cacc{sfx}", f"cps{sfx}")

    def bn_math(ci, b, S_ap, Q_ap):
        e = bn[ci][b]
        tt = lambda tag: P_tmpv.tile([128, 1], f32, name=tag, tag=tag)
        m = tt("bnm"); e2 = tt("bne"); m2 = tt("bnm2"); v = tt("bnv")
        sq = tt("bnsq"); iv = tt("bniv"); mb = tt("bnmb"); ab = tt("bnab")
        nc.vector.tensor_scalar(m[:], S_ap, inv_n, None, op0=AL.mult)
        nc.vector.tensor_scalar(e2[:], Q_ap, inv_n, None, op0=AL.mult)
        nc.vector.tensor_tensor(m2[:], m[:], m[:], op=AL.mult)
        nc.vector.tensor_tensor(v[:], e2[:], m2[:], op=AL.subtract)
        nc.vector.tensor_scalar(v[:], v[:], EPS, None, op0=AL.add)
        nc.scalar.activation(sq[:], v[:], AF.Sqrt)
        nc.vector.reciprocal(iv[:], sq[:])
        nc.vector.tensor_tensor(e["a"][:], e["g"], iv[:], op=AL.mult)
        nc.vector.tensor_tensor(mb[:], m[:], e["bb"], op=AL.add)
        nc.vector.tensor_tensor(ab[:], e["a"][:], mb[:], op=AL.mult)
        nc.vector.tensor_tensor(e["b"][:], e["be"], ab[:], op=AL.subtract)

    def allreduce(pack, gst, ncols, tag):
        if n_cores == 1:
            nc.vector.tensor_copy(gst[:], pack[:])
            return
        ib = P_dram.tile([128, ncols], f32, name=f"cc_in{tag}", tag=f"cc_in{tag}")
        ob = P_dram.tile([128, ncols], f32, name=f"cc_out{tag}", tag=f"cc_out{tag}")
        nc.gpsimd.dma_start(ib[:], pack[:])
        nc.gpsimd.collective_compute(
            "AllReduce", AL.add,
            replica_groups=[list(range(n_cores))],
            ins=[ib.opt()], outs=[ob.opt()])
        nc.gpsimd.dma_start(gst[:], ob[:])

    def reduce_stats(pack, cols):
        for i, (ci, b) in enumerate(cols):
            nc.vector.tensor_reduce(pack[:, 2 * i:2 * i + 1], Sp[ci][b][:],
                                    axis=mybir.AxisListType.X, op=AL.add)
            nc.vector.tensor_reduce(pack[:, 2 * i + 1:2 * i + 2], Qp[ci][b][:],
                                    axis=mybir.AxisListType.X, op=AL.add)

    # persistent SBUF activations (fp16)
    P_hf = pools.open("hfp", bufs=1)
    hf1a = P_hf.tile([128, R], f16, name="hf1a", tag="hf1a")
    hf1b = P_hf.tile([64, R], f16, name="hf1b", tag="hf1b")

    # =================== stage A: fc1_low + fc1_full ==================
    P_z12 = pools.open("z12p", bufs=1)
    z1_sb = [P_z12.tile([128, R], f16, name="z1sb", tag="z1sb")]
    z2_sb = [P_z12.tile([128, R], f16, name=f"z2sb{b}", tag=f"z2sb{b}")
             for b in range(2)]
    yl_sb = P_z12.tile([128, R], f16, name="ylsb", tag="ylsb")

    P_pad2 = pools.open("pads2", bufs=1)
    P_pad1 = pools.open("pads1", bufs=1)
    y1p = [P_pad1.tile([128, PAREA], f16, name=f"y1p{i}", tag=f"y1p{i}")
           for i in range(2)]
    y2p = [[P_pad2.tile([128, PAREA], f16, name=f"y2p{b}{i}", tag=f"y2p{b}{i}")
            for i in range(2)] for b in range(2)]
    for t in y1p:
        nc.vector.memset(t[:], 0.0)
    for i in range(2):            # image-major: img0 pads ready first
        for b in range(2):
            nc.gpsimd.memset(y2p[b][i][:], 0.0)

    P_wA = pools.open("wAp", bufs=1)
    P_xk = pools.open("xk", bufs=2)
    P_tmpA = pools.open("tmpA", bufs=2)
    P_psA = pools.open("psA", bufs=2, space="PSUM")
    wA_sb = P_wA.tile([128, 24 * 128], f16, name="wAsb", tag="wAsb")
    wAt = lambda k, m: wA_sb[:, (k * 6 + m) * 128:(k * 6 + m + 1) * 128]
    pairs = [(0, 1, lambda img: y1p[img]),
             (2, 4, lambda img: y2p[0][img]),
             (3, 5, lambda img: y2p[1][img])]
    for ch in range(8):
        img, lrow = ch // 4, (ch % 4) * 16
        xs = []
        for k in range(4):
            xt = P_xk.tile([128, 1024], f16, name=f"xk{k}", tag=f"xk{k}")
            eng = nc.sync if k % 2 == 0 else nc.scalar
            eng.dma_start(
                xt[:], x_d[k * 128:(k + 1) * 128, ch * 1024:(ch + 1) * 1024])
            xs.append(xt)
        if ch == 0:
            nc.sync.dma_start(wA_sb[:], wA_d[:])
        for bm, sm, dest in pairs:
            psB = P_psA.tile([128, 1024], f32, name="psB", tag="psB")
            psS = P_psA.tile([128, 1024], f32, name="psS", tag="psS")
            for k in range(4):
                for nn in range(2):
                    sl = slice(nn * 512, (nn + 1) * 512)
                    nc.tensor.matmul(psB[:, sl], wAt(k, bm), xs[k][:, sl],
                                     start=(k == 0), stop=(k == 3))
                    nc.tensor.matmul(psS[:, sl], wAt(k, sm), xs[k][:, sl],
                                     start=(k == 0), stop=(k == 3))
            tmp = P_tmpA.tile([128, 1024], f16, name="siluA", tag="siluA")
            nc.scalar.activation(tmp[:], psB[:], AF.Silu)
            outap = pad3(dest(img))[:, 1 + lrow:1 + lrow + 16, 1:65]
            nc.vector.scalar_tensor_tensor(
                outap,
                psS[:].rearrange("p (a b) -> p a b", a=16),
                0.0,
                tmp[:].rearrange("p (a b) -> p a b", a=16),
                op0=AL.bypass, op1=AL.add)
    pools.close("psA", "tmpA", "xk", "wAp")
    nc.scalar.dma_start(smalls[:], smalls_d[:])

    # ============ conv1 -> AR1 (overlapped with conv2+fc2) =============
    cpools1 = open_conv_pools("c1")
    emit_conv(0, [y1p], *cpools1, zdst=z1_sb)
    close_conv_pools("c1")
    pools.close("pads1")
    reduce_stats(pack1, [(0, 0)])
    allreduce(pack1, gst1, 2, "a1")

    # fc2 work units, interleaved into conv2's 16 slab iterations
    P_w2 = pools.open("wfc2p", bufs=1)
    P_t2 = pools.open("fc2t", bufs=2)
    P_ps2 = pools.open("psF2", bufs=1, space="PSUM")
    w2_sb = P_w2.tile([128, 256], f16, name="w2sb", tag="w2sb")
    nc.sync.dma_start(w2_sb[:], wfc2_d[:])

    def fc2_chunk(ch):
        sl = slice(ch * 1024, (ch + 1) * 1024)
        z1b = P_t2.tile([128, 1024], f16, name="z1b", tag="z1b")
        nc.scalar.activation(z1b[:], z1_sb[0][:, sl], AF.Relu,
                             bias=bn[0][0]["b"][:], scale=bn[0][0]["a"][:])
        psB = P_ps2.tile([128, 1024], f32, name="ps2B", tag="ps2B")
        psS = P_ps2.tile([128, 1024], f32, name="ps2S", tag="ps2S")
        for nn in range(2):
            s2 = slice(nn * 512, (nn + 1) * 512)
            nc.tensor.matmul(psB[:, s2], w2_sb[:, 0:128], z1b[:, s2],
                             start=True, stop=True)
            nc.tensor.matmul(psS[:, s2], w2_sb[:, 128:256], z1b[:, s2],
                             start=True, stop=True)
        tmp = P_t2.tile([128, 1024], f16, name="silu2", tag="silu2")
        nc.scalar.activation(tmp[:], psB[:], AF.Silu)
        nc.vector.scalar_tensor_tensor(yl_sb[:, sl], psS[:], 0.0, tmp[:],
                                       op0=AL.bypass, op1=AL.add)

    INTERLEAVE_FC2 = True
    if INTERLEAVE_FC2:
        todo = [lambda: bn_math(0, 0, gst1[:, 0:1], gst1[:, 1:2])]
        todo += [lambda ch=ch: fc2_chunk(ch) for ch in range(8)]
        ilv = [lambda: None] * 5 + todo
        ilv += [lambda: None] * (16 - len(ilv))
    else:
        ilv = None

    cpools2 = open_conv_pools("c2")
    emit_conv(1, y2p, *cpools2, zdst=z2_sb, interleave=ilv)
    close_conv_pools("c2")
    if not INTERLEAVE_FC2:
        bn_math(0, 0, gst1[:, 0:1], gst1[:, 1:2])
        for ch in range(8):
            fc2_chunk(ch)
    pools.close("psF2", "fc2t", "wfc2p", "pads2")
    reduce_stats(pack2, [(1, 0), (1, 1)])
    allreduce(pack2, gst2, 4, "a2")
    bn_math(1, 0, gst2[:, 0:1], gst2[:, 1:2])
    bn_math(1, 1, gst2[:, 2:3], gst2[:, 3:4])

    # keep the PE array busy through the AR2 collective so the HAM clock
    # gate stays at 8/8 into fusion/fc3 (a >3.4us PE-idle window would
    # re-throttle to 1.2 GHz for tens of us). Garbage-in, never-read-out.
    P_warm = pools.open("pswarm", bufs=1, space="PSUM")
    wps = P_warm.tile([128, 512], f32, name="wps", tag="wps")
    NWARM = 130
    for i in range(NWARM):
        nc.tensor.matmul(wps[:], dummy_w[:], dummy_rhs[:],
                         start=(i == 0), stop=(i == NWARM - 1),
                         skip_group_check=True)
    pools.close("pswarm")

    # =================== fusion linear 1 -> hf1 (SBUF) ==================
    P_wf1 = pools.open("wfu1", bufs=1)
    P_tf1 = pools.open("fu1t", bufs=3)
    P_psf1 = pools.open("psFu1", bufs=2, space="PSUM")
    wf1_sb = P_wf1.tile([128, 3 * FUSH], f16, name="wf1sb", tag="wf1sb")
    nc.sync.dma_start(wf1_sb[:], wfus1_d[:])
    wf1t = lambda k, m: wf1_sb[:, k * FUSH + m * 128:k * FUSH + m * 128 + (64 if m else 128)]
    zero64 = P_wf1.tile([64, 1024], f16, name="zero64", tag="zero64")
    nc.gpsimd.memset(zero64[:], 0.0)
    for ch in range(8):
        sl = slice(ch * 1024, (ch + 1) * 1024)
        z2b0 = P_tf1.tile([128, 1024], f16, name="z2b0", tag="z2b0")
        z2b1 = P_tf1.tile([128, 1024], f16, name="z2b1", tag="z2b1")
        nc.scalar.activation(z2b0[:], z2_sb[0][:, sl], AF.Relu,
                             bias=bn[1][0]["b"][:], scale=bn[1][0]["a"][:])
        nc.vector.tensor_scalar(z2b1[:], z2_sb[1][:, sl], bn[1][1]["a"][:],
                                bn[1][1]["b"][:], op0=AL.mult, op1=AL.add)
        nc.vector.tensor_scalar(z2b1[:], z2b1[:], 0.0, None, op0=AL.max)
        rhs = [yl_sb[:, sl], z2b0[:], z2b1[:]]
        ps0 = P_psf1.tile([128, 1024], f32, name="psf1a", tag="psf1a")
        ps1 = P_psf1.tile([64, 1024], f32, name="psf1b", tag="psf1b")
        for k in range(3):
            for nn in range(2):
                s2 = slice(nn * 512, (nn + 1) * 512)
                nc.tensor.matmul(ps0[:, s2], wf1t(k, 0), rhs[k][:, s2],
                                 start=(k == 0), stop=(k == 2))
                nc.tensor.matmul(ps1[:, s2], wf1t(k, 1), rhs[k][:, s2],
                                 start=(k == 0), stop=(k == 2))
        nc.scalar.activation(hf1a[:, sl], ps0[:], AF.Relu, bias=bf1a)
        nc.vector.scalar_tensor_tensor(hf1b[:, sl], ps1[:], bf1b, zero64[:],
                                       op0=AL.add, op1=AL.max)
    pools.close("psFu1", "fu1t", "wfu1")
    pools.close("z12p")

    # ============= fc3' + conv3 (block-split stats) + finals ============
    P_z3 = pools.open("z3p", bufs=1)
    z3_sb = [P_z3.tile([128, R], f16, name=f"z3sb{b}", tag=f"z3sb{b}")
             for b in range(4)]
    P_w3 = pools.open("wfc3p", bufs=1)
    P_h3 = pools.open("h3p", bufs=1)
    P_t3 = pools.open("fc3t", bufs=3)
    P_ps3 = pools.open("psF3", bufs=2, space="PSUM")
    P_xc = pools.open("xcp", bufs=3)
    P_fin = pools.open("fint", bufs=3)
    cpools3 = open_conv_pools("c3")
    w3k = [P_w3.tile([128, 1024], f16, name="w3k0", tag="w3k0"),
           P_w3.tile([64, 1024], f16, name="w3k1", tag="w3k1")]
    nc.sync.dma_start(w3k[0][:], wfc3_d[0:128, :])
    nc.sync.dma_start(w3k[1][:], wfc3_d[128:192, :])

    def fin_chunk(b, ch):
        rows = slice(b * 128, (b + 1) * 128)
        sl = slice(ch * 2048, (ch + 1) * 2048)
        xc = P_xc.tile([128, 2048], f16, name="xc", tag="xc")
        nc.sync.dma_start(xc[:], xs_d[rows, sl])
        t = P_fin.tile([128, 2048], f16, name="trelu", tag="trelu")
        if ch == 3:   # balance: every 4th BN+ReLU on the vector engine
            nc.vector.tensor_scalar(t[:], z3_sb[b][:, sl],
                                    bn[2][b]["a"][:], bn[2][b]["b"][:],
                                    op0=AL.mult, op1=AL.add)
            nc.vector.tensor_scalar(t[:], t[:], 0.0, None, op0=AL.max)
        else:
            nc.scalar.activation(t[:], z3_sb[b][:, sl], AF.Relu,
                                 bias=bn[2][b]["b"][:], scale=bn[2][b]["a"][:])
        ob = P_fin.tile([128, 2048], f16, name="ob", tag="ob")
        nc.vector.tensor_tensor(ob[:], xc[:], t[:], op=AL.add)
        nc.gpsimd.dma_start(out_d[rows, sl], ob[:])

    h3sets = []
    for img in range(2):
        h3 = [P_h3.tile([128, PAREA], f16, name=f"h3p{b}", tag=f"h3p{b}")
              for b in range(4)]
        h3sets.append(h3)
        for t in h3:
            nc.gpsimd.memset(t[:], 0.0)
        for ch in range(8):           # 512-px chunks within image
            r0 = ch * 8
            sl = slice(img * NPIX + ch * 512, img * NPIX + (ch + 1) * 512)
            rhs = [hf1a[:, sl], hf1b[:, sl]]
            for mp in range(4):
                psB = P_ps3.tile([128, 512], f32, name="ps3B", tag="ps3B")
                psS = P_ps3.tile([128, 512], f32, name="ps3S", tag="ps3S")
                for kk in range(2):
                    nc.tensor.matmul(psB[:], w3k[kk][:, mp * 128:(mp + 1) * 128],
                                     rhs[kk], start=(kk == 0), stop=(kk == 1))
                    nc.tensor.matmul(psS[:], w3k[kk][:, (4 + mp) * 128:(5 + mp) * 128],
                                     rhs[kk], start=(kk == 0), stop=(kk == 1))
                tmp = P_t3.tile([128, 512], f16, name="silu3", tag="silu3")
                nc.scalar.activation(tmp[:], psB[:], AF.Silu, bias=b3bt[mp])
                outap = pad3(h3[mp])[:, 1 + r0:1 + r0 + 8, 1:65]
                nc.vector.scalar_tensor_tensor(
                    outap,
                    psS[:].rearrange("p (a b) -> p a b", a=8),
                    b3st[mp],
                    tmp[:].rearrange("p (a b) -> p a b", a=8),
                    op0=AL.add, op1=AL.add)
        if img == 0:
            emit_conv(2, [{0: h3[b]} for b in range(4)], *cpools3,
                      zdst=z3_sb, imgs=(0,))
    # conv3 img1: blocks 0-1, then AR3a fires while blocks 2-3 conv and
    # the finals for blocks 0-1 interleave into their slab loop.
    emit_conv(2, [{1: h3sets[1][b]} for b in range(4)], *cpools3,
              zdst=z3_sb, imgs=(1,), blocks=(0, 1))
    reduce_stats(pack3a, [(2, 0), (2, 1)])
    allreduce(pack3a, gst3a, 4, "a3a")
    bn_math(2, 0, gst3a[:, 0:1], gst3a[:, 1:2])
    bn_math(2, 1, gst3a[:, 2:3], gst3a[:, 3:4])
    ilv3 = [lambda b=b, ch=ch: fin_chunk(b, ch)
            for b in (0, 1) for ch in range(4)]
    emit_conv(2, [{1: h3sets[1][b]} for b in range(4)], *cpools3,
              zdst=z3_sb, imgs=(1,), blocks=(2, 3), interleave=ilv3)
    reduce_stats(pack3b, [(2, 2), (2, 3)])
    allreduce(pack3b, gst3b, 4, "a3b")
    bn_math(2, 2, gst3b[:, 0:1], gst3b[:, 1:2])
    bn_math(2, 3, gst3b[:, 2:3], gst3b[:, 3:4])
    for b in (2, 3):
        for ch in range(4):
            fin_chunk(b, ch)
    pools.close_all()


def _get_built(n_cores):
    if n_cores not in _CACHE:
        _CACHE[n_cores] = _build(n_cores)
    return _CACHE[n_cores]


def make_in_maps(inputs, n_cores):
    shared = _prep_shared(inputs)
    xt = _prep_x(inputs["x"], n_cores)
    rsv = float(np.asarray(inputs["res_scale"]).reshape(-1)[0])
    xst = _prep_x(inputs["x"], n_cores, scale=rsv)
    return [dict(shared, x_t=xt[c], xs_t=xst[c]) for c in range(n_cores)]


def kernel(**inputs):
    from concourse.bass_utils import run_bass_kernel_spmd

    assert int(np.asarray(inputs["H"])) == HH and int(np.asarray(inputs["W"])) == HH
    n_cores = 8
    nc = _get_built(n_cores)
    in_maps = make_in_maps(inputs, n_cores)
    res = run_bass_kernel_spmd(nc, in_maps, core_ids=list(range(n_cores)))
    B = np.asarray(inputs["x"]).shape[0]
    per = B // n_cores
    out = np.empty((B, NPIX, CIN), np.float32)
    for c in range(n_cores):
        out[c * per:(c + 1) * per] = \
            res.results[c]["out_t"].astype(np.float32).T.reshape(per, NPIX, CIN)
    return out


# ------------------------------------------------------------- profiling

def _install_ntff_hook():
    """The agent image's antenv lacks axon_hooks; recreate the NTFF profile
    hook via ctypes on the axon PJRT .so (same ABI as trn_boot's)."""
    import contextlib, ctypes, sys, types
    so = "/opt/axon/libaxon_pjrt.so"
    try:
        import antenv.axon_hooks  # noqa: F401
        return True
    except ImportError:
        pass
    try:
        lib = ctypes.CDLL(so)
    except OSError:
        return False
    if not hasattr(lib, "axon_start_nrt_profile"):
        return False
    lib.axon_start_nrt_profile.argtypes = [
        ctypes.POINTER(ctypes.c_int64), ctypes.c_size_t]
    lib.axon_start_nrt_profile.restype = ctypes.c_int64
    lib.axon_stop_nrt_profile.argtypes = [ctypes.c_char_p]
    lib.axon_stop_nrt_profile.restype = ctypes.c_int64

    @contextlib.contextmanager
    def _hook(output_dir, device_ids):
        import jax
        jax.devices()
        if device_ids:
            ids = (ctypes.c_int64 * len(device_ids))(*device_ids)
            rc = lib.axon_start_nrt_profile(ids, len(device_ids))
        else:
            rc = lib.axon_start_nrt_profile(None, 0)
        if rc != 0:
            raise RuntimeError(f"axon_start_nrt_profile rc={rc}")
        try:
            yield
        finally:
            n = lib.axon_stop_nrt_profile(str(output_dir).encode())
            print(f"profile: {n} ntff file(s) -> {output_dir}", file=sys.stderr)

    mod = types.ModuleType("antenv.axon_hooks")
    mod.get_axon_ntff_profile_hook = lambda: _hook
    mod.set_axon_ntff_profile_hook = lambda h: None
    sys.modules["antenv.axon_hooks"] = mod
    import concourse.bass_utils as bu
    bu.upload_artifacts = lambda tmpdir: f"local:{tmpdir}"
    return True


def benchmark(inputs, iters=2, tmpdir=None):
    """Device-only HW execution time (ns) via neuron-profile NTFF trace."""
    import os, tempfile
    from concourse.bass_utils import run_bass_kernel_spmd

    if not _install_ntff_hook():
        raise RuntimeError("NTFF profiling unavailable")
    if tmpdir:
        os.makedirs(tmpdir, exist_ok=True)
    n_cores = 8
    nc = _get_built(n_cores)
    in_maps = make_in_maps(inputs, n_cores)
    times = []
    for i in range(max(1, min(iters, 3))):
        td = tempfile.mkdtemp(dir=tmpdir) if tmpdir else None
        res = run_bass_kernel_spmd(nc, in_maps, core_ids=list(range(n_cores)),
                                   trace=True, tmpdir=td)
        if res.exec_time_ns is not None:
            times.append(res.exec_time_ns)
    if not times:
        raise RuntimeError("no exec_time_ns from traced runs")
    return min(times)


# revision 32
# speedup vs baseline: 1.0213x; 1.0213x over previous
"""Trainium2 Bass kernel for nn_ConvLinearLayer (KAN-style conv-linear block).

Strategy
--------
Data-parallel over batch: 16 images -> 8 cores x 2 images. All activations
live on-chip in transposed layout [channels(partitions), pixels(free)], so
GEMMs (PE, fp16), depthwise 3x3 convs (8 PE diag-matmul taps + 1 fused DVE
tap), BN stats (accumulator outputs) and BN-apply+ReLU (ACT, per-partition
scale/bias) all hit their natural axes. Train-mode BN needs global batch
stats -> three tiny AllReduces (per-channel sum/sumsq), each overlapped with
surrounding compute (fc2 is interleaved into conv2's slab loop).

All matmul operands are fp16 (fp32 PSUM accumulation, fp32 BN statistics,
fp32 output): fp32 moving operands stream at ~half rate through the PE
array, fp16 streams at full rate with ample mantissa for this tolerance.
All conv outputs stay SBUF-resident; weights are host-pre-tiled so every
DMA is a single contiguous 2D transfer.

Host-side precompute: input/weight transposes + fp16 casts, spline-weight
sum (sum_k sw[:,:,k]/K == one GEMM), channel_scale folded into fus_w1,
fus_w2+b2 folded into fc3 (W3_eff = W3 @ W2, b3_eff = W3 @ b2), conv-bias
folded into the BN affine.
"""

import numpy as np

F16 = np.dtype(np.float16)

K_SPLINE = 10
EPS = 1e-5
HH = 64
PW = 66           # padded row stride (64 + 2 zero border)
PAREA = PW * PW   # 4356
NPIX = HH * HH    # 4096 pixels per image
R = 2 * NPIX      # rows per core (2 images)
CIN = 512
LOW = 128
FULL = 256
CAT = 384
FUSH = 192
COUT = 512

TAPS = [(di, dj) for di in (-1, 0, 1) for dj in (-1, 0, 1)]
DVE_TAP = 0                           # fused into the PSUM-combine stt
PE_TAPS = [t for t in range(9) if t != DVE_TAP]
NBLKS = [1, 2, 4]

_CACHE = {}


def _smalls_layout():
    """Column layout of the packed [128, N] fp32 constants tensor."""
    col = 0
    lay = {}
    lay["rs"] = col; col += 1
    for ci, nblk in enumerate(NBLKS):
        for b in range(nblk):
            for nm in ("g", "be", "bb"):
                lay[f"bn{ci}{nm}{b}"] = col; col += 1
    for ci, nblk in enumerate(NBLKS):
        for b in range(nblk):
            lay[f"wv{ci}{b}"] = col; col += 9
    lay["bf1a"] = col; col += 1
    lay["bf1b"] = col; col += 1
    for m in range(4):
        lay[f"b3b{m}"] = col; col += 1
    for m in range(4):
        lay[f"b3s{m}"] = col; col += 1
    return lay, col


# ---------------------------------------------------------------- host prep

def _prep_shared(inp):
    """All non-x device tensors (replicated across cores), as numpy 2D."""
    f = lambda a: np.ascontiguousarray(np.asarray(a, dtype=np.float32))
    h = lambda a: np.ascontiguousarray(
        np.asarray(a, dtype=np.float32).astype(F16))
    sws = lambda sw: np.asarray(sw, np.float64).sum(-1) / K_SPLINE

    fc1_low_bw = f(inp["fc1_low_bw"]); s1l = f(sws(inp["fc1_low_sw"]))
    fc1_full_bw = f(inp["fc1_full_bw"]); s1f = f(sws(inp["fc1_full_sw"]))
    fc2_bw = f(inp["fc2_low_bw"]); s2 = f(sws(inp["fc2_low_sw"]))
    fc3_bw = f(inp["fc3_bw"]); s3 = f(sws(inp["fc3_sw"]))
    w1 = f(inp["fus_w1"]); b1 = f(inp["fus_b1"])
    w2 = f(inp["fus_w2"]); b2 = f(inp["fus_b2"])
    cs = f(inp["channel_scale"])

    d = {}
    # stage A lhsT tiles (k,m) of [512, 768] packed as [128, 24*128]
    # m-blocks: [lowb, lows, fullb0, fullb1, fulls0, fulls1]
    wA = np.concatenate([fc1_low_bw.T, s1l.T, fc1_full_bw.T, s1f.T], axis=1)
    wAt = np.empty((128, 24 * 128), np.float32)
    for k in range(4):
        for m in range(6):
            wAt[:, (k * 6 + m) * 128:(k * 6 + m + 1) * 128] = \
                wA[k * 128:(k + 1) * 128, m * 128:(m + 1) * 128]
    d["wA"] = h(wAt)
    d["wfc2"] = h(np.concatenate([fc2_bw.T, s2.T], axis=1))      # [128, 256]
    wfus1 = (w1 * cs[None, :]).T                                 # [384, 192]
    wf1t = np.empty((128, 3 * FUSH), np.float32)
    for k in range(3):
        wf1t[:, k * FUSH:(k + 1) * FUSH] = wfus1[k * 128:(k + 1) * 128, :]
    d["wfus1"] = h(wf1t)                                         # [128, 576]
    w3b = fc3_bw @ w2                                            # [512, 192]
    w3s = s3 @ w2
    d["wfc3"] = h(np.concatenate([w3b.T, w3s.T], axis=1))        # [192, 1024]
    b3b = (fc3_bw @ b2).reshape(-1)
    b3s = (s3 @ b2).reshape(-1)

    lay, ncols = _smalls_layout()
    sm = np.zeros((128, ncols), np.float32)
    sm[:, lay["rs"]] = float(np.asarray(inp["res_scale"]).reshape(-1)[0])
    sm[0:128, lay["bf1a"]] = b1[0:128]
    sm[0:64, lay["bf1b"]] = b1[128:192]
    for m in range(4):
        sm[:, lay[f"b3b{m}"]] = b3b[m * 128:(m + 1) * 128]
        sm[:, lay[f"b3s{m}"]] = b3s[m * 128:(m + 1) * 128]

    # depthwise convs: diag tiles (b,t) packed as [128, nblk*9*128]
    for ci, (wname, gname, bname, bbname, Cc) in enumerate([
            ("dw1_w", "dw1_g", "dw1_beta", "dw1_b", LOW),
            ("dw2_w", "dw2_g", "dw2_beta", "dw2_b", FULL),
            ("dw3_w", "dw3_g", "dw3_beta", "dw3_b", COUT)]):
        w = f(inp[wname]).reshape(Cc, 9)                          # [C, taps]
        g = f(inp[gname]).reshape(-1)
        be = f(inp[bname]).reshape(-1)
        bb = f(inp[bbname]).reshape(-1)
        nblk = Cc // 128
        diag = np.zeros((128, nblk * 9 * 128), np.float32)
        for b in range(nblk):
            rows = slice(b * 128, (b + 1) * 128)
            for t in range(9):
                c0 = (b * 9 + t) * 128
                diag[:, c0:c0 + 128] = np.diag(w[rows, t])
            c = lay[f"wv{ci}{b}"]
            sm[:, c:c + 9] = w[rows]
            sm[:, lay[f"bn{ci}g{b}"]] = g[rows]
            sm[:, lay[f"bn{ci}be{b}"]] = be[rows]
            sm[:, lay[f"bn{ci}bb{b}"]] = bb[rows]
        d[f"diag{ci+1}"] = h(diag)
    d["smalls"] = np.ascontiguousarray(sm)
    return d


def _prep_x(x, n_cores, scale=None):
    """Per-core transposed fp16 shards [512, 8192]."""
    x = np.asarray(x, np.float32)
    if scale is not None:
        x = x * scale
    x = x.astype(F16)
    B = x.shape[0]
    per = B // n_cores
    return [np.ascontiguousarray(
        x[c * per:(c + 1) * per].reshape(per * NPIX, CIN).T)
        for c in range(n_cores)]


# ---------------------------------------------------------------- builder

def _build(n_cores):
    import concourse.bacc as bacc
    import concourse.mybir as mybir
    import concourse.tile as tile

    f32 = mybir.dt.float32
    f16 = mybir.dt.float16

    nc = bacc.Bacc("TRN2", target_bir_lowering=False, debug=False,
                   num_devices=n_cores)

    def din(name, shape, dt=f16):
        return nc.dram_tensor(name, list(shape), dt, kind="ExternalInput").ap()

    x_d = din("x_t", (CIN, R))
    xs_d = din("xs_t", (CIN, R))
    wA_d = din("wA", (128, 24 * 128))
    wfc2_d = din("wfc2", (128, 256))
    wfus1_d = din("wfus1", (128, 3 * FUSH))
    wfc3_d = din("wfc3", (FUSH, 1024))
    lay, ncols = _smalls_layout()
    smalls_d = din("smalls", (128, ncols), f32)
    conv_d = []
    for ci, Cc in [(1, LOW), (2, FULL), (3, COUT)]:
        nblk = Cc // 128
        conv_d.append(dict(
            diag=din(f"diag{ci}", (128, nblk * 9 * 128)),
            nblk=nblk))
    out_d = nc.dram_tensor("out_t", [COUT, R], f16, kind="ExternalOutput").ap()

    with tile.TileContext(nc) as tc:
        _emit(nc, tc, mybir, n_cores, x_d, xs_d, wA_d, wfc2_d, wfus1_d,
              wfc3_d, conv_d, smalls_d, lay, ncols, out_d)
    nc.compile()
    return nc


def _emit(nc, tc, mybir, n_cores, x_d, xs_d, wA_d, wfc2_d, wfus1_d,
          wfc3_d, conv_d, smalls_d, lay, ncols, out_d):
    f32 = mybir.dt.float32
    f16 = mybir.dt.float16
    AL = mybir.AluOpType
    AF = mybir.ActivationFunctionType
    inv_n = 1.0 / (n_cores * R)

    class _Pools:
        def __init__(self, tc):
            self.tc = tc
            self.cms = {}
            self.order = []
        def open(self, name, **kw):
            cm = self.tc.tile_pool(name=name, **kw)
            pool = cm.__enter__()
            self.cms[name] = cm
            self.order.append(name)
            return pool
        def close(self, *names):
            names = sorted(names, key=self.order.index, reverse=True)
            for n in names:
                assert n == self.order[-1], (n, self.order)
                self.order.pop()
                self.cms.pop(n).__exit__(None, None, None)
        def close_all(self):
            self.close(*self.order)

    pools = _Pools(tc)

    def pad3(t):
        return t[:].rearrange("p (a b) -> p a b", a=PW)

    # ---------------- persistent small tiles ----------------
    P_pers = pools.open("pers", bufs=1)
    P_tmpv = pools.open("tmpv", bufs=4)
    P_dram = pools.open("dramp", bufs=1, space="DRAM")

    # one DMA for every small constant; everything below is a column slice
    smalls = P_pers.tile([128, ncols], f32, name="smalls", tag="smalls")
    sc = lambda key: smalls[:, lay[key]:lay[key] + 1]

    rs_t = sc("rs")
    bf1a = sc("bf1a")
    bf1b = smalls[0:64, lay["bf1b"]:lay["bf1b"] + 1]
    b3bt = [sc(f"b3b{m}") for m in range(4)]
    b3st = [sc(f"b3s{m}") for m in range(4)]

    dummy_w = P_pers.tile([128, 128], f16, name="dummy_w", tag="dummy_w")
    dummy_rhs = P_pers.tile([128, 512], f16, name="dummy_rhs", tag="dummy_rhs")
    nc.gpsimd.memset(dummy_w[:], 0.0)
    nc.gpsimd.memset(dummy_rhs[:], 0.0)

    bn = []  # bn[ci][blk] = dict(g, be, bb(slices), a, b(tiles))
    for ci in range(3):
        blks = []
        for b in range(conv_d[ci]["nblk"]):
            e = {nm: sc(f"bn{ci}{nm}{b}") for nm in ("g", "be", "bb")}
            e["a"] = P_pers.tile([128, 1], f32, name=f"bn{ci}a{b}", tag=f"bn{ci}a{b}")
            e["b"] = P_pers.tile([128, 1], f32, name=f"bn{ci}b{b}", tag=f"bn{ci}b{b}")
            blks.append(e)
        bn.append(blks)

    wv_t = [[smalls[:, lay[f"wv{ci}{b}"]:lay[f"wv{ci}{b}"] + 9]
             for b in range(conv_d[ci]["nblk"])] for ci in range(3)]

    SLAB = 1024                      # conv slab (PSUM-resident px per step)
    NSLAB = NPIX // SLAB             # 4 slabs per image
    Sp, Qp = [], []
    for ci in range(3):
        Sp.append([P_pers.tile([128, 2 * NSLAB], f32, name=f"Sp{ci}{b}",
                               tag=f"Sp{ci}{b}") for b in range(conv_d[ci]["nblk"])])
        Qp.append([P_pers.tile([128, 2 * NSLAB], f32, name=f"Qp{ci}{b}",
                               tag=f"Qp{ci}{b}") for b in range(conv_d[ci]["nblk"])])
    pack1 = P_pers.tile([128, 2], f32, name="pack1", tag="pack1")
    pack2 = P_pers.tile([128, 4], f32, name="pack2", tag="pack2")
    pack3a = P_pers.tile([128, 4], f32, name="pack3a", tag="pack3a")
    pack3b = P_pers.tile([128, 4], f32, name="pack3b", tag="pack3b")
    gst1 = P_pers.tile([128, 2], f32, name="gst1", tag="gst1")
    gst2 = P_pers.tile([128, 4], f32, name="gst2", tag="gst2")
    gst3a = P_pers.tile([128, 4], f32, name="gst3a", tag="gst3a")
    gst3b = P_pers.tile([128, 4], f32, name="gst3b", tag="gst3b")

    # --------- conv emitter: 8 PE taps + fused DVE tap/combine/stats -----
    # zdst[b] = persistent SBUF tile [128, R]; slab written at
    # [:, img*NPIX + s*SLAB : +SLAB]. After each slab, one queued
    # interleave callback is drained (used to overlap fc2 with conv2).
    FUSED_STT = True     # fuse DVE tap + PSUM-combine + Sp into one stt
    ACT_SQUARE = True     # Qp via ACT Square (v1) vs DVE tensor_tensor_reduce

    def emit_conv(ci, pads, P_cps, P_cacc, P_csq, P_diag, zdst, imgs=(0, 1),
                  interleave=None, blocks=None):
        nblk = conv_d[ci]["nblk"]
        rows = SLAB // HH
        diag_dram = conv_d[ci]["diag"]
        for b in (range(nblk) if blocks is None else blocks):
            dg = P_diag.tile([128, 9 * 128], f16, name="dg", tag="dg")
            nc.sync.dma_start(
                dg[:], diag_dram[:, b * 9 * 128:(b + 1) * 9 * 128])
            for img in imgs:
                p3 = pad3(pads[b][img])
                for s in range(NSLAB):
                    r0 = s * rows
                    ps = P_cps.tile([128, SLAB], f32, name=f"cps{ci}", tag=f"cps{ci}")
                    for ti, t in enumerate(PE_TAPS):
                        di, dj = TAPS[t]
                        rhs = p3[:, 1 + di + r0:1 + di + r0 + rows,
                                 1 + dj:1 + dj + HH]
                        for nn in range(SLAB // 512):
                            rr = nn * (512 // HH)
                            nc.tensor.matmul(
                                ps[:, nn * 512:(nn + 1) * 512],
                                dg[:, t * 128:(t + 1) * 128],
                                rhs[:, rr:rr + (512 // HH), :],
                                start=(ti == 0), stop=(ti == len(PE_TAPS) - 1))
                    slot = img * NSLAB + s
                    col = img * NPIX + s * SLAB
                    zsl = zdst[b][:, col:col + SLAB]
                    di, dj = TAPS[DVE_TAP]
                    tap_ap = p3[:, 1 + di + r0:1 + di + r0 + rows,
                                1 + dj:1 + dj + HH]
                    if FUSED_STT:
                        nc.vector.scalar_tensor_tensor(
                            zsl.rearrange("p (a b) -> p a b", a=rows),
                            tap_ap,
                            wv_t[ci][b][:, DVE_TAP:DVE_TAP + 1],
                            ps[:].rearrange("p (a b) -> p a b", a=rows),
                            op0=AL.mult, op1=AL.add,
                            accum_out=Sp[ci][b][:, slot:slot + 1])
                    else:
                        acc = P_cacc.tile([128, SLAB], f16, name="cacc", tag="cacc")
                        nc.vector.tensor_scalar(
                            acc[:].rearrange("p (a b) -> p a b", a=rows),
                            tap_ap, wv_t[ci][b][:, DVE_TAP:DVE_TAP + 1], None,
                            op0=AL.mult)
                        nc.vector.scalar_tensor_tensor(
                            zsl, acc[:], 0.0, ps[:], op0=AL.bypass, op1=AL.add,
                            accum_out=Sp[ci][b][:, slot:slot + 1])
                    sq = P_csq.tile([128, SLAB], f16, name="sqs", tag="sqs")
                    if ACT_SQUARE:
                        nc.scalar.activation(sq[:], zsl, AF.Square,
                                             accum_out=Qp[ci][b][:, slot:slot + 1])
                    else:
                        nc.vector.tensor_tensor_reduce(
                            sq[:], zsl, zsl, 1.0, 0.0, op0=AL.mult, op1=AL.add,
                            accum_out=Qp[ci][b][:, slot:slot + 1])
                    if interleave:
                        interleave.pop(0)()

    def open_conv_pools(sfx):
        return (pools.open(f"cps{sfx}", bufs=2, space="PSUM"),
                pools.open(f"cacc{sfx}", bufs=2),
                pools.open(f"csq{sfx}", bufs=2),
                pools.open(f"diag{sfx}", bufs=2))

    def close_conv_pools(sfx):
        pools.close(f"diag{sfx}", f"csq{sfx}", f"cacc{sfx}", f"cps{sfx}")

    def bn_math(ci, b, S_ap, Q_ap):
        e = bn[ci][b]
        tt = lambda tag: P_tmpv.tile([128, 1], f32, name=tag, tag=tag)
        m = tt("bnm"); e2 = tt("bne"); m2 = tt("bnm2"); v = tt("bnv")
        sq = tt("bnsq"); iv = tt("bniv"); mb = tt("bnmb"); ab = tt("bnab")
        nc.vector.tensor_scalar(m[:], S_ap, inv_n, None, op0=AL.mult)
        nc.vector.tensor_scalar(e2[:], Q_ap, inv_n, None, op0=AL.mult)
        nc.vector.tensor_tensor(m2[:], m[:], m[:], op=AL.mult)
        nc.vector.tensor_tensor(v[:], e2[:], m2[:], op=AL.subtract)
        nc.vector.tensor_scalar(v[:], v[:], EPS, None, op0=AL.add)
        nc.scalar.activation(sq[:], v[:], AF.Sqrt)
        nc.vector.reciprocal(iv[:], sq[:])
        nc.vector.tensor_tensor(e["a"][:], e["g"], iv[:], op=AL.mult)
        nc.vector.tensor_tensor(mb[:], m[:], e["bb"], op=AL.add)
        nc.vector.tensor_tensor(ab[:], e["a"][:], mb[:], op=AL.mult)
        nc.vector.tensor_tensor(e["b"][:], e["be"], ab[:], op=AL.subtract)

    def allreduce(pack, gst, ncols, tag):
        if n_cores == 1:
            nc.vector.tensor_copy(gst[:], pack[:])
            return
        ib = P_dram.tile([128, ncols], f32, name=f"cc_in{tag}", tag=f"cc_in{tag}")
        ob = P_dram.tile([128, ncols], f32, name=f"cc_out{tag}", tag=f"cc_out{tag}")
        nc.gpsimd.dma_start(ib[:], pack[:])
        nc.gpsimd.collective_compute(
            "AllReduce", AL.add,
            replica_groups=[list(range(n_cores))],
            ins=[ib.opt()], outs=[ob.opt()])
        nc.gpsimd.dma_start(gst[:], ob[:])

    def reduce_stats(pack, cols):
        for i, (ci, b) in enumerate(cols):
            nc.vector.tensor_reduce(pack[:, 2 * i:2 * i + 1], Sp[ci][b][:],
                                    axis=mybir.AxisListType.X, op=AL.add)
            nc.vector.tensor_reduce(pack[:, 2 * i + 1:2 * i + 2], Qp[ci][b][:],
                                    axis=mybir.AxisListType.X, op=AL.add)

    # persistent SBUF activations (fp16)
    P_hf = pools.open("hfp", bufs=1)
    hf1a = P_hf.tile([128, R], f16, name="hf1a", tag="hf1a")
    hf1b = P_hf.tile([64, R], f16, name="hf1b", tag="hf1b")

    # =================== stage A: fc1_low + fc1_full ==================
    P_z12 = pools.open("z12p", bufs=1)
    z1_sb = [P_z12.tile([128, R], f16, name="z1sb", tag="z1sb")]
    z2_sb = [P_z12.tile([128, R], f16, name=f"z2sb{b}", tag=f"z2sb{b}")
             for b in range(2)]
    yl_sb = P_z12.tile([128, R], f16, name="ylsb", tag="ylsb")

    P_pad2 = pools.open("pads2", bufs=1)
    P_pad1 = pools.open("pads1", bufs=1)
    y1p = [P_pad1.tile([128, PAREA], f16, name=f"y1p{i}", tag=f"y1p{i}")
           for i in range(2)]
    y2p = [[P_pad2.tile([128, PAREA], f16, name=f"y2p{b}{i}", tag=f"y2p{b}{i}")
            for i in range(2)] for b in range(2)]
    for t in y1p:
        nc.vector.memset(t[:], 0.0)
    for i in range(2):            # image-major: img0 pads ready first
        for b in range(2):
            nc.gpsimd.memset(y2p[b][i][:], 0.0)

    P_wA = pools.open("wAp", bufs=1)
    P_xk = pools.open("xk", bufs=2)
    P_tmpA = pools.open("tmpA", bufs=2)
    P_psA = pools.open("psA", bufs=2, space="PSUM")
    wA_sb = P_wA.tile([128, 24 * 128], f16, name="wAsb", tag="wAsb")
    wAt = lambda k, m: wA_sb[:, (k * 6 + m) * 128:(k * 6 + m + 1) * 128]
    pairs = [(0, 1, lambda img: y1p[img]),
             (2, 4, lambda img: y2p[0][img]),
             (3, 5, lambda img: y2p[1][img])]
    for ch in range(8):
        img, lrow = ch // 4, (ch % 4) * 16
        xs = []
        for k in range(4):
            xt = P_xk.tile([128, 1024], f16, name=f"xk{k}", tag=f"xk{k}")
            eng = nc.sync if k % 2 == 0 else nc.scalar
            eng.dma_start(
                xt[:], x_d[k * 128:(k + 1) * 128, ch * 1024:(ch + 1) * 1024])
            xs.append(xt)
        if ch == 0:
            nc.sync.dma_start(wA_sb[:], wA_d[:])
        for bm, sm, dest in pairs:
            psB = P_psA.tile([128, 1024], f32, name="psB", tag="psB")
            psS = P_psA.tile([128, 1024], f32, name="psS", tag="psS")
            for k in range(4):
                for nn in range(2):
                    sl = slice(nn * 512, (nn + 1) * 512)
                    nc.tensor.matmul(psB[:, sl], wAt(k, bm), xs[k][:, sl],
                                     start=(k == 0), stop=(k == 3))
                    nc.tensor.matmul(psS[:, sl], wAt(k, sm), xs[k][:, sl],
                                     start=(k == 0), stop=(k == 3))
            tmp = P_tmpA.tile([128, 1024], f16, name="siluA", tag="siluA")
            nc.scalar.activation(tmp[:], psB[:], AF.Silu)
            outap = pad3(dest(img))[:, 1 + lrow:1 + lrow + 16, 1:65]
            nc.vector.scalar_tensor_tensor(
                outap,
                psS[:].rearrange("p (a b) -> p a b", a=16),
                0.0,
                tmp[:].rearrange("p (a b) -> p a b", a=16),
                op0=AL.bypass, op1=AL.add)
    pools.close("psA", "tmpA", "xk", "wAp")
    nc.scalar.dma_start(smalls[:], smalls_d[:])

    # ============ conv1 -> AR1 (overlapped with conv2+fc2) =============
    cpools1 = open_conv_pools("c1")
    emit_conv(0, [y1p], *cpools1, zdst=z1_sb)
    close_conv_pools("c1")
    pools.close("pads1")
    reduce_stats(pack1, [(0, 0)])
    allreduce(pack1, gst1, 2, "a1")

    # fc2 work units, interleaved into conv2's 16 slab iterations
    P_w2 = pools.open("wfc2p", bufs=1)
    P_t2 = pools.open("fc2t", bufs=2)
    P_ps2 = pools.open("psF2", bufs=1, space="PSUM")
    w2_sb = P_w2.tile([128, 256], f16, name="w2sb", tag="w2sb")
    nc.sync.dma_start(w2_sb[:], wfc2_d[:])

    def fc2_chunk(ch):
        sl = slice(ch * 1024, (ch + 1) * 1024)
        z1b = P_t2.tile([128, 1024], f16, name="z1b", tag="z1b")
        nc.scalar.activation(z1b[:], z1_sb[0][:, sl], AF.Relu,
                             bias=bn[0][0]["b"][:], scale=bn[0][0]["a"][:])
        psB = P_ps2.tile([128, 1024], f32, name="ps2B", tag="ps2B")
        psS = P_ps2.tile([128, 1024], f32, name="ps2S", tag="ps2S")
        for nn in range(2):
            s2 = slice(nn * 512, (nn + 1) * 512)
            nc.tensor.matmul(psB[:, s2], w2_sb[:, 0:128], z1b[:, s2],
                             start=True, stop=True)
            nc.tensor.matmul(psS[:, s2], w2_sb[:, 128:256], z1b[:, s2],
                             start=True, stop=True)
        tmp = P_t2.tile([128, 1024], f16, name="silu2", tag="silu2")
        nc.scalar.activation(tmp[:], psB[:], AF.Silu)
        nc.vector.scalar_tensor_tensor(yl_sb[:, sl], psS[:], 0.0, tmp[:],
                                       op0=AL.bypass, op1=AL.add)

    INTERLEAVE_FC2 = True
    if INTERLEAVE_FC2:
        todo = [lambda: bn_math(0, 0, gst1[:, 0:1], gst1[:, 1:2])]
        todo += [lambda ch=ch: fc2_chunk(ch) for ch in range(8)]
        ilv = [lambda: None] * 5 + todo
        ilv += [lambda: None] * (16 - len(ilv))
    else:
        ilv = None

    cpools2 = open_conv_pools("c2")
    emit_conv(1, y2p, *cpools2, zdst=z2_sb, interleave=ilv)
    close_conv_pools("c2")
    if not INTERLEAVE_FC2:
        bn_math(0, 0, gst1[:, 0:1], gst1[:, 1:2])
        for ch in range(8):
            fc2_chunk(ch)
    pools.close("psF2", "fc2t", "wfc2p", "pads2")
    reduce_stats(pack2, [(1, 0), (1, 1)])
    allreduce(pack2, gst2, 4, "a2")
    bn_math(1, 0, gst2[:, 0:1], gst2[:, 1:2])
    bn_math(1, 1, gst2[:, 2:3], gst2[:, 3:4])

    # keep the PE array busy through the AR2 collective so the HAM clock
    # gate stays at 8/8 into fusion/fc3 (a >3.4us PE-idle window would
    # re-throttle to 1.2 GHz for tens of us). Garbage-in, never-read-out.
    P_warm = pools.open("pswarm", bufs=1, space="PSUM")
    wps = P_warm.tile([128, 512], f32, name="wps", tag="wps")
    NWARM = 130
    for i in range(NWARM):
        nc.tensor.matmul(wps[:], dummy_w[:], dummy_rhs[:],
                         start=(i == 0), stop=(i == NWARM - 1),
                         skip_group_check=True)
    pools.close("pswarm")

    # =================== fusion linear 1 -> hf1 (SBUF) ==================
    P_wf1 = pools.open("wfu1", bufs=1)
    P_tf1 = pools.open("fu1t", bufs=3)
    P_psf1 = pools.open("psFu1", bufs=2, space="PSUM")
    wf1_sb = P_wf1.tile([128, 3 * FUSH], f16, name="wf1sb", tag="wf1sb")
    nc.sync.dma_start(wf1_sb[:], wfus1_d[:])
    wf1t = lambda k, m: wf1_sb[:, k * FUSH + m * 128:k * FUSH + m * 128 + (64 if m else 128)]
    zero64 = P_wf1.tile([64, 1024], f16, name="zero64", tag="zero64")
    nc.gpsimd.memset(zero64[:], 0.0)
    for ch in range(8):
        sl = slice(ch * 1024, (ch + 1) * 1024)
        z2b0 = P_tf1.tile([128, 1024], f16, name="z2b0", tag="z2b0")
        z2b1 = P_tf1.tile([128, 1024], f16, name="z2b1", tag="z2b1")
        nc.scalar.activation(z2b0[:], z2_sb[0][:, sl], AF.Relu,
                             bias=bn[1][0]["b"][:], scale=bn[1][0]["a"][:])
        nc.vector.tensor_scalar(z2b1[:], z2_sb[1][:, sl], bn[1][1]["a"][:],
                                bn[1][1]["b"][:], op0=AL.mult, op1=AL.add)
        nc.vector.tensor_scalar(z2b1[:], z2b1[:], 0.0, None, op0=AL.max)
        rhs = [yl_sb[:, sl], z2b0[:], z2b1[:]]
        ps0 = P_psf1.tile([128, 1024], f32, name="psf1a", tag="psf1a")
        ps1 = P_psf1.tile([64, 1024], f32, name="psf1b", tag="psf1b")
        for k in range(3):
            for nn in range(2):
                s2 = slice(nn * 512, (nn + 1) * 512)
                nc.tensor.matmul(ps0[:, s2], wf1t(k, 0), rhs[k][:, s2],
                                 start=(k == 0), stop=(k == 2))
                nc.tensor.matmul(ps1[:, s2], wf1t(k, 1), rhs[k][:, s2],
                                 start=(k == 0), stop=(k == 2))
        nc.scalar.activation(hf1a[:, sl], ps0[:], AF.Relu, bias=bf1a)
        nc.vector.scalar_tensor_tensor(hf1b[:, sl], ps1[:], bf1b, zero64[:],
                                       op0=AL.add, op1=AL.max)
    pools.close("psFu1", "fu1t", "wfu1")
    pools.close("z12p")

    # ============= fc3' + conv3 (block-split stats) + finals ============
    P_z3 = pools.open("z3p", bufs=1)
    z3_sb = [P_z3.tile([128, R], f16, name=f"z3sb{b}", tag=f"z3sb{b}")
             for b in range(4)]
    P_w3 = pools.open("wfc3p", bufs=1)
    P_h3 = pools.open("h3p", bufs=1)
    P_t3 = pools.open("fc3t", bufs=3)
    P_ps3 = pools.open("psF3", bufs=2, space="PSUM")
    P_xc = pools.open("xcp", bufs=3)
    P_fin = pools.open("fint", bufs=3)
    cpools3 = open_conv_pools("c3")
    w3k = [P_w3.tile([128, 1024], f16, name="w3k0", tag="w3k0"),
           P_w3.tile([64, 1024], f16, name="w3k1", tag="w3k1")]
    nc.sync.dma_start(w3k[0][:], wfc3_d[0:128, :])
    nc.sync.dma_start(w3k[1][:], wfc3_d[128:192, :])

    def fin_chunk(b, ch, dve_bn=True):
        rows = slice(b * 128, (b + 1) * 128)
        sl = slice(ch * 2048, (ch + 1) * 2048)
        xc = P_xc.tile([128, 2048], f16, name="xc", tag="xc")
        nc.sync.dma_start(xc[:], xs_d[rows, sl])
        t = P_fin.tile([128, 2048], f16, name="trelu", tag="trelu")
        if dve_bn and ch == 3:   # balance: every 4th BN+ReLU on the DVE
            nc.vector.tensor_scalar(t[:], z3_sb[b][:, sl],
                                    bn[2][b]["a"][:], bn[2][b]["b"][:],
                                    op0=AL.mult, op1=AL.add)
            nc.vector.tensor_scalar(t[:], t[:], 0.0, None, op0=AL.max)
        else:
            nc.scalar.activation(t[:], z3_sb[b][:, sl], AF.Relu,
                                 bias=bn[2][b]["b"][:], scale=bn[2][b]["a"][:])
        ob = P_fin.tile([128, 2048], f16, name="ob", tag="ob")
        nc.vector.tensor_tensor(ob[:], xc[:], t[:], op=AL.add)
        nc.gpsimd.dma_start(out_d[rows, sl], ob[:])

    h3sets = []
    for img in range(2):
        h3 = [P_h3.tile([128, PAREA], f16, name=f"h3p{b}", tag=f"h3p{b}")
              for b in range(4)]
        h3sets.append(h3)
        for t in h3:
            nc.gpsimd.memset(t[:], 0.0)
        for ch in range(8):           # 512-px chunks within image
            r0 = ch * 8
            sl = slice(img * NPIX + ch * 512, img * NPIX + (ch + 1) * 512)
            rhs = [hf1a[:, sl], hf1b[:, sl]]
            for mp in range(4):
                psB = P_ps3.tile([128, 512], f32, name="ps3B", tag="ps3B")
                psS = P_ps3.tile([128, 512], f32, name="ps3S", tag="ps3S")
                for kk in range(2):
                    nc.tensor.matmul(psB[:], w3k[kk][:, mp * 128:(mp + 1) * 128],
                                     rhs[kk], start=(kk == 0), stop=(kk == 1))
                    nc.tensor.matmul(psS[:], w3k[kk][:, (4 + mp) * 128:(5 + mp) * 128],
                                     rhs[kk], start=(kk == 0), stop=(kk == 1))
                tmp = P_t3.tile([128, 512], f16, name="silu3", tag="silu3")
                nc.scalar.activation(tmp[:], psB[:], AF.Silu, bias=b3bt[mp])
                outap = pad3(h3[mp])[:, 1 + r0:1 + r0 + 8, 1:65]
                nc.vector.scalar_tensor_tensor(
                    outap,
                    psS[:].rearrange("p (a b) -> p a b", a=8),
                    b3st[mp],
                    tmp[:].rearrange("p (a b) -> p a b", a=8),
                    op0=AL.add, op1=AL.add)
        if img == 0:
            emit_conv(2, [{0: h3[b]} for b in range(4)], *cpools3,
                      zdst=z3_sb, imgs=(0,))
    # conv3 img1: blocks 0-1, then AR3a fires while blocks 2-3 conv and
    # the finals for blocks 0-1 interleave into their slab loop.
    emit_conv(2, [{1: h3sets[1][b]} for b in range(4)], *cpools3,
              zdst=z3_sb, imgs=(1,), blocks=(0, 1))
    reduce_stats(pack3a, [(2, 0), (2, 1)])
    allreduce(pack3a, gst3a, 4, "a3a")
    # bn_math + the first finals are staggered into conv3 blocks 2-3 via the
    # interleave hooks so their AR3a-gated ops never head-of-line-block the
    # DVE/ACT queues ahead of conv3's own slab work.
    ilv3 = [lambda: None] * 3
    ilv3.append(lambda: bn_math(2, 0, gst3a[:, 0:1], gst3a[:, 1:2]))
    ilv3.append(lambda: bn_math(2, 1, gst3a[:, 2:3], gst3a[:, 3:4]))
    ilv3 += [lambda ch=ch: fin_chunk(0, ch, dve_bn=False) for ch in range(3)]
    emit_conv(2, [{1: h3sets[1][b]} for b in range(4)], *cpools3,
              zdst=z3_sb, imgs=(1,), blocks=(2, 3), interleave=ilv3)
    fin_chunk(0, 3, dve_bn=False)
    for ch in range(4):
        fin_chunk(1, ch, dve_bn=False)
    reduce_stats(pack3b, [(2, 2), (2, 3)])
    allreduce(pack3b, gst3b, 4, "a3b")
    bn_math(2, 2, gst3b[:, 0:1], gst3b[:, 1:2])
    bn_math(2, 3, gst3b[:, 2:3], gst3b[:, 3:4])
    for b in (2, 3):
        for ch in range(4):
            fin_chunk(b, ch)
    pools.close_all()


def _get_built(n_cores):
    if n_cores not in _CACHE:
        _CACHE[n_cores] = _build(n_cores)
    return _CACHE[n_cores]


def make_in_maps(inputs, n_cores):
    shared = _prep_shared(inputs)
    xt = _prep_x(inputs["x"], n_cores)
    rsv = float(np.asarray(inputs["res_scale"]).reshape(-1)[0])
    xst = _prep_x(inputs["x"], n_cores, scale=rsv)
    return [dict(shared, x_t=xt[c], xs_t=xst[c]) for c in range(n_cores)]


def kernel(**inputs):
    from concourse.bass_utils import run_bass_kernel_spmd

    assert int(np.asarray(inputs["H"])) == HH and int(np.asarray(inputs["W"])) == HH
    n_cores = 8
    nc = _get_built(n_cores)
    in_maps = make_in_maps(inputs, n_cores)
    res = run_bass_kernel_spmd(nc, in_maps, core_ids=list(range(n_cores)))
    B = np.asarray(inputs["x"]).shape[0]
    per = B // n_cores
    out = np.empty((B, NPIX, CIN), np.float32)
    for c in range(n_cores):
        out[c * per:(c + 1) * per] = \
            res.results[c]["out_t"].astype(np.float32).T.reshape(per, NPIX, CIN)
    return out


# ------------------------------------------------------------- profiling

def _install_ntff_hook():
    """The agent image's antenv lacks axon_hooks; recreate the NTFF profile
    hook via ctypes on the axon PJRT .so (same ABI as trn_boot's)."""
    import contextlib, ctypes, sys, types
    so = "/opt/axon/libaxon_pjrt.so"
    try:
        import antenv.axon_hooks  # noqa: F401
        return True
    except ImportError:
        pass
    try:
        lib = ctypes.CDLL(so)
    except OSError:
        return False
    if not hasattr(lib, "axon_start_nrt_profile"):
        return False
    lib.axon_start_nrt_profile.argtypes = [
        ctypes.POINTER(ctypes.c_int64), ctypes.c_size_t]
    lib.axon_start_nrt_profile.restype = ctypes.c_int64
    lib.axon_stop_nrt_profile.argtypes = [ctypes.c_char_p]
    lib.axon_stop_nrt_profile.restype = ctypes.c_int64

    @contextlib.contextmanager
    def _hook(output_dir, device_ids):
        import jax
        jax.devices()
        if device_ids:
            ids = (ctypes.c_int64 * len(device_ids))(*device_ids)
            rc = lib.axon_start_nrt_profile(ids, len(device_ids))
        else:
            rc = lib.axon_start_nrt_profile(None, 0)
        if rc != 0:
            raise RuntimeError(f"axon_start_nrt_profile rc={rc}")
        try:
            yield
        finally:
            n = lib.axon_stop_nrt_profile(str(output_dir).encode())
            print(f"profile: {n} ntff file(s) -> {output_dir}", file=sys.stderr)

    mod = types.ModuleType("antenv.axon_hooks")
    mod.get_axon_ntff_profile_hook = lambda: _hook
    mod.set_axon_ntff_profile_hook = lambda h: None
    sys.modules["antenv.axon_hooks"] = mod
    import concourse.bass_utils as bu
    bu.upload_artifacts = lambda tmpdir: f"local:{tmpdir}"
    return True


def benchmark(inputs, iters=2, tmpdir=None):
    """Device-only HW execution time (ns) via neuron-profile NTFF trace."""
    import os, tempfile
    from concourse.bass_utils import run_bass_kernel_spmd

    if not _install_ntff_hook():
        raise RuntimeError("NTFF profiling unavailable")
    if tmpdir:
        os.makedirs(tmpdir, exist_ok=True)
    n_cores = 8
    nc = _get_built(n_cores)
    in_maps = make_in_maps(inputs, n_cores)
    times = []
    for i in range(max(1, min(iters, 3))):
        td = tempfile.mkdtemp(dir=tmpdir) if tmpdir else None
        res = run_bass_kernel_spmd(nc, in_maps, core_ids=list(range(n_cores)),
                                   trace=True, tmpdir=td)
        if res.exec_time_ns is not None:
            times.append(res.exec_time_ns)
    if not times:
        raise RuntimeError("no exec_time_ns from traced runs")
    return min(times)


# revision 33
# speedup vs baseline: 1.0429x; 1.0212x over previous
"""Trainium2 Bass kernel for nn_ConvLinearLayer (KAN-style conv-linear block).

Strategy
--------
Data-parallel over batch: 16 images -> 8 cores x 2 images. All activations
live on-chip in transposed layout [channels(partitions), pixels(free)], so
GEMMs (PE, fp16), depthwise 3x3 convs (8 PE diag-matmul taps + 1 fused DVE
tap), BN stats (accumulator outputs) and BN-apply+ReLU (ACT, per-partition
scale/bias) all hit their natural axes. Train-mode BN needs global batch
stats -> three tiny AllReduces (per-channel sum/sumsq), each overlapped with
surrounding compute (fc2 is interleaved into conv2's slab loop).

All matmul operands are fp16 (fp32 PSUM accumulation, fp32 BN statistics,
fp32 output): fp32 moving operands stream at ~half rate through the PE
array, fp16 streams at full rate with ample mantissa for this tolerance.
All conv outputs stay SBUF-resident; weights are host-pre-tiled so every
DMA is a single contiguous 2D transfer.

Host-side precompute: input/weight transposes + fp16 casts, spline-weight
sum (sum_k sw[:,:,k]/K == one GEMM), channel_scale folded into fus_w1,
fus_w2+b2 folded into fc3 (W3_eff = W3 @ W2, b3_eff = W3 @ b2), conv-bias
folded into the BN affine.
"""

import numpy as np

F16 = np.dtype(np.float16)

K_SPLINE = 10
EPS = 1e-5
HH = 64
PW = 66           # padded row stride (64 + 2 zero border)
PAREA = PW * PW   # 4356
NPIX = HH * HH    # 4096 pixels per image
R = 2 * NPIX      # rows per core (2 images)
CIN = 512
LOW = 128
FULL = 256
CAT = 384
FUSH = 192
COUT = 512

TAPS = [(di, dj) for di in (-1, 0, 1) for dj in (-1, 0, 1)]
DVE_TAP = 0                           # fused into the PSUM-combine stt
PE_TAPS = [t for t in range(9) if t != DVE_TAP]
NBLKS = [1, 2, 4]

_CACHE = {}


def _smalls_layout():
    """Column layout of the packed [128, N] fp32 constants tensor."""
    col = 0
    lay = {}
    lay["rs"] = col; col += 1
    for ci, nblk in enumerate(NBLKS):
        for b in range(nblk):
            for nm in ("g", "be", "bb"):
                lay[f"bn{ci}{nm}{b}"] = col; col += 1
    for ci, nblk in enumerate(NBLKS):
        for b in range(nblk):
            lay[f"wv{ci}{b}"] = col; col += 9
    lay["bf1a"] = col; col += 1
    lay["bf1b"] = col; col += 1
    for m in range(4):
        lay[f"b3b{m}"] = col; col += 1
    for m in range(4):
        lay[f"b3s{m}"] = col; col += 1
    return lay, col


# ---------------------------------------------------------------- host prep

def _prep_shared(inp):
    """All non-x device tensors (replicated across cores), as numpy 2D."""
    f = lambda a: np.ascontiguousarray(np.asarray(a, dtype=np.float32))
    h = lambda a: np.ascontiguousarray(
        np.asarray(a, dtype=np.float32).astype(F16))
    sws = lambda sw: np.asarray(sw, np.float64).sum(-1) / K_SPLINE

    fc1_low_bw = f(inp["fc1_low_bw"]); s1l = f(sws(inp["fc1_low_sw"]))
    fc1_full_bw = f(inp["fc1_full_bw"]); s1f = f(sws(inp["fc1_full_sw"]))
    fc2_bw = f(inp["fc2_low_bw"]); s2 = f(sws(inp["fc2_low_sw"]))
    fc3_bw = f(inp["fc3_bw"]); s3 = f(sws(inp["fc3_sw"]))
    w1 = f(inp["fus_w1"]); b1 = f(inp["fus_b1"])
    w2 = f(inp["fus_w2"]); b2 = f(inp["fus_b2"])
    cs = f(inp["channel_scale"])

    d = {}
    # stage A lhsT tiles (k,m) of [512, 768] packed as [128, 24*128]
    # m-blocks: [lowb, lows, fullb0, fullb1, fulls0, fulls1]
    wA = np.concatenate([fc1_low_bw.T, s1l.T, fc1_full_bw.T, s1f.T], axis=1)
    wAt = np.empty((128, 24 * 128), np.float32)
    for k in range(4):
        for m in range(6):
            wAt[:, (k * 6 + m) * 128:(k * 6 + m + 1) * 128] = \
                wA[k * 128:(k + 1) * 128, m * 128:(m + 1) * 128]
    d["wA"] = h(wAt)
    d["wfc2"] = h(np.concatenate([fc2_bw.T, s2.T], axis=1))      # [128, 256]
    wfus1 = (w1 * cs[None, :]).T                                 # [384, 192]
    wf1t = np.empty((128, 3 * FUSH), np.float32)
    for k in range(3):
        wf1t[:, k * FUSH:(k + 1) * FUSH] = wfus1[k * 128:(k + 1) * 128, :]
    d["wfus1"] = h(wf1t)                                         # [128, 576]
    w3b = fc3_bw @ w2                                            # [512, 192]
    w3s = s3 @ w2
    d["wfc3"] = h(np.concatenate([w3b.T, w3s.T], axis=1))        # [192, 1024]
    b3b = (fc3_bw @ b2).reshape(-1)
    b3s = (s3 @ b2).reshape(-1)

    lay, ncols = _smalls_layout()
    sm = np.zeros((128, ncols), np.float32)
    sm[:, lay["rs"]] = float(np.asarray(inp["res_scale"]).reshape(-1)[0])
    sm[0:128, lay["bf1a"]] = b1[0:128]
    sm[0:64, lay["bf1b"]] = b1[128:192]
    for m in range(4):
        sm[:, lay[f"b3b{m}"]] = b3b[m * 128:(m + 1) * 128]
        sm[:, lay[f"b3s{m}"]] = b3s[m * 128:(m + 1) * 128]

    # depthwise convs: diag tiles (b,t) packed as [128, nblk*9*128]
    for ci, (wname, gname, bname, bbname, Cc) in enumerate([
            ("dw1_w", "dw1_g", "dw1_beta", "dw1_b", LOW),
            ("dw2_w", "dw2_g", "dw2_beta", "dw2_b", FULL),
            ("dw3_w", "dw3_g", "dw3_beta", "dw3_b", COUT)]):
        w = f(inp[wname]).reshape(Cc, 9)                          # [C, taps]
        g = f(inp[gname]).reshape(-1)
        be = f(inp[bname]).reshape(-1)
        bb = f(inp[bbname]).reshape(-1)
        nblk = Cc // 128
        diag = np.zeros((128, nblk * 9 * 128), np.float32)
        for b in range(nblk):
            rows = slice(b * 128, (b + 1) * 128)
            for t in range(9):
                c0 = (b * 9 + t) * 128
                diag[:, c0:c0 + 128] = np.diag(w[rows, t])
            c = lay[f"wv{ci}{b}"]
            sm[:, c:c + 9] = w[rows]
            sm[:, lay[f"bn{ci}g{b}"]] = g[rows]
            sm[:, lay[f"bn{ci}be{b}"]] = be[rows]
            sm[:, lay[f"bn{ci}bb{b}"]] = bb[rows]
        d[f"diag{ci+1}"] = h(diag)
    d["smalls"] = np.ascontiguousarray(sm)
    return d


def _prep_x(x, n_cores, scale=None):
    """Per-core transposed fp16 shards [512, 8192]."""
    x = np.asarray(x, np.float32)
    if scale is not None:
        x = x * scale
    x = x.astype(F16)
    B = x.shape[0]
    per = B // n_cores
    return [np.ascontiguousarray(
        x[c * per:(c + 1) * per].reshape(per * NPIX, CIN).T)
        for c in range(n_cores)]


# ---------------------------------------------------------------- builder

def _build(n_cores):
    import concourse.bacc as bacc
    import concourse.mybir as mybir
    import concourse.tile as tile

    f32 = mybir.dt.float32
    f16 = mybir.dt.float16

    nc = bacc.Bacc("TRN2", target_bir_lowering=False, debug=False,
                   num_devices=n_cores)

    def din(name, shape, dt=f16):
        return nc.dram_tensor(name, list(shape), dt, kind="ExternalInput").ap()

    x_d = din("x_t", (CIN, R))
    xs_d = din("xs_t", (CIN, R))
    wA_d = din("wA", (128, 24 * 128))
    wfc2_d = din("wfc2", (128, 256))
    wfus1_d = din("wfus1", (128, 3 * FUSH))
    wfc3_d = din("wfc3", (FUSH, 1024))
    lay, ncols = _smalls_layout()
    smalls_d = din("smalls", (128, ncols), f32)
    conv_d = []
    for ci, Cc in [(1, LOW), (2, FULL), (3, COUT)]:
        nblk = Cc // 128
        conv_d.append(dict(
            diag=din(f"diag{ci}", (128, nblk * 9 * 128)),
            nblk=nblk))
    out_d = nc.dram_tensor("out_t", [COUT, R], f16, kind="ExternalOutput").ap()

    with tile.TileContext(nc) as tc:
        _emit(nc, tc, mybir, n_cores, x_d, xs_d, wA_d, wfc2_d, wfus1_d,
              wfc3_d, conv_d, smalls_d, lay, ncols, out_d)
    nc.compile()
    return nc


def _emit(nc, tc, mybir, n_cores, x_d, xs_d, wA_d, wfc2_d, wfus1_d,
          wfc3_d, conv_d, smalls_d, lay, ncols, out_d):
    f32 = mybir.dt.float32
    f16 = mybir.dt.float16
    AL = mybir.AluOpType
    AF = mybir.ActivationFunctionType
    inv_n = 1.0 / (n_cores * R)

    class _Pools:
        def __init__(self, tc):
            self.tc = tc
            self.cms = {}
            self.order = []
        def open(self, name, **kw):
            cm = self.tc.tile_pool(name=name, **kw)
            pool = cm.__enter__()
            self.cms[name] = cm
            self.order.append(name)
            return pool
        def close(self, *names):
            names = sorted(names, key=self.order.index, reverse=True)
            for n in names:
                assert n == self.order[-1], (n, self.order)
                self.order.pop()
                self.cms.pop(n).__exit__(None, None, None)
        def close_all(self):
            self.close(*self.order)

    pools = _Pools(tc)

    def pad3(t):
        return t[:].rearrange("p (a b) -> p a b", a=PW)

    # ---------------- persistent small tiles ----------------
    P_pers = pools.open("pers", bufs=1)
    P_tmpv = pools.open("tmpv", bufs=4)
    P_dram = pools.open("dramp", bufs=1, space="DRAM")

    # one DMA for every small constant; everything below is a column slice
    smalls = P_pers.tile([128, ncols], f32, name="smalls", tag="smalls")
    sc = lambda key: smalls[:, lay[key]:lay[key] + 1]

    rs_t = sc("rs")
    bf1a = sc("bf1a")
    bf1b = smalls[0:64, lay["bf1b"]:lay["bf1b"] + 1]
    b3bt = [sc(f"b3b{m}") for m in range(4)]
    b3st = [sc(f"b3s{m}") for m in range(4)]

    dummy_w = P_pers.tile([128, 128], f16, name="dummy_w", tag="dummy_w")
    dummy_rhs = P_pers.tile([128, 512], f16, name="dummy_rhs", tag="dummy_rhs")
    nc.gpsimd.memset(dummy_w[:], 0.0)
    nc.gpsimd.memset(dummy_rhs[:], 0.0)

    bn = []  # bn[ci][blk] = dict(g, be, bb(slices), a, b(tiles))
    for ci in range(3):
        blks = []
        for b in range(conv_d[ci]["nblk"]):
            e = {nm: sc(f"bn{ci}{nm}{b}") for nm in ("g", "be", "bb")}
            e["a"] = P_pers.tile([128, 1], f32, name=f"bn{ci}a{b}", tag=f"bn{ci}a{b}")
            e["b"] = P_pers.tile([128, 1], f32, name=f"bn{ci}b{b}", tag=f"bn{ci}b{b}")
            blks.append(e)
        bn.append(blks)

    wv_t = [[smalls[:, lay[f"wv{ci}{b}"]:lay[f"wv{ci}{b}"] + 9]
             for b in range(conv_d[ci]["nblk"])] for ci in range(3)]

    SLAB = 1024                      # conv slab (PSUM-resident px per step)
    NSLAB = NPIX // SLAB             # 4 slabs per image
    Sp, Qp = [], []
    for ci in range(3):
        Sp.append([P_pers.tile([128, 2 * NSLAB], f32, name=f"Sp{ci}{b}",
                               tag=f"Sp{ci}{b}") for b in range(conv_d[ci]["nblk"])])
        Qp.append([P_pers.tile([128, 2 * NSLAB], f32, name=f"Qp{ci}{b}",
                               tag=f"Qp{ci}{b}") for b in range(conv_d[ci]["nblk"])])
    pack1 = P_pers.tile([128, 2], f32, name="pack1", tag="pack1")
    pack2 = P_pers.tile([128, 4], f32, name="pack2", tag="pack2")
    pack3a = P_pers.tile([128, 4], f32, name="pack3a", tag="pack3a")
    pack3b = P_pers.tile([128, 4], f32, name="pack3b", tag="pack3b")
    gst1 = P_pers.tile([128, 2], f32, name="gst1", tag="gst1")
    gst2 = P_pers.tile([128, 4], f32, name="gst2", tag="gst2")
    gst3a = P_pers.tile([128, 4], f32, name="gst3a", tag="gst3a")
    gst3b = P_pers.tile([128, 4], f32, name="gst3b", tag="gst3b")

    # --------- conv emitter: 8 PE taps + fused DVE tap/combine/stats -----
    # zdst[b] = persistent SBUF tile [128, R]; slab written at
    # [:, img*NPIX + s*SLAB : +SLAB]. After each slab, one queued
    # interleave callback is drained (used to overlap fc2 with conv2).
    FUSED_STT = True     # fuse DVE tap + PSUM-combine + Sp into one stt
    ACT_SQUARE = True     # Qp via ACT Square (v1) vs DVE tensor_tensor_reduce

    def emit_conv(ci, pads, P_cps, P_cacc, P_csq, P_diag, zdst, imgs=(0, 1),
                  interleave=None, blocks=None):
        nblk = conv_d[ci]["nblk"]
        rows = SLAB // HH
        diag_dram = conv_d[ci]["diag"]
        for b in (range(nblk) if blocks is None else blocks):
            dg = P_diag.tile([128, 9 * 128], f16, name="dg", tag="dg")
            nc.sync.dma_start(
                dg[:], diag_dram[:, b * 9 * 128:(b + 1) * 9 * 128])
            for img in imgs:
                p3 = pad3(pads[b][img])
                for s in range(NSLAB):
                    r0 = s * rows
                    ps = P_cps.tile([128, SLAB], f32, name=f"cps{ci}", tag=f"cps{ci}")
                    for ti, t in enumerate(PE_TAPS):
                        di, dj = TAPS[t]
                        rhs = p3[:, 1 + di + r0:1 + di + r0 + rows,
                                 1 + dj:1 + dj + HH]
                        for nn in range(SLAB // 512):
                            rr = nn * (512 // HH)
                            nc.tensor.matmul(
                                ps[:, nn * 512:(nn + 1) * 512],
                                dg[:, t * 128:(t + 1) * 128],
                                rhs[:, rr:rr + (512 // HH), :],
                                start=(ti == 0), stop=(ti == len(PE_TAPS) - 1))
                    slot = img * NSLAB + s
                    col = img * NPIX + s * SLAB
                    zsl = zdst[b][:, col:col + SLAB]
                    di, dj = TAPS[DVE_TAP]
                    tap_ap = p3[:, 1 + di + r0:1 + di + r0 + rows,
                                1 + dj:1 + dj + HH]
                    if FUSED_STT:
                        nc.vector.scalar_tensor_tensor(
                            zsl.rearrange("p (a b) -> p a b", a=rows),
                            tap_ap,
                            wv_t[ci][b][:, DVE_TAP:DVE_TAP + 1],
                            ps[:].rearrange("p (a b) -> p a b", a=rows),
                            op0=AL.mult, op1=AL.add,
                            accum_out=Sp[ci][b][:, slot:slot + 1])
                    else:
                        acc = P_cacc.tile([128, SLAB], f16, name="cacc", tag="cacc")
                        nc.vector.tensor_scalar(
                            acc[:].rearrange("p (a b) -> p a b", a=rows),
                            tap_ap, wv_t[ci][b][:, DVE_TAP:DVE_TAP + 1], None,
                            op0=AL.mult)
                        nc.vector.scalar_tensor_tensor(
                            zsl, acc[:], 0.0, ps[:], op0=AL.bypass, op1=AL.add,
                            accum_out=Sp[ci][b][:, slot:slot + 1])
                    sq = P_csq.tile([128, SLAB], f16, name="sqs", tag="sqs")
                    if ACT_SQUARE:
                        nc.scalar.activation(sq[:], zsl, AF.Square,
                                             accum_out=Qp[ci][b][:, slot:slot + 1])
                    else:
                        nc.vector.tensor_tensor_reduce(
                            sq[:], zsl, zsl, 1.0, 0.0, op0=AL.mult, op1=AL.add,
                            accum_out=Qp[ci][b][:, slot:slot + 1])
                    if interleave:
                        interleave.pop(0)()

    def open_conv_pools(sfx):
        return (pools.open(f"cps{sfx}", bufs=2, space="PSUM"),
                pools.open(f"cacc{sfx}", bufs=2),
                pools.open(f"csq{sfx}", bufs=2),
                pools.open(f"diag{sfx}", bufs=2))

    def close_conv_pools(sfx):
        pools.close(f"diag{sfx}", f"csq{sfx}", f"cacc{sfx}", f"cps{sfx}")

    def bn_math(ci, b, S_ap, Q_ap):
        e = bn[ci][b]
        tt = lambda tag: P_tmpv.tile([128, 1], f32, name=tag, tag=tag)
        m = tt("bnm"); e2 = tt("bne"); m2 = tt("bnm2"); v = tt("bnv")
        sq = tt("bnsq"); iv = tt("bniv"); mb = tt("bnmb"); ab = tt("bnab")
        nc.vector.tensor_scalar(m[:], S_ap, inv_n, None, op0=AL.mult)
        nc.vector.tensor_scalar(e2[:], Q_ap, inv_n, None, op0=AL.mult)
        nc.vector.tensor_tensor(m2[:], m[:], m[:], op=AL.mult)
        nc.vector.tensor_tensor(v[:], e2[:], m2[:], op=AL.subtract)
        nc.vector.tensor_scalar(v[:], v[:], EPS, None, op0=AL.add)
        nc.scalar.activation(sq[:], v[:], AF.Sqrt)
        nc.vector.reciprocal(iv[:], sq[:])
        nc.vector.tensor_tensor(e["a"][:], e["g"], iv[:], op=AL.mult)
        nc.vector.tensor_tensor(mb[:], m[:], e["bb"], op=AL.add)
        nc.vector.tensor_tensor(ab[:], e["a"][:], mb[:], op=AL.mult)
        nc.vector.tensor_tensor(e["b"][:], e["be"], ab[:], op=AL.subtract)

    def allreduce(pack, gst, ncols, tag):
        if n_cores == 1:
            nc.vector.tensor_copy(gst[:], pack[:])
            return
        ib = P_dram.tile([128, ncols], f32, name=f"cc_in{tag}", tag=f"cc_in{tag}")
        ob = P_dram.tile([128, ncols], f32, name=f"cc_out{tag}", tag=f"cc_out{tag}")
        nc.gpsimd.dma_start(ib[:], pack[:])
        nc.gpsimd.collective_compute(
            "AllReduce", AL.add,
            replica_groups=[list(range(n_cores))],
            ins=[ib.opt()], outs=[ob.opt()])
        nc.gpsimd.dma_start(gst[:], ob[:])

    def reduce_stats(pack, cols):
        for i, (ci, b) in enumerate(cols):
            nc.vector.tensor_reduce(pack[:, 2 * i:2 * i + 1], Sp[ci][b][:],
                                    axis=mybir.AxisListType.X, op=AL.add)
            nc.vector.tensor_reduce(pack[:, 2 * i + 1:2 * i + 2], Qp[ci][b][:],
                                    axis=mybir.AxisListType.X, op=AL.add)

    # persistent SBUF activations (fp16)
    P_hf = pools.open("hfp", bufs=1)
    hf1a = P_hf.tile([128, R], f16, name="hf1a", tag="hf1a")
    hf1b = P_hf.tile([64, R], f16, name="hf1b", tag="hf1b")

    # =================== stage A: fc1_low + fc1_full ==================
    P_z12 = pools.open("z12p", bufs=1)
    z1_sb = [P_z12.tile([128, R], f16, name="z1sb", tag="z1sb")]
    z2_sb = [P_z12.tile([128, R], f16, name=f"z2sb{b}", tag=f"z2sb{b}")
             for b in range(2)]
    yl_sb = P_z12.tile([128, R], f16, name="ylsb", tag="ylsb")

    P_pad2 = pools.open("pads2", bufs=1)
    P_pad1 = pools.open("pads1", bufs=1)
    y1p = [P_pad1.tile([128, PAREA], f16, name=f"y1p{i}", tag=f"y1p{i}")
           for i in range(2)]
    y2p = [[P_pad2.tile([128, PAREA], f16, name=f"y2p{b}{i}", tag=f"y2p{b}{i}")
            for i in range(2)] for b in range(2)]
    for t in y1p:
        nc.vector.memset(t[:], 0.0)
    for i in range(2):            # image-major: img0 pads ready first
        for b in range(2):
            nc.gpsimd.memset(y2p[b][i][:], 0.0)

    P_wA = pools.open("wAp", bufs=1)
    P_xk = pools.open("xk", bufs=2)
    P_tmpA = pools.open("tmpA", bufs=2)
    P_psA = pools.open("psA", bufs=2, space="PSUM")
    wA_sb = P_wA.tile([128, 24 * 128], f16, name="wAsb", tag="wAsb")
    wAt = lambda k, m: wA_sb[:, (k * 6 + m) * 128:(k * 6 + m + 1) * 128]
    pairs = [(0, 1, lambda img: y1p[img]),
             (2, 4, lambda img: y2p[0][img]),
             (3, 5, lambda img: y2p[1][img])]
    for ch in range(8):
        img, lrow = ch // 4, (ch % 4) * 16
        xs = []
        for k in range(4):
            xt = P_xk.tile([128, 1024], f16, name=f"xk{k}", tag=f"xk{k}")
            eng = nc.sync if k % 2 == 0 else nc.scalar
            eng.dma_start(
                xt[:], x_d[k * 128:(k + 1) * 128, ch * 1024:(ch + 1) * 1024])
            xs.append(xt)
        if ch == 0:
            nc.sync.dma_start(wA_sb[:], wA_d[:])
        for bm, sm, dest in pairs:
            psB = P_psA.tile([128, 1024], f32, name="psB", tag="psB")
            psS = P_psA.tile([128, 1024], f32, name="psS", tag="psS")
            for k in range(4):
                for nn in range(2):
                    sl = slice(nn * 512, (nn + 1) * 512)
                    nc.tensor.matmul(psB[:, sl], wAt(k, bm), xs[k][:, sl],
                                     start=(k == 0), stop=(k == 3))
                    nc.tensor.matmul(psS[:, sl], wAt(k, sm), xs[k][:, sl],
                                     start=(k == 0), stop=(k == 3))
            tmp = P_tmpA.tile([128, 1024], f16, name="siluA", tag="siluA")
            nc.scalar.activation(tmp[:], psB[:], AF.Silu)
            outap = pad3(dest(img))[:, 1 + lrow:1 + lrow + 16, 1:65]
            nc.vector.scalar_tensor_tensor(
                outap,
                psS[:].rearrange("p (a b) -> p a b", a=16),
                0.0,
                tmp[:].rearrange("p (a b) -> p a b", a=16),
                op0=AL.bypass, op1=AL.add)
    pools.close("psA", "tmpA", "xk", "wAp")
    nc.scalar.dma_start(smalls[:], smalls_d[:])

    # ============ conv1 -> AR1 (overlapped with conv2+fc2) =============
    cpools1 = open_conv_pools("c1")
    emit_conv(0, [y1p], *cpools1, zdst=z1_sb)
    close_conv_pools("c1")
    pools.close("pads1")
    reduce_stats(pack1, [(0, 0)])
    allreduce(pack1, gst1, 2, "a1")

    # fc2 work units, interleaved into conv2's 16 slab iterations
    P_w2 = pools.open("wfc2p", bufs=1)
    P_t2 = pools.open("fc2t", bufs=2)
    P_ps2 = pools.open("psF2", bufs=1, space="PSUM")
    w2_sb = P_w2.tile([128, 256], f16, name="w2sb", tag="w2sb")
    nc.sync.dma_start(w2_sb[:], wfc2_d[:])

    def fc2_chunk(ch):
        sl = slice(ch * 1024, (ch + 1) * 1024)
        z1b = P_t2.tile([128, 1024], f16, name="z1b", tag="z1b")
        nc.scalar.activation(z1b[:], z1_sb[0][:, sl], AF.Relu,
                             bias=bn[0][0]["b"][:], scale=bn[0][0]["a"][:])
        psB = P_ps2.tile([128, 1024], f32, name="ps2B", tag="ps2B")
        psS = P_ps2.tile([128, 1024], f32, name="ps2S", tag="ps2S")
        for nn in range(2):
            s2 = slice(nn * 512, (nn + 1) * 512)
            nc.tensor.matmul(psB[:, s2], w2_sb[:, 0:128], z1b[:, s2],
                             start=True, stop=True)
            nc.tensor.matmul(psS[:, s2], w2_sb[:, 128:256], z1b[:, s2],
                             start=True, stop=True)
        tmp = P_t2.tile([128, 1024], f16, name="silu2", tag="silu2")
        nc.scalar.activation(tmp[:], psB[:], AF.Silu)
        nc.vector.scalar_tensor_tensor(yl_sb[:, sl], psS[:], 0.0, tmp[:],
                                       op0=AL.bypass, op1=AL.add)

    INTERLEAVE_FC2 = True
    if INTERLEAVE_FC2:
        todo = [lambda: bn_math(0, 0, gst1[:, 0:1], gst1[:, 1:2])]
        todo += [lambda ch=ch: fc2_chunk(ch) for ch in range(8)]
        ilv = [lambda: None] * 5 + todo
        ilv += [lambda: None] * (16 - len(ilv))
    else:
        ilv = None

    cpools2 = open_conv_pools("c2")
    emit_conv(1, y2p, *cpools2, zdst=z2_sb, interleave=ilv)
    close_conv_pools("c2")
    if not INTERLEAVE_FC2:
        bn_math(0, 0, gst1[:, 0:1], gst1[:, 1:2])
        for ch in range(8):
            fc2_chunk(ch)
    pools.close("psF2", "fc2t", "wfc2p", "pads2")
    reduce_stats(pack2, [(1, 0), (1, 1)])
    allreduce(pack2, gst2, 4, "a2")
    bn_math(1, 0, gst2[:, 0:1], gst2[:, 1:2])
    bn_math(1, 1, gst2[:, 2:3], gst2[:, 3:4])

    # keep the PE array busy through the AR2 collective so the HAM clock
    # gate stays at 8/8 into fusion/fc3 (a >3.4us PE-idle window would
    # re-throttle to 1.2 GHz for tens of us). Garbage-in, never-read-out.
    P_warm = pools.open("pswarm", bufs=1, space="PSUM")
    wps = P_warm.tile([128, 512], f32, name="wps", tag="wps")
    NWARM = 130
    for i in range(NWARM):
        nc.tensor.matmul(wps[:], dummy_w[:], dummy_rhs[:],
                         start=(i == 0), stop=(i == NWARM - 1),
                         skip_group_check=True)
    pools.close("pswarm")

    # =================== fusion linear 1 -> hf1 (SBUF) ==================
    P_wf1 = pools.open("wfu1", bufs=1)
    P_tf1 = pools.open("fu1t", bufs=3)
    P_psf1 = pools.open("psFu1", bufs=2, space="PSUM")
    wf1_sb = P_wf1.tile([128, 3 * FUSH], f16, name="wf1sb", tag="wf1sb")
    nc.sync.dma_start(wf1_sb[:], wfus1_d[:])
    wf1t = lambda k, m: wf1_sb[:, k * FUSH + m * 128:k * FUSH + m * 128 + (64 if m else 128)]
    zero64 = P_wf1.tile([64, 1024], f16, name="zero64", tag="zero64")
    nc.gpsimd.memset(zero64[:], 0.0)
    for ch in range(8):
        sl = slice(ch * 1024, (ch + 1) * 1024)
        z2b0 = P_tf1.tile([128, 1024], f16, name="z2b0", tag="z2b0")
        z2b1 = P_tf1.tile([128, 1024], f16, name="z2b1", tag="z2b1")
        nc.scalar.activation(z2b0[:], z2_sb[0][:, sl], AF.Relu,
                             bias=bn[1][0]["b"][:], scale=bn[1][0]["a"][:])
        nc.vector.tensor_scalar(z2b1[:], z2_sb[1][:, sl], bn[1][1]["a"][:],
                                bn[1][1]["b"][:], op0=AL.mult, op1=AL.add)
        nc.vector.tensor_scalar(z2b1[:], z2b1[:], 0.0, None, op0=AL.max)
        rhs = [yl_sb[:, sl], z2b0[:], z2b1[:]]
        ps0 = P_psf1.tile([128, 1024], f32, name="psf1a", tag="psf1a")
        ps1 = P_psf1.tile([64, 1024], f32, name="psf1b", tag="psf1b")
        for k in range(3):
            for nn in range(2):
                s2 = slice(nn * 512, (nn + 1) * 512)
                nc.tensor.matmul(ps0[:, s2], wf1t(k, 0), rhs[k][:, s2],
                                 start=(k == 0), stop=(k == 2))
                nc.tensor.matmul(ps1[:, s2], wf1t(k, 1), rhs[k][:, s2],
                                 start=(k == 0), stop=(k == 2))
        nc.scalar.activation(hf1a[:, sl], ps0[:], AF.Relu, bias=bf1a)
        nc.vector.scalar_tensor_tensor(hf1b[:, sl], ps1[:], bf1b, zero64[:],
                                       op0=AL.add, op1=AL.max)
    pools.close("psFu1", "fu1t", "wfu1")
    pools.close("z12p")

    # ============= fc3' + conv3 (block-split stats) + finals ============
    P_z3 = pools.open("z3p", bufs=1)
    z3_sb = [P_z3.tile([128, R], f16, name=f"z3sb{b}", tag=f"z3sb{b}")
             for b in range(4)]
    P_w3 = pools.open("wfc3p", bufs=1)
    P_h3 = pools.open("h3p", bufs=1)
    P_t3 = pools.open("fc3t", bufs=3)
    P_ps3 = pools.open("psF3", bufs=2, space="PSUM")
    P_xc = pools.open("xcp", bufs=3)
    P_fin = pools.open("fint", bufs=3)
    cpools3 = open_conv_pools("c3")
    w3k = [P_w3.tile([128, 1024], f16, name="w3k0", tag="w3k0"),
           P_w3.tile([64, 1024], f16, name="w3k1", tag="w3k1")]
    nc.sync.dma_start(w3k[0][:], wfc3_d[0:128, :])
    nc.sync.dma_start(w3k[1][:], wfc3_d[128:192, :])

    def fin_chunk(b, ch, dve_bn=True):
        rows = slice(b * 128, (b + 1) * 128)
        sl = slice(ch * 2048, (ch + 1) * 2048)
        xc = P_xc.tile([128, 2048], f16, name="xc", tag="xc")
        nc.scalar.dma_start(xc[:], xs_d[rows, sl])
        t = P_fin.tile([128, 2048], f16, name="trelu", tag="trelu")
        if dve_bn and ch == 3:   # balance: every 4th BN+ReLU on the DVE
            nc.vector.tensor_scalar(t[:], z3_sb[b][:, sl],
                                    bn[2][b]["a"][:], bn[2][b]["b"][:],
                                    op0=AL.mult, op1=AL.add)
            nc.vector.tensor_scalar(t[:], t[:], 0.0, None, op0=AL.max)
        else:
            nc.scalar.activation(t[:], z3_sb[b][:, sl], AF.Relu,
                                 bias=bn[2][b]["b"][:], scale=bn[2][b]["a"][:])
        ob = P_fin.tile([128, 2048], f16, name="ob", tag="ob")
        nc.vector.tensor_tensor(ob[:], xc[:], t[:], op=AL.add)
        nc.sync.dma_start(out_d[rows, sl], ob[:])

    h3sets = []
    for img in range(2):
        h3 = [P_h3.tile([128, PAREA], f16, name=f"h3p{b}", tag=f"h3p{b}")
              for b in range(4)]
        h3sets.append(h3)
        for t in h3:
            nc.gpsimd.memset(t[:], 0.0)
        for ch in range(8):           # 512-px chunks within image
            r0 = ch * 8
            sl = slice(img * NPIX + ch * 512, img * NPIX + (ch + 1) * 512)
            rhs = [hf1a[:, sl], hf1b[:, sl]]
            for mp in range(4):
                psB = P_ps3.tile([128, 512], f32, name="ps3B", tag="ps3B")
                psS = P_ps3.tile([128, 512], f32, name="ps3S", tag="ps3S")
                for kk in range(2):
                    nc.tensor.matmul(psB[:], w3k[kk][:, mp * 128:(mp + 1) * 128],
                                     rhs[kk], start=(kk == 0), stop=(kk == 1))
                    nc.tensor.matmul(psS[:], w3k[kk][:, (4 + mp) * 128:(5 + mp) * 128],
                                     rhs[kk], start=(kk == 0), stop=(kk == 1))
                tmp = P_t3.tile([128, 512], f16, name="silu3", tag="silu3")
                nc.scalar.activation(tmp[:], psB[:], AF.Silu, bias=b3bt[mp])
                outap = pad3(h3[mp])[:, 1 + r0:1 + r0 + 8, 1:65]
                nc.vector.scalar_tensor_tensor(
                    outap,
                    psS[:].rearrange("p (a b) -> p a b", a=8),
                    b3st[mp],
                    tmp[:].rearrange("p (a b) -> p a b", a=8),
                    op0=AL.add, op1=AL.add)
        if img == 0:
            emit_conv(2, [{0: h3[b]} for b in range(4)], *cpools3,
                      zdst=z3_sb, imgs=(0,))
    # conv3 img1: blocks 0-1, then AR3a fires while blocks 2-3 conv and
    # the finals for blocks 0-1 interleave into their slab loop.
    emit_conv(2, [{1: h3sets[1][b]} for b in range(4)], *cpools3,
              zdst=z3_sb, imgs=(1,), blocks=(0, 1))
    reduce_stats(pack3a, [(2, 0), (2, 1)])
    allreduce(pack3a, gst3a, 4, "a3a")
    # bn_math + the first finals are staggered into conv3 blocks 2-3 via the
    # interleave hooks so their AR3a-gated ops never head-of-line-block the
    # DVE/ACT queues ahead of conv3's own slab work.
    ilv3 = [lambda: None] * 3
    ilv3.append(lambda: bn_math(2, 0, gst3a[:, 0:1], gst3a[:, 1:2]))
    ilv3.append(lambda: bn_math(2, 1, gst3a[:, 2:3], gst3a[:, 3:4]))
    ilv3 += [lambda ch=ch: fin_chunk(0, ch, dve_bn=False) for ch in range(3)]
    emit_conv(2, [{1: h3sets[1][b]} for b in range(4)], *cpools3,
              zdst=z3_sb, imgs=(1,), blocks=(2, 3), interleave=ilv3)
    fin_chunk(0, 3, dve_bn=False)
    for ch in range(4):
        fin_chunk(1, ch, dve_bn=False)
    reduce_stats(pack3b, [(2, 2), (2, 3)])
    allreduce(pack3b, gst3b, 4, "a3b")
    bn_math(2, 2, gst3b[:, 0:1], gst3b[:, 1:2])
    bn_math(2, 3, gst3b[:, 2:3], gst3b[:, 3:4])
    for b in (2, 3):
        for ch in range(4):
            fin_chunk(b, ch)
    pools.close_all()


def _get_built(n_cores):
    if n_cores not in _CACHE:
        _CACHE[n_cores] = _build(n_cores)
    return _CACHE[n_cores]


def make_in_maps(inputs, n_cores):
    shared = _prep_shared(inputs)
    xt = _prep_x(inputs["x"], n_cores)
    rsv = float(np.asarray(inputs["res_scale"]).reshape(-1)[0])
    xst = _prep_x(inputs["x"], n_cores, scale=rsv)
    return [dict(shared, x_t=xt[c], xs_t=xst[c]) for c in range(n_cores)]


def kernel(**inputs):
    from concourse.bass_utils import run_bass_kernel_spmd

    assert int(np.asarray(inputs["H"])) == HH and int(np.asarray(inputs["W"])) == HH
    n_cores = 8
    nc = _get_built(n_cores)
    in_maps = make_in_maps(inputs, n_cores)
    res = run_bass_kernel_spmd(nc, in_maps, core_ids=list(range(n_cores)))
    B = np.asarray(inputs["x"]).shape[0]
    per = B // n_cores
    out = np.empty((B, NPIX, CIN), np.float32)
    for c in range(n_cores):
        out[c * per:(c + 1) * per] = \
            res.results[c]["out_t"].astype(np.float32).T.reshape(per, NPIX, CIN)
    return out


# ------------------------------------------------------------- profiling

def _install_ntff_hook():
    """The agent image's antenv lacks axon_hooks; recreate the NTFF profile
    hook via ctypes on the axon PJRT .so (same ABI as trn_boot's)."""
    import contextlib, ctypes, sys, types
    so = "/opt/axon/libaxon_pjrt.so"
    try:
        import antenv.axon_hooks  # noqa: F401
        return True
    except ImportError:
        pass
    try:
        lib = ctypes.CDLL(so)
    except OSError:
        return False
    if not hasattr(lib, "axon_start_nrt_profile"):
        return False
    lib.axon_start_nrt_profile.argtypes = [
        ctypes.POINTER(ctypes.c_int64), ctypes.c_size_t]
    lib.axon_start_nrt_profile.restype = ctypes.c_int64
    lib.axon_stop_nrt_profile.argtypes = [ctypes.c_char_p]
    lib.axon_stop_nrt_profile.restype = ctypes.c_int64

    @contextlib.contextmanager
    def _hook(output_dir, device_ids):
        import jax
        jax.devices()
        if device_ids:
            ids = (ctypes.c_int64 * len(device_ids))(*device_ids)
            rc = lib.axon_start_nrt_profile(ids, len(device_ids))
        else:
            rc = lib.axon_start_nrt_profile(None, 0)
        if rc != 0:
            raise RuntimeError(f"axon_start_nrt_profile rc={rc}")
        try:
            yield
        finally:
            n = lib.axon_stop_nrt_profile(str(output_dir).encode())
            print(f"profile: {n} ntff file(s) -> {output_dir}", file=sys.stderr)

    mod = types.ModuleType("antenv.axon_hooks")
    mod.get_axon_ntff_profile_hook = lambda: _hook
    mod.set_axon_ntff_profile_hook = lambda h: None
    sys.modules["antenv.axon_hooks"] = mod
    import concourse.bass_utils as bu
    bu.upload_artifacts = lambda tmpdir: f"local:{tmpdir}"
    return True


def benchmark(inputs, iters=2, tmpdir=None):
    """Device-only HW execution time (ns) via neuron-profile NTFF trace."""
    import os, tempfile
    from concourse.bass_utils import run_bass_kernel_spmd

    if not _install_ntff_hook():
        raise RuntimeError("NTFF profiling unavailable")
    if tmpdir:
        os.makedirs(tmpdir, exist_ok=True)
    n_cores = 8
    nc = _get_built(n_cores)
    in_maps = make_in_maps(inputs, n_cores)
    times = []
    for i in range(max(1, min(iters, 3))):
        td = tempfile.mkdtemp(dir=tmpdir) if tmpdir else None
        res = run_bass_kernel_spmd(nc, in_maps, core_ids=list(range(n_cores)),
                                   trace=True, tmpdir=td)
        if res.exec_time_ns is not None:
            times.append(res.exec_time_ns)
    if not times:
        raise RuntimeError("no exec_time_ns from traced runs")
    return min(times)


# revision 37
# speedup vs baseline: 1.0518x; 1.0085x over previous
"""Trainium2 Bass kernel for nn_ConvLinearLayer (KAN-style conv-linear block).

Strategy
--------
Data-parallel over batch: 16 images -> 8 cores x 2 images. All activations
live on-chip in transposed layout [channels(partitions), pixels(free)], so
GEMMs (PE, fp16), depthwise 3x3 convs (8 PE diag-matmul taps + 1 fused DVE
tap), BN stats (accumulator outputs) and BN-apply+ReLU (ACT, per-partition
scale/bias) all hit their natural axes. Train-mode BN needs global batch
stats -> three tiny AllReduces (per-channel sum/sumsq), each overlapped with
surrounding compute (fc2 is interleaved into conv2's slab loop).

All matmul operands are fp16 (fp32 PSUM accumulation, fp32 BN statistics,
fp32 output): fp32 moving operands stream at ~half rate through the PE
array, fp16 streams at full rate with ample mantissa for this tolerance.
All conv outputs stay SBUF-resident; weights are host-pre-tiled so every
DMA is a single contiguous 2D transfer.

Host-side precompute: input/weight transposes + fp16 casts, spline-weight
sum (sum_k sw[:,:,k]/K == one GEMM), channel_scale folded into fus_w1,
fus_w2+b2 folded into fc3 (W3_eff = W3 @ W2, b3_eff = W3 @ b2), conv-bias
folded into the BN affine.
"""

import numpy as np

F16 = np.dtype(np.float16)

K_SPLINE = 10
EPS = 1e-5
HH = 64
PW = 66           # padded row stride (64 + 2 zero border)
PAREA = PW * PW   # 4356
NPIX = HH * HH    # 4096 pixels per image
R = 2 * NPIX      # rows per core (2 images)
CIN = 512
LOW = 128
FULL = 256
CAT = 384
FUSH = 192
COUT = 512

TAPS = [(di, dj) for di in (-1, 0, 1) for dj in (-1, 0, 1)]
DVE_TAP = 0                           # fused into the PSUM-combine stt
PE_TAPS = [t for t in range(9) if t != DVE_TAP]
NBLKS = [1, 2, 4]

_CACHE = {}


def _smalls_layout():
    """Column layout of the packed [128, N] fp32 constants tensor."""
    col = 0
    lay = {}
    lay["rs"] = col; col += 1
    for ci, nblk in enumerate(NBLKS):
        for b in range(nblk):
            for nm in ("g", "be", "bb"):
                lay[f"bn{ci}{nm}{b}"] = col; col += 1
    for ci, nblk in enumerate(NBLKS):
        for b in range(nblk):
            lay[f"wv{ci}{b}"] = col; col += 9
    lay["bf1a"] = col; col += 1
    lay["bf1b"] = col; col += 1
    for m in range(4):
        lay[f"b3b{m}"] = col; col += 1
    for m in range(4):
        lay[f"b3s{m}"] = col; col += 1
    return lay, col


# ---------------------------------------------------------------- host prep

def _prep_shared(inp):
    """All non-x device tensors (replicated across cores), as numpy 2D."""
    f = lambda a: np.ascontiguousarray(np.asarray(a, dtype=np.float32))
    h = lambda a: np.ascontiguousarray(
        np.asarray(a, dtype=np.float32).astype(F16))
    sws = lambda sw: np.asarray(sw, np.float64).sum(-1) / K_SPLINE

    fc1_low_bw = f(inp["fc1_low_bw"]); s1l = f(sws(inp["fc1_low_sw"]))
    fc1_full_bw = f(inp["fc1_full_bw"]); s1f = f(sws(inp["fc1_full_sw"]))
    fc2_bw = f(inp["fc2_low_bw"]); s2 = f(sws(inp["fc2_low_sw"]))
    fc3_bw = f(inp["fc3_bw"]); s3 = f(sws(inp["fc3_sw"]))
    w1 = f(inp["fus_w1"]); b1 = f(inp["fus_b1"])
    w2 = f(inp["fus_w2"]); b2 = f(inp["fus_b2"])
    cs = f(inp["channel_scale"])

    d = {}
    # stage A lhsT tiles (k,m) of [512, 768] packed as [128, 24*128]
    # m-blocks: [lowb, lows, fullb0, fullb1, fulls0, fulls1]
    wA = np.concatenate([fc1_low_bw.T, s1l.T, fc1_full_bw.T, s1f.T], axis=1)
    wAt = np.empty((128, 24 * 128), np.float32)
    for k in range(4):
        for m in range(6):
            wAt[:, (k * 6 + m) * 128:(k * 6 + m + 1) * 128] = \
                wA[k * 128:(k + 1) * 128, m * 128:(m + 1) * 128]
    d["wA"] = h(wAt)
    d["wfc2"] = h(np.concatenate([fc2_bw.T, s2.T], axis=1))      # [128, 256]
    wfus1 = (w1 * cs[None, :]).T                                 # [384, 192]
    wf1t = np.empty((128, 3 * FUSH), np.float32)
    for k in range(3):
        wf1t[:, k * FUSH:(k + 1) * FUSH] = wfus1[k * 128:(k + 1) * 128, :]
    d["wfus1"] = h(wf1t)                                         # [128, 576]
    w3b = fc3_bw @ w2                                            # [512, 192]
    w3s = s3 @ w2
    d["wfc3"] = h(np.concatenate([w3b.T, w3s.T], axis=1))        # [192, 1024]
    b3b = (fc3_bw @ b2).reshape(-1)
    b3s = (s3 @ b2).reshape(-1)

    lay, ncols = _smalls_layout()
    sm = np.zeros((128, ncols), np.float32)
    sm[:, lay["rs"]] = float(np.asarray(inp["res_scale"]).reshape(-1)[0])
    sm[0:128, lay["bf1a"]] = b1[0:128]
    sm[0:64, lay["bf1b"]] = b1[128:192]
    for m in range(4):
        sm[:, lay[f"b3b{m}"]] = b3b[m * 128:(m + 1) * 128]
        sm[:, lay[f"b3s{m}"]] = b3s[m * 128:(m + 1) * 128]

    # depthwise convs: diag tiles (b,t) packed as [128, nblk*9*128]
    for ci, (wname, gname, bname, bbname, Cc) in enumerate([
            ("dw1_w", "dw1_g", "dw1_beta", "dw1_b", LOW),
            ("dw2_w", "dw2_g", "dw2_beta", "dw2_b", FULL),
            ("dw3_w", "dw3_g", "dw3_beta", "dw3_b", COUT)]):
        w = f(inp[wname]).reshape(Cc, 9)                          # [C, taps]
        g = f(inp[gname]).reshape(-1)
        be = f(inp[bname]).reshape(-1)
        bb = f(inp[bbname]).reshape(-1)
        nblk = Cc // 128
        diag = np.zeros((128, nblk * 9 * 128), np.float32)
        for b in range(nblk):
            rows = slice(b * 128, (b + 1) * 128)
            for t in range(9):
                c0 = (b * 9 + t) * 128
                diag[:, c0:c0 + 128] = np.diag(w[rows, t])
            c = lay[f"wv{ci}{b}"]
            sm[:, c:c + 9] = w[rows]
            sm[:, lay[f"bn{ci}g{b}"]] = g[rows]
            sm[:, lay[f"bn{ci}be{b}"]] = be[rows]
            sm[:, lay[f"bn{ci}bb{b}"]] = bb[rows]
        d[f"diag{ci+1}"] = h(diag)
    d["smalls"] = np.ascontiguousarray(sm)
    return d


def _prep_x(x, n_cores, scale=None):
    """Per-core transposed fp16 shards [512, 8192]."""
    x = np.asarray(x, np.float32)
    if scale is not None:
        x = x * scale
    x = x.astype(F16)
    B = x.shape[0]
    per = B // n_cores
    return [np.ascontiguousarray(
        x[c * per:(c + 1) * per].reshape(per * NPIX, CIN).T)
        for c in range(n_cores)]


# ---------------------------------------------------------------- builder

def _build(n_cores):
    import concourse.bacc as bacc
    import concourse.mybir as mybir
    import concourse.tile as tile

    f32 = mybir.dt.float32
    f16 = mybir.dt.float16

    nc = bacc.Bacc("TRN2", target_bir_lowering=False, debug=False,
                   num_devices=n_cores)

    def din(name, shape, dt=f16):
        return nc.dram_tensor(name, list(shape), dt, kind="ExternalInput").ap()

    x_d = din("x_t", (CIN, R))
    xs_d = din("xs_t", (CIN, R))
    wA_d = din("wA", (128, 24 * 128))
    wfc2_d = din("wfc2", (128, 256))
    wfus1_d = din("wfus1", (128, 3 * FUSH))
    wfc3_d = din("wfc3", (FUSH, 1024))
    lay, ncols = _smalls_layout()
    smalls_d = din("smalls", (128, ncols), f32)
    conv_d = []
    for ci, Cc in [(1, LOW), (2, FULL), (3, COUT)]:
        nblk = Cc // 128
        conv_d.append(dict(
            diag=din(f"diag{ci}", (128, nblk * 9 * 128)),
            nblk=nblk))
    out_d = nc.dram_tensor("out_t", [COUT, R], f16, kind="ExternalOutput").ap()

    with tile.TileContext(nc) as tc:
        _emit(nc, tc, mybir, n_cores, x_d, xs_d, wA_d, wfc2_d, wfus1_d,
              wfc3_d, conv_d, smalls_d, lay, ncols, out_d)
    nc.compile()
    return nc


def _emit(nc, tc, mybir, n_cores, x_d, xs_d, wA_d, wfc2_d, wfus1_d,
          wfc3_d, conv_d, smalls_d, lay, ncols, out_d):
    f32 = mybir.dt.float32
    f16 = mybir.dt.float16
    AL = mybir.AluOpType
    AF = mybir.ActivationFunctionType
    inv_n = 1.0 / (n_cores * R)

    class _Pools:
        def __init__(self, tc):
            self.tc = tc
            self.cms = {}
            self.order = []
        def open(self, name, **kw):
            cm = self.tc.tile_pool(name=name, **kw)
            pool = cm.__enter__()
            self.cms[name] = cm
            self.order.append(name)
            return pool
        def close(self, *names):
            names = sorted(names, key=self.order.index, reverse=True)
            for n in names:
                assert n == self.order[-1], (n, self.order)
                self.order.pop()
                self.cms.pop(n).__exit__(None, None, None)
        def close_all(self):
            self.close(*self.order)

    pools = _Pools(tc)

    def pad3(t):
        return t[:].rearrange("p (a b) -> p a b", a=PW)

    # ---------------- persistent small tiles ----------------
    P_pers = pools.open("pers", bufs=1)
    P_tmpv = pools.open("tmpv", bufs=4)
    P_dram = pools.open("dramp", bufs=1, space="DRAM")

    # one DMA for every small constant; everything below is a column slice
    smalls = P_pers.tile([128, ncols], f32, name="smalls", tag="smalls")
    sc = lambda key: smalls[:, lay[key]:lay[key] + 1]

    rs_t = sc("rs")
    bf1a = sc("bf1a")
    bf1b = smalls[0:64, lay["bf1b"]:lay["bf1b"] + 1]
    b3bt = [sc(f"b3b{m}") for m in range(4)]
    b3st = [sc(f"b3s{m}") for m in range(4)]

    dummy_w = P_pers.tile([128, 128], f16, name="dummy_w", tag="dummy_w")
    dummy_rhs = P_pers.tile([128, 512], f16, name="dummy_rhs", tag="dummy_rhs")
    nc.gpsimd.memset(dummy_w[:], 0.0)
    nc.gpsimd.memset(dummy_rhs[:], 0.0)

    bn = []  # bn[ci][blk] = dict(g, be, bb(slices), a, b(tiles))
    for ci in range(3):
        blks = []
        for b in range(conv_d[ci]["nblk"]):
            e = {nm: sc(f"bn{ci}{nm}{b}") for nm in ("g", "be", "bb")}
            e["a"] = P_pers.tile([128, 1], f32, name=f"bn{ci}a{b}", tag=f"bn{ci}a{b}")
            e["b"] = P_pers.tile([128, 1], f32, name=f"bn{ci}b{b}", tag=f"bn{ci}b{b}")
            blks.append(e)
        bn.append(blks)

    wv_t = [[smalls[:, lay[f"wv{ci}{b}"]:lay[f"wv{ci}{b}"] + 9]
             for b in range(conv_d[ci]["nblk"])] for ci in range(3)]

    SLAB = 1024                      # conv slab (PSUM-resident px per step)
    NSLAB = NPIX // SLAB             # 4 slabs per image
    Sp, Qp = [], []
    for ci in range(3):
        Sp.append([P_pers.tile([128, 2 * NSLAB], f32, name=f"Sp{ci}{b}",
                               tag=f"Sp{ci}{b}") for b in range(conv_d[ci]["nblk"])])
        Qp.append([P_pers.tile([128, 2 * NSLAB], f32, name=f"Qp{ci}{b}",
                               tag=f"Qp{ci}{b}") for b in range(conv_d[ci]["nblk"])])
    pack1 = P_pers.tile([128, 2], f32, name="pack1", tag="pack1")
    pack2 = P_pers.tile([128, 4], f32, name="pack2", tag="pack2")
    pack3a = P_pers.tile([128, 4], f32, name="pack3a", tag="pack3a")
    pack3b = P_pers.tile([128, 4], f32, name="pack3b", tag="pack3b")
    gst1 = P_pers.tile([128, 2], f32, name="gst1", tag="gst1")
    gst2 = P_pers.tile([128, 4], f32, name="gst2", tag="gst2")
    gst3a = P_pers.tile([128, 4], f32, name="gst3a", tag="gst3a")
    gst3b = P_pers.tile([128, 4], f32, name="gst3b", tag="gst3b")

    # --------- conv emitter: 8 PE taps + fused DVE tap/combine/stats -----
    # zdst[b] = persistent SBUF tile [128, R]; slab written at
    # [:, img*NPIX + s*SLAB : +SLAB]. After each slab, one queued
    # interleave callback is drained (used to overlap fc2 with conv2).
    FUSED_STT = True     # fuse DVE tap + PSUM-combine + Sp into one stt
    ACT_SQUARE = True     # Qp via ACT Square (v1) vs DVE tensor_tensor_reduce

    def emit_conv(ci, pads, P_cps, P_cacc, P_csq, P_diag, zdst, imgs=(0, 1),
                  interleave=None, blocks=None, extra_dve_tap=None):
        nblk = conv_d[ci]["nblk"]
        rows = SLAB // HH
        diag_dram = conv_d[ci]["diag"]
        pe_taps = [t for t in PE_TAPS if t != extra_dve_tap]
        for b in (range(nblk) if blocks is None else blocks):
            dg = P_diag.tile([128, 9 * 128], f16, name="dg", tag="dg")
            nc.sync.dma_start(
                dg[:], diag_dram[:, b * 9 * 128:(b + 1) * 9 * 128])
            for img in imgs:
                p3 = pad3(pads[b][img])
                for s in range(NSLAB):
                    r0 = s * rows
                    ps = P_cps.tile([128, SLAB], f32, name=f"cps{ci}", tag="cps")
                    for ti, t in enumerate(pe_taps):
                        di, dj = TAPS[t]
                        rhs = p3[:, 1 + di + r0:1 + di + r0 + rows,
                                 1 + dj:1 + dj + HH]
                        for nn in range(SLAB // 512):
                            rr = nn * (512 // HH)
                            nc.tensor.matmul(
                                ps[:, nn * 512:(nn + 1) * 512],
                                dg[:, t * 128:(t + 1) * 128],
                                rhs[:, rr:rr + (512 // HH), :],
                                start=(ti == 0), stop=(ti == len(pe_taps) - 1))
                    slot = img * NSLAB + s
                    col = img * NPIX + s * SLAB
                    zsl = zdst[b][:, col:col + SLAB]
                    psum_in = ps[:].rearrange("p (a b) -> p a b", a=rows)
                    if extra_dve_tap is not None:
                        di, dj = TAPS[extra_dve_tap]
                        acc = P_cacc.tile([128, SLAB], f16, name="cacc", tag="cacc")
                        nc.vector.scalar_tensor_tensor(
                            acc[:].rearrange("p (a b) -> p a b", a=rows),
                            p3[:, 1 + di + r0:1 + di + r0 + rows,
                               1 + dj:1 + dj + HH],
                            wv_t[ci][b][:, extra_dve_tap:extra_dve_tap + 1],
                            psum_in, op0=AL.mult, op1=AL.add)
                        psum_in = acc[:].rearrange("p (a b) -> p a b", a=rows)
                    di, dj = TAPS[DVE_TAP]
                    tap_ap = p3[:, 1 + di + r0:1 + di + r0 + rows,
                                1 + dj:1 + dj + HH]
                    if FUSED_STT:
                        nc.vector.scalar_tensor_tensor(
                            zsl.rearrange("p (a b) -> p a b", a=rows),
                            tap_ap,
                            wv_t[ci][b][:, DVE_TAP:DVE_TAP + 1],
                            psum_in,
                            op0=AL.mult, op1=AL.add,
                            accum_out=Sp[ci][b][:, slot:slot + 1])
                    else:
                        acc = P_cacc.tile([128, SLAB], f16, name="cacc", tag="cacc")
                        nc.vector.tensor_scalar(
                            acc[:].rearrange("p (a b) -> p a b", a=rows),
                            tap_ap, wv_t[ci][b][:, DVE_TAP:DVE_TAP + 1], None,
                            op0=AL.mult)
                        nc.vector.scalar_tensor_tensor(
                            zsl, acc[:], 0.0, ps[:], op0=AL.bypass, op1=AL.add,
                            accum_out=Sp[ci][b][:, slot:slot + 1])
                    sq = P_csq.tile([128, SLAB], f16, name="sqs", tag="sqs")
                    if ACT_SQUARE:
                        nc.scalar.activation(sq[:], zsl, AF.Square,
                                             accum_out=Qp[ci][b][:, slot:slot + 1])
                    else:
                        nc.vector.tensor_tensor_reduce(
                            sq[:], zsl, zsl, 1.0, 0.0, op0=AL.mult, op1=AL.add,
                            accum_out=Qp[ci][b][:, slot:slot + 1])
                    if interleave:
                        interleave.pop(0)()

    def open_conv_pools(sfx):
        return (pools.open(f"cps{sfx}", bufs=2, space="PSUM"),
                pools.open(f"cacc{sfx}", bufs=2),
                pools.open(f"csq{sfx}", bufs=2),
                pools.open(f"diag{sfx}", bufs=2))

    def close_conv_pools(sfx):
        pools.close(f"diag{sfx}", f"csq{sfx}", f"cacc{sfx}", f"cps{sfx}")

    def bn_math(ci, b, S_ap, Q_ap):
        e = bn[ci][b]
        tt = lambda tag: P_tmpv.tile([128, 1], f32, name=tag, tag=tag)
        m = tt("bnm"); e2 = tt("bne"); m2 = tt("bnm2"); v = tt("bnv")
        sq = tt("bnsq"); iv = tt("bniv"); mb = tt("bnmb"); ab = tt("bnab")
        nc.vector.tensor_scalar(m[:], S_ap, inv_n, None, op0=AL.mult)
        nc.vector.tensor_scalar(e2[:], Q_ap, inv_n, None, op0=AL.mult)
        nc.vector.tensor_tensor(m2[:], m[:], m[:], op=AL.mult)
        nc.vector.tensor_tensor(v[:], e2[:], m2[:], op=AL.subtract)
        nc.vector.tensor_scalar(v[:], v[:], EPS, None, op0=AL.add)
        nc.scalar.activation(sq[:], v[:], AF.Sqrt)
        nc.vector.reciprocal(iv[:], sq[:])
        nc.vector.tensor_tensor(e["a"][:], e["g"], iv[:], op=AL.mult)
        nc.vector.tensor_tensor(mb[:], m[:], e["bb"], op=AL.add)
        nc.vector.tensor_tensor(ab[:], e["a"][:], mb[:], op=AL.mult)
        nc.vector.tensor_tensor(e["b"][:], e["be"], ab[:], op=AL.subtract)

    def allreduce(pack, gst, ncols, tag):
        if n_cores == 1:
            nc.vector.tensor_copy(gst[:], pack[:])
            return
        ib = P_dram.tile([128, ncols], f32, name=f"cc_in{tag}", tag=f"cc_in{tag}")
        ob = P_dram.tile([128, ncols], f32, name=f"cc_out{tag}", tag=f"cc_out{tag}")
        nc.gpsimd.dma_start(ib[:], pack[:])
        nc.gpsimd.collective_compute(
            "AllReduce", AL.add,
            replica_groups=[list(range(n_cores))],
            ins=[ib.opt()], outs=[ob.opt()])
        nc.gpsimd.dma_start(gst[:], ob[:])

    def reduce_stats(pack, cols):
        for i, (ci, b) in enumerate(cols):
            nc.vector.tensor_reduce(pack[:, 2 * i:2 * i + 1], Sp[ci][b][:],
                                    axis=mybir.AxisListType.X, op=AL.add)
            nc.vector.tensor_reduce(pack[:, 2 * i + 1:2 * i + 2], Qp[ci][b][:],
                                    axis=mybir.AxisListType.X, op=AL.add)

    # persistent SBUF activations (fp16)
    P_hf = pools.open("hfp", bufs=1)
    hf1a = P_hf.tile([128, R], f16, name="hf1a", tag="hf1a")
    hf1b = P_hf.tile([64, R], f16, name="hf1b", tag="hf1b")

    # =================== stage A: fc1_low + fc1_full ==================
    P_z12 = pools.open("z12p", bufs=1)
    z1_sb = [P_z12.tile([128, R], f16, name="z1sb", tag="z1sb")]
    z2_sb = [P_z12.tile([128, R], f16, name=f"z2sb{b}", tag=f"z2sb{b}")
             for b in range(2)]
    yl_sb = P_z12.tile([128, R], f16, name="ylsb", tag="ylsb")

    P_pad2 = pools.open("pads2", bufs=1)
    P_pad1 = pools.open("pads1", bufs=1)
    y1p = [P_pad1.tile([128, PAREA], f16, name=f"y1p{i}", tag=f"y1p{i}")
           for i in range(2)]
    y2p = [[P_pad2.tile([128, PAREA], f16, name=f"y2p{b}{i}", tag=f"y2p{b}{i}")
            for i in range(2)] for b in range(2)]
    for t in y1p:
        nc.vector.memset(t[:], 0.0)
    for i in range(2):            # image-major: img0 pads ready first
        for b in range(2):
            nc.gpsimd.memset(y2p[b][i][:], 0.0)

    P_wA = pools.open("wAp", bufs=1)
    P_xk = pools.open("xk", bufs=2)
    P_tmpA = pools.open("tmpA", bufs=2)
    P_psA = pools.open("psA", bufs=2, space="PSUM")
    wA_sb = P_wA.tile([128, 24 * 128], f16, name="wAsb", tag="wAsb")
    wAt = lambda k, m: wA_sb[:, (k * 6 + m) * 128:(k * 6 + m + 1) * 128]
    pairs = [(0, 1, lambda img: y1p[img]),
             (2, 4, lambda img: y2p[0][img]),
             (3, 5, lambda img: y2p[1][img])]
    for ch in range(8):
        img, lrow = ch // 4, (ch % 4) * 16
        xs = []
        for k in range(4):
            xt = P_xk.tile([128, 1024], f16, name=f"xk{k}", tag=f"xk{k}")
            eng = nc.sync if k % 2 == 0 else nc.scalar
            eng.dma_start(
                xt[:], x_d[k * 128:(k + 1) * 128, ch * 1024:(ch + 1) * 1024])
            xs.append(xt)
        if ch == 0:
            nc.sync.dma_start(wA_sb[:], wA_d[:])
        for bm, sm, dest in pairs:
            psB = P_psA.tile([128, 1024], f32, name="psB", tag="psB")
            psS = P_psA.tile([128, 1024], f32, name="psS", tag="psS")
            for k in range(4):
                for nn in range(2):
                    sl = slice(nn * 512, (nn + 1) * 512)
                    nc.tensor.matmul(psB[:, sl], wAt(k, bm), xs[k][:, sl],
                                     start=(k == 0), stop=(k == 3))
                    nc.tensor.matmul(psS[:, sl], wAt(k, sm), xs[k][:, sl],
                                     start=(k == 0), stop=(k == 3))
            tmp = P_tmpA.tile([128, 1024], f16, name="siluA", tag="siluA")
            nc.scalar.activation(tmp[:], psB[:], AF.Silu)
            outap = pad3(dest(img))[:, 1 + lrow:1 + lrow + 16, 1:65]
            nc.vector.scalar_tensor_tensor(
                outap,
                psS[:].rearrange("p (a b) -> p a b", a=16),
                0.0,
                tmp[:].rearrange("p (a b) -> p a b", a=16),
                op0=AL.bypass, op1=AL.add)
    pools.close("psA", "tmpA", "xk", "wAp")
    nc.scalar.dma_start(smalls[:], smalls_d[:])

    # ============ conv1 -> AR1 (overlapped with conv2+fc2) =============
    cpools12 = open_conv_pools("c12")
    emit_conv(0, [y1p], *cpools12, zdst=z1_sb, extra_dve_tap=8)
    reduce_stats(pack1, [(0, 0)])
    allreduce(pack1, gst1, 2, "a1")

    # fc2 work units, interleaved into conv2's 16 slab iterations
    P_w2 = pools.open("wfc2p", bufs=1)
    P_t2 = pools.open("fc2t", bufs=2)
    P_ps2 = pools.open("psF2", bufs=1, space="PSUM")
    w2_sb = P_w2.tile([128, 256], f16, name="w2sb", tag="w2sb")
    nc.sync.dma_start(w2_sb[:], wfc2_d[:])

    def fc2_chunk(ch):
        sl = slice(ch * 1024, (ch + 1) * 1024)
        z1b = P_t2.tile([128, 1024], f16, name="z1b", tag="z1b")
        nc.scalar.activation(z1b[:], z1_sb[0][:, sl], AF.Relu,
                             bias=bn[0][0]["b"][:], scale=bn[0][0]["a"][:])
        psB = P_ps2.tile([128, 1024], f32, name="ps2B", tag="ps2B")
        psS = P_ps2.tile([128, 1024], f32, name="ps2S", tag="ps2S")
        for nn in range(2):
            s2 = slice(nn * 512, (nn + 1) * 512)
            nc.tensor.matmul(psB[:, s2], w2_sb[:, 0:128], z1b[:, s2],
                             start=True, stop=True)
            nc.tensor.matmul(psS[:, s2], w2_sb[:, 128:256], z1b[:, s2],
                             start=True, stop=True)
        tmp = P_t2.tile([128, 1024], f16, name="silu2", tag="silu2")
        nc.scalar.activation(tmp[:], psB[:], AF.Silu)
        nc.vector.scalar_tensor_tensor(yl_sb[:, sl], psS[:], 0.0, tmp[:],
                                       op0=AL.bypass, op1=AL.add)

    INTERLEAVE_FC2 = True
    if INTERLEAVE_FC2:
        todo = [lambda: bn_math(0, 0, gst1[:, 0:1], gst1[:, 1:2])]
        todo += [lambda ch=ch: fc2_chunk(ch) for ch in range(8)]
        ilv = [lambda: None] * 5 + todo
        ilv += [lambda: None] * (16 - len(ilv))
    else:
        ilv = None

    emit_conv(1, y2p, *cpools12, zdst=z2_sb, interleave=ilv)
    if not INTERLEAVE_FC2:
        bn_math(0, 0, gst1[:, 0:1], gst1[:, 1:2])
        for ch in range(8):
            fc2_chunk(ch)
    pools.close("psF2", "fc2t", "wfc2p")
    close_conv_pools("c12")
    pools.close("pads1", "pads2")
    reduce_stats(pack2, [(1, 0), (1, 1)])
    allreduce(pack2, gst2, 4, "a2")
    bn_math(1, 0, gst2[:, 0:1], gst2[:, 1:2])
    bn_math(1, 1, gst2[:, 2:3], gst2[:, 3:4])

    # keep the PE array busy through the AR2 collective so the HAM clock
    # gate stays at 8/8 into fusion/fc3 (a >3.4us PE-idle window would
    # re-throttle to 1.2 GHz for tens of us). Garbage-in, never-read-out.
    P_warm = pools.open("pswarm", bufs=1, space="PSUM")
    wps = P_warm.tile([128, 512], f32, name="wps", tag="wps")
    NWARM = 130
    for i in range(NWARM):
        nc.tensor.matmul(wps[:], dummy_w[:], dummy_rhs[:],
                         start=(i == 0), stop=(i == NWARM - 1),
                         skip_group_check=True)
    pools.close("pswarm")

    # =================== fusion linear 1 -> hf1 (SBUF) ==================
    P_wf1 = pools.open("wfu1", bufs=1)
    P_tf1 = pools.open("fu1t", bufs=3)
    P_psf1 = pools.open("psFu1", bufs=2, space="PSUM")
    wf1_sb = P_wf1.tile([128, 3 * FUSH], f16, name="wf1sb", tag="wf1sb")
    nc.sync.dma_start(wf1_sb[:], wfus1_d[:])
    wf1t = lambda k, m: wf1_sb[:, k * FUSH + m * 128:k * FUSH + m * 128 + (64 if m else 128)]
    zero64 = P_wf1.tile([64, 1024], f16, name="zero64", tag="zero64")
    nc.gpsimd.memset(zero64[:], 0.0)
    for ch in range(8):
        sl = slice(ch * 1024, (ch + 1) * 1024)
        z2b0 = P_tf1.tile([128, 1024], f16, name="z2b0", tag="z2b0")
        z2b1 = P_tf1.tile([128, 1024], f16, name="z2b1", tag="z2b1")
        nc.scalar.activation(z2b0[:], z2_sb[0][:, sl], AF.Relu,
                             bias=bn[1][0]["b"][:], scale=bn[1][0]["a"][:])
        nc.vector.tensor_scalar(z2b1[:], z2_sb[1][:, sl], bn[1][1]["a"][:],
                                bn[1][1]["b"][:], op0=AL.mult, op1=AL.add)
        nc.vector.tensor_scalar(z2b1[:], z2b1[:], 0.0, None, op0=AL.max)
        rhs = [yl_sb[:, sl], z2b0[:], z2b1[:]]
        ps0 = P_psf1.tile([128, 1024], f32, name="psf1a", tag="psf1a")
        ps1 = P_psf1.tile([64, 1024], f32, name="psf1b", tag="psf1b")
        for k in range(3):
            for nn in range(2):
                s2 = slice(nn * 512, (nn + 1) * 512)
                nc.tensor.matmul(ps0[:, s2], wf1t(k, 0), rhs[k][:, s2],
                                 start=(k == 0), stop=(k == 2))
                nc.tensor.matmul(ps1[:, s2], wf1t(k, 1), rhs[k][:, s2],
                                 start=(k == 0), stop=(k == 2))
        nc.scalar.activation(hf1a[:, sl], ps0[:], AF.Relu, bias=bf1a)
        nc.vector.scalar_tensor_tensor(hf1b[:, sl], ps1[:], bf1b, zero64[:],
                                       op0=AL.add, op1=AL.max)
    pools.close("psFu1", "fu1t", "wfu1")
    pools.close("z12p")

    # ============= fc3' + conv3 (block-split stats) + finals ============
    P_z3 = pools.open("z3p", bufs=1)
    z3_sb = [P_z3.tile([128, R], f16, name=f"z3sb{b}", tag=f"z3sb{b}")
             for b in range(4)]
    P_w3 = pools.open("wfc3p", bufs=1)
    P_h3 = pools.open("h3p", bufs=1)
    P_t3 = pools.open("fc3t", bufs=3)
    P_ps3 = pools.open("psF3", bufs=2, space="PSUM")
    P_xc = pools.open("xcp", bufs=3)
    P_fin = pools.open("fint", bufs=3)
    cpools3 = open_conv_pools("c3")
    w3k = [P_w3.tile([128, 1024], f16, name="w3k0", tag="w3k0"),
           P_w3.tile([64, 1024], f16, name="w3k1", tag="w3k1")]
    nc.sync.dma_start(w3k[0][:], wfc3_d[0:128, :])
    nc.sync.dma_start(w3k[1][:], wfc3_d[128:192, :])

    def fin_chunk(b, ch, dve_bn=True):
        rows = slice(b * 128, (b + 1) * 128)
        sl = slice(ch * 2048, (ch + 1) * 2048)
        xc = P_xc.tile([128, 2048], f16, name="xc", tag="xc")
        nc.scalar.dma_start(xc[:], xs_d[rows, sl])
        t = P_fin.tile([128, 2048], f16, name="trelu", tag="trelu")
        if dve_bn and ch == 3:   # balance: every 4th BN+ReLU on the DVE
            nc.vector.tensor_scalar(t[:], z3_sb[b][:, sl],
                                    bn[2][b]["a"][:], bn[2][b]["b"][:],
                                    op0=AL.mult, op1=AL.add)
            nc.vector.tensor_scalar(t[:], t[:], 0.0, None, op0=AL.max)
        else:
            nc.scalar.activation(t[:], z3_sb[b][:, sl], AF.Relu,
                                 bias=bn[2][b]["b"][:], scale=bn[2][b]["a"][:])
        ob = P_fin.tile([128, 2048], f16, name="ob", tag="ob")
        nc.vector.tensor_tensor(ob[:], xc[:], t[:], op=AL.add)
        nc.sync.dma_start(out_d[rows, sl], ob[:])

    h3sets = []
    for img in range(2):
        h3 = [P_h3.tile([128, PAREA], f16, name=f"h3p{b}", tag=f"h3p{b}")
              for b in range(4)]
        h3sets.append(h3)
        for t in h3:
            nc.gpsimd.memset(t[:], 0.0)
        for ch in range(8):           # 512-px chunks within image
            r0 = ch * 8
            sl = slice(img * NPIX + ch * 512, img * NPIX + (ch + 1) * 512)
            rhs = [hf1a[:, sl], hf1b[:, sl]]
            for mp in range(4):
                psB = P_ps3.tile([128, 512], f32, name="ps3B", tag="ps3B")
                psS = P_ps3.tile([128, 512], f32, name="ps3S", tag="ps3S")
                for kk in range(2):
                    nc.tensor.matmul(psB[:], w3k[kk][:, mp * 128:(mp + 1) * 128],
                                     rhs[kk], start=(kk == 0), stop=(kk == 1))
                    nc.tensor.matmul(psS[:], w3k[kk][:, (4 + mp) * 128:(5 + mp) * 128],
                                     rhs[kk], start=(kk == 0), stop=(kk == 1))
                tmp = P_t3.tile([128, 512], f16, name="silu3", tag="silu3")
                nc.scalar.activation(tmp[:], psB[:], AF.Silu, bias=b3bt[mp])
                outap = pad3(h3[mp])[:, 1 + r0:1 + r0 + 8, 1:65]
                nc.vector.scalar_tensor_tensor(
                    outap,
                    psS[:].rearrange("p (a b) -> p a b", a=8),
                    b3st[mp],
                    tmp[:].rearrange("p (a b) -> p a b", a=8),
                    op0=AL.add, op1=AL.add)
        if img == 0:
            emit_conv(2, [{0: h3[b]} for b in range(4)], *cpools3,
                      zdst=z3_sb, imgs=(0,), extra_dve_tap=8)
    # conv3 img1: blocks 0-1, then AR3a fires while blocks 2-3 conv and
    # the finals for blocks 0-1 interleave into their slab loop.
    emit_conv(2, [{1: h3sets[1][b]} for b in range(4)], *cpools3,
              zdst=z3_sb, imgs=(1,), blocks=(0, 1), extra_dve_tap=8)
    reduce_stats(pack3a, [(2, 0), (2, 1)])
    allreduce(pack3a, gst3a, 4, "a3a")
    # bn_math + the first finals are staggered into conv3 blocks 2-3 via the
    # interleave hooks so their AR3a-gated ops never head-of-line-block the
    # DVE/ACT queues ahead of conv3's own slab work.
    ilv3 = [lambda: None] * 3
    ilv3.append(lambda: bn_math(2, 0, gst3a[:, 0:1], gst3a[:, 1:2]))
    ilv3.append(lambda: bn_math(2, 1, gst3a[:, 2:3], gst3a[:, 3:4]))
    ilv3 += [lambda ch=ch: fin_chunk(0, ch, dve_bn=False) for ch in range(3)]
    emit_conv(2, [{1: h3sets[1][b]} for b in range(4)], *cpools3,
              zdst=z3_sb, imgs=(1,), blocks=(2, 3), interleave=ilv3)
    fin_chunk(0, 3, dve_bn=False)
    for ch in range(4):
        fin_chunk(1, ch, dve_bn=False)
    reduce_stats(pack3b, [(2, 2), (2, 3)])
    allreduce(pack3b, gst3b, 4, "a3b")
    bn_math(2, 2, gst3b[:, 0:1], gst3b[:, 1:2])
    bn_math(2, 3, gst3b[:, 2:3], gst3b[:, 3:4])
    for b in (2, 3):
        for ch in range(4):
            fin_chunk(b, ch)
    pools.close_all()


def _get_built(n_cores):
    if n_cores not in _CACHE:
        _CACHE[n_cores] = _build(n_cores)
    return _CACHE[n_cores]


def make_in_maps(inputs, n_cores):
    shared = _prep_shared(inputs)
    xt = _prep_x(inputs["x"], n_cores)
    rsv = float(np.asarray(inputs["res_scale"]).reshape(-1)[0])
    xst = _prep_x(inputs["x"], n_cores, scale=rsv)
    return [dict(shared, x_t=xt[c], xs_t=xst[c]) for c in range(n_cores)]


def kernel(**inputs):
    from concourse.bass_utils import run_bass_kernel_spmd

    assert int(np.asarray(inputs["H"])) == HH and int(np.asarray(inputs["W"])) == HH
    n_cores = 8
    nc = _get_built(n_cores)
    in_maps = make_in_maps(inputs, n_cores)
    res = run_bass_kernel_spmd(nc, in_maps, core_ids=list(range(n_cores)))
    B = np.asarray(inputs["x"]).shape[0]
    per = B // n_cores
    out = np.empty((B, NPIX, CIN), np.float32)
    for c in range(n_cores):
        out[c * per:(c + 1) * per] = \
            res.results[c]["out_t"].astype(np.float32).T.reshape(per, NPIX, CIN)
    return out


# ------------------------------------------------------------- profiling

def _install_ntff_hook():
    """The agent image's antenv lacks axon_hooks; recreate the NTFF profile
    hook via ctypes on the axon PJRT .so (same ABI as trn_boot's)."""
    import contextlib, ctypes, sys, types
    so = "/opt/axon/libaxon_pjrt.so"
    try:
        import antenv.axon_hooks  # noqa: F401
        return True
    except ImportError:
        pass
    try:
        lib = ctypes.CDLL(so)
    except OSError:
        return False
    if not hasattr(lib, "axon_start_nrt_profile"):
        return False
    lib.axon_start_nrt_profile.argtypes = [
        ctypes.POINTER(ctypes.c_int64), ctypes.c_size_t]
    lib.axon_start_nrt_profile.restype = ctypes.c_int64
    lib.axon_stop_nrt_profile.argtypes = [ctypes.c_char_p]
    lib.axon_stop_nrt_profile.restype = ctypes.c_int64

    @contextlib.contextmanager
    def _hook(output_dir, device_ids):
        import jax
        jax.devices()
        if device_ids:
            ids = (ctypes.c_int64 * len(device_ids))(*device_ids)
            rc = lib.axon_start_nrt_profile(ids, len(device_ids))
        else:
            rc = lib.axon_start_nrt_profile(None, 0)
        if rc != 0:
            raise RuntimeError(f"axon_start_nrt_profile rc={rc}")
        try:
            yield
        finally:
            n = lib.axon_stop_nrt_profile(str(output_dir).encode())
            print(f"profile: {n} ntff file(s) -> {output_dir}", file=sys.stderr)

    mod = types.ModuleType("antenv.axon_hooks")
    mod.get_axon_ntff_profile_hook = lambda: _hook
    mod.set_axon_ntff_profile_hook = lambda h: None
    sys.modules["antenv.axon_hooks"] = mod
    import concourse.bass_utils as bu
    bu.upload_artifacts = lambda tmpdir: f"local:{tmpdir}"
    return True


def benchmark(inputs, iters=2, tmpdir=None):
    """Device-only HW execution time (ns) via neuron-profile NTFF trace."""
    import os, tempfile
    from concourse.bass_utils import run_bass_kernel_spmd

    if not _install_ntff_hook():
        raise RuntimeError("NTFF profiling unavailable")
    if tmpdir:
        os.makedirs(tmpdir, exist_ok=True)
    n_cores = 8
    nc = _get_built(n_cores)
    in_maps = make_in_maps(inputs, n_cores)
    times = []
    for i in range(max(1, min(iters, 3))):
        td = tempfile.mkdtemp(dir=tmpdir) if tmpdir else None
        res = run_bass_kernel_spmd(nc, in_maps, core_ids=list(range(n_cores)),
                                   trace=True, tmpdir=td)
        if res.exec_time_ns is not None:
            times.append(res.exec_time_ns)
    if not times:
        raise RuntimeError("no exec_time_ns from traced runs")
    return min(times)


# revision 38
# speedup vs baseline: 1.0530x; 1.0012x over previous
"""Trainium2 Bass kernel for nn_ConvLinearLayer (KAN-style conv-linear block).

Strategy
--------
Data-parallel over batch: 16 images -> 8 cores x 2 images. All activations
live on-chip in transposed layout [channels(partitions), pixels(free)], so
GEMMs (PE, fp16), depthwise 3x3 convs (8 PE diag-matmul taps + 1 fused DVE
tap), BN stats (accumulator outputs) and BN-apply+ReLU (ACT, per-partition
scale/bias) all hit their natural axes. Train-mode BN needs global batch
stats -> three tiny AllReduces (per-channel sum/sumsq), each overlapped with
surrounding compute (fc2 is interleaved into conv2's slab loop).

All matmul operands are fp16 (fp32 PSUM accumulation, fp32 BN statistics,
fp32 output): fp32 moving operands stream at ~half rate through the PE
array, fp16 streams at full rate with ample mantissa for this tolerance.
All conv outputs stay SBUF-resident; weights are host-pre-tiled so every
DMA is a single contiguous 2D transfer.

Host-side precompute: input/weight transposes + fp16 casts, spline-weight
sum (sum_k sw[:,:,k]/K == one GEMM), channel_scale folded into fus_w1,
fus_w2+b2 folded into fc3 (W3_eff = W3 @ W2, b3_eff = W3 @ b2), conv-bias
folded into the BN affine.
"""

import numpy as np

F16 = np.dtype(np.float16)

K_SPLINE = 10
EPS = 1e-5
HH = 64
PW = 66           # padded row stride (64 + 2 zero border)
PAREA = PW * PW   # 4356
NPIX = HH * HH    # 4096 pixels per image
R = 2 * NPIX      # rows per core (2 images)
CIN = 512
LOW = 128
FULL = 256
CAT = 384
FUSH = 192
COUT = 512

TAPS = [(di, dj) for di in (-1, 0, 1) for dj in (-1, 0, 1)]
DVE_TAP = 0                           # fused into the PSUM-combine stt
PE_TAPS = [t for t in range(9) if t != DVE_TAP]
NBLKS = [1, 2, 4]

_CACHE = {}


def _smalls_layout():
    """Column layout of the packed [128, N] fp32 constants tensor."""
    col = 0
    lay = {}
    lay["rs"] = col; col += 1
    for ci, nblk in enumerate(NBLKS):
        for b in range(nblk):
            for nm in ("g", "be", "bb"):
                lay[f"bn{ci}{nm}{b}"] = col; col += 1
    for ci, nblk in enumerate(NBLKS):
        for b in range(nblk):
            lay[f"wv{ci}{b}"] = col; col += 9
    lay["bf1a"] = col; col += 1
    lay["bf1b"] = col; col += 1
    for m in range(4):
        lay[f"b3b{m}"] = col; col += 1
    for m in range(4):
        lay[f"b3s{m}"] = col; col += 1
    return lay, col


# ---------------------------------------------------------------- host prep

def _prep_shared(inp):
    """All non-x device tensors (replicated across cores), as numpy 2D."""
    f = lambda a: np.ascontiguousarray(np.asarray(a, dtype=np.float32))
    h = lambda a: np.ascontiguousarray(
        np.asarray(a, dtype=np.float32).astype(F16))
    sws = lambda sw: np.asarray(sw, np.float64).sum(-1) / K_SPLINE

    fc1_low_bw = f(inp["fc1_low_bw"]); s1l = f(sws(inp["fc1_low_sw"]))
    fc1_full_bw = f(inp["fc1_full_bw"]); s1f = f(sws(inp["fc1_full_sw"]))
    fc2_bw = f(inp["fc2_low_bw"]); s2 = f(sws(inp["fc2_low_sw"]))
    fc3_bw = f(inp["fc3_bw"]); s3 = f(sws(inp["fc3_sw"]))
    w1 = f(inp["fus_w1"]); b1 = f(inp["fus_b1"])
    w2 = f(inp["fus_w2"]); b2 = f(inp["fus_b2"])
    cs = f(inp["channel_scale"])

    d = {}
    # stage A lhsT tiles (k,m) of [512, 768] packed as [128, 24*128]
    # m-blocks: [lowb, lows, fullb0, fullb1, fulls0, fulls1]
    wA = np.concatenate([fc1_low_bw.T, s1l.T, fc1_full_bw.T, s1f.T], axis=1)
    wAt = np.empty((128, 24 * 128), np.float32)
    for k in range(4):
        for m in range(6):
            wAt[:, (k * 6 + m) * 128:(k * 6 + m + 1) * 128] = \
                wA[k * 128:(k + 1) * 128, m * 128:(m + 1) * 128]
    d["wA"] = h(wAt)
    d["wfc2"] = h(np.concatenate([fc2_bw.T, s2.T], axis=1))      # [128, 256]
    wfus1 = (w1 * cs[None, :]).T                                 # [384, 192]
    wf1t = np.empty((128, 3 * FUSH), np.float32)
    for k in range(3):
        wf1t[:, k * FUSH:(k + 1) * FUSH] = wfus1[k * 128:(k + 1) * 128, :]
    d["wfus1"] = h(wf1t)                                         # [128, 576]
    w3b = fc3_bw @ w2                                            # [512, 192]
    w3s = s3 @ w2
    d["wfc3"] = h(np.concatenate([w3b.T, w3s.T], axis=1))        # [192, 1024]
    b3b = (fc3_bw @ b2).reshape(-1)
    b3s = (s3 @ b2).reshape(-1)

    lay, ncols = _smalls_layout()
    sm = np.zeros((128, ncols), np.float32)
    sm[:, lay["rs"]] = float(np.asarray(inp["res_scale"]).reshape(-1)[0])
    sm[0:128, lay["bf1a"]] = b1[0:128]
    sm[0:64, lay["bf1b"]] = b1[128:192]
    for m in range(4):
        sm[:, lay[f"b3b{m}"]] = b3b[m * 128:(m + 1) * 128]
        sm[:, lay[f"b3s{m}"]] = b3s[m * 128:(m + 1) * 128]

    # depthwise convs: diag tiles (b,t) packed as [128, nblk*9*128]
    for ci, (wname, gname, bname, bbname, Cc) in enumerate([
            ("dw1_w", "dw1_g", "dw1_beta", "dw1_b", LOW),
            ("dw2_w", "dw2_g", "dw2_beta", "dw2_b", FULL),
            ("dw3_w", "dw3_g", "dw3_beta", "dw3_b", COUT)]):
        w = f(inp[wname]).reshape(Cc, 9)                          # [C, taps]
        g = f(inp[gname]).reshape(-1)
        be = f(inp[bname]).reshape(-1)
        bb = f(inp[bbname]).reshape(-1)
        nblk = Cc // 128
        diag = np.zeros((128, nblk * 9 * 128), np.float32)
        for b in range(nblk):
            rows = slice(b * 128, (b + 1) * 128)
            for t in range(9):
                c0 = (b * 9 + t) * 128
                diag[:, c0:c0 + 128] = np.diag(w[rows, t])
            c = lay[f"wv{ci}{b}"]
            sm[:, c:c + 9] = w[rows]
            sm[:, lay[f"bn{ci}g{b}"]] = g[rows]
            sm[:, lay[f"bn{ci}be{b}"]] = be[rows]
            sm[:, lay[f"bn{ci}bb{b}"]] = bb[rows]
        d[f"diag{ci+1}"] = h(diag)
    d["smalls"] = np.ascontiguousarray(sm)
    return d


def _prep_x(x, n_cores, scale=None):
    """Per-core transposed fp16 shards [512, 8192]."""
    x = np.asarray(x, np.float32)
    if scale is not None:
        x = x * scale
    x = x.astype(F16)
    B = x.shape[0]
    per = B // n_cores
    return [np.ascontiguousarray(
        x[c * per:(c + 1) * per].reshape(per * NPIX, CIN).T)
        for c in range(n_cores)]


# ---------------------------------------------------------------- builder

def _build(n_cores):
    import concourse.bacc as bacc
    import concourse.mybir as mybir
    import concourse.tile as tile

    f32 = mybir.dt.float32
    f16 = mybir.dt.float16

    nc = bacc.Bacc("TRN2", target_bir_lowering=False, debug=False,
                   num_devices=n_cores)

    def din(name, shape, dt=f16):
        return nc.dram_tensor(name, list(shape), dt, kind="ExternalInput").ap()

    x_d = din("x_t", (CIN, R))
    xs_d = din("xs_t", (CIN, R))
    wA_d = din("wA", (128, 24 * 128))
    wfc2_d = din("wfc2", (128, 256))
    wfus1_d = din("wfus1", (128, 3 * FUSH))
    wfc3_d = din("wfc3", (FUSH, 1024))
    lay, ncols = _smalls_layout()
    smalls_d = din("smalls", (128, ncols), f32)
    conv_d = []
    for ci, Cc in [(1, LOW), (2, FULL), (3, COUT)]:
        nblk = Cc // 128
        conv_d.append(dict(
            diag=din(f"diag{ci}", (128, nblk * 9 * 128)),
            nblk=nblk))
    out_d = nc.dram_tensor("out_t", [COUT, R], f16, kind="ExternalOutput").ap()

    with tile.TileContext(nc) as tc:
        _emit(nc, tc, mybir, n_cores, x_d, xs_d, wA_d, wfc2_d, wfus1_d,
              wfc3_d, conv_d, smalls_d, lay, ncols, out_d)
    nc.compile()
    return nc


def _emit(nc, tc, mybir, n_cores, x_d, xs_d, wA_d, wfc2_d, wfus1_d,
          wfc3_d, conv_d, smalls_d, lay, ncols, out_d):
    f32 = mybir.dt.float32
    f16 = mybir.dt.float16
    AL = mybir.AluOpType
    AF = mybir.ActivationFunctionType
    inv_n = 1.0 / (n_cores * R)

    class _Pools:
        def __init__(self, tc):
            self.tc = tc
            self.cms = {}
            self.order = []
        def open(self, name, **kw):
            cm = self.tc.tile_pool(name=name, **kw)
            pool = cm.__enter__()
            self.cms[name] = cm
            self.order.append(name)
            return pool
        def close(self, *names):
            names = sorted(names, key=self.order.index, reverse=True)
            for n in names:
                assert n == self.order[-1], (n, self.order)
                self.order.pop()
                self.cms.pop(n).__exit__(None, None, None)
        def close_all(self):
            self.close(*self.order)

    pools = _Pools(tc)

    def pad3(t):
        return t[:].rearrange("p (a b) -> p a b", a=PW)

    # ---------------- persistent small tiles ----------------
    P_pers = pools.open("pers", bufs=1)
    P_tmpv = pools.open("tmpv", bufs=4)
    P_dram = pools.open("dramp", bufs=1, space="DRAM")

    # one DMA for every small constant; everything below is a column slice
    smalls = P_pers.tile([128, ncols], f32, name="smalls", tag="smalls")
    sc = lambda key: smalls[:, lay[key]:lay[key] + 1]

    rs_t = sc("rs")
    bf1a = sc("bf1a")
    bf1b = smalls[0:64, lay["bf1b"]:lay["bf1b"] + 1]
    b3bt = [sc(f"b3b{m}") for m in range(4)]
    b3st = [sc(f"b3s{m}") for m in range(4)]

    w2_sb = P_pers.tile([128, 256], f16, name="w2sb", tag="w2sb")
    wf1_sb = P_pers.tile([128, 3 * FUSH], f16, name="wf1sb", tag="wf1sb")
    w3k = [P_pers.tile([128, 1024], f16, name="w3k0", tag="w3k0"),
           P_pers.tile([64, 1024], f16, name="w3k1", tag="w3k1")]
    zero64 = P_pers.tile([64, 1024], f16, name="zero64", tag="zero64")
    nc.gpsimd.memset(zero64[:], 0.0)
    dummy_w = P_pers.tile([128, 128], f16, name="dummy_w", tag="dummy_w")
    dummy_rhs = P_pers.tile([128, 512], f16, name="dummy_rhs", tag="dummy_rhs")
    nc.gpsimd.memset(dummy_w[:], 0.0)
    nc.gpsimd.memset(dummy_rhs[:], 0.0)

    bn = []  # bn[ci][blk] = dict(g, be, bb(slices), a, b(tiles))
    for ci in range(3):
        blks = []
        for b in range(conv_d[ci]["nblk"]):
            e = {nm: sc(f"bn{ci}{nm}{b}") for nm in ("g", "be", "bb")}
            e["a"] = P_pers.tile([128, 1], f32, name=f"bn{ci}a{b}", tag=f"bn{ci}a{b}")
            e["b"] = P_pers.tile([128, 1], f32, name=f"bn{ci}b{b}", tag=f"bn{ci}b{b}")
            blks.append(e)
        bn.append(blks)

    wv_t = [[smalls[:, lay[f"wv{ci}{b}"]:lay[f"wv{ci}{b}"] + 9]
             for b in range(conv_d[ci]["nblk"])] for ci in range(3)]

    SLAB = 1024                      # conv slab (PSUM-resident px per step)
    NSLAB = NPIX // SLAB             # 4 slabs per image
    Sp, Qp = [], []
    for ci in range(3):
        Sp.append([P_pers.tile([128, 2 * NSLAB], f32, name=f"Sp{ci}{b}",
                               tag=f"Sp{ci}{b}") for b in range(conv_d[ci]["nblk"])])
        Qp.append([P_pers.tile([128, 2 * NSLAB], f32, name=f"Qp{ci}{b}",
                               tag=f"Qp{ci}{b}") for b in range(conv_d[ci]["nblk"])])
    pack1 = P_pers.tile([128, 2], f32, name="pack1", tag="pack1")
    pack2 = P_pers.tile([128, 4], f32, name="pack2", tag="pack2")
    pack3a = P_pers.tile([128, 4], f32, name="pack3a", tag="pack3a")
    pack3b = P_pers.tile([128, 4], f32, name="pack3b", tag="pack3b")
    gst1 = P_pers.tile([128, 2], f32, name="gst1", tag="gst1")
    gst2 = P_pers.tile([128, 4], f32, name="gst2", tag="gst2")
    gst3a = P_pers.tile([128, 4], f32, name="gst3a", tag="gst3a")
    gst3b = P_pers.tile([128, 4], f32, name="gst3b", tag="gst3b")

    # --------- conv emitter: 8 PE taps + fused DVE tap/combine/stats -----
    # zdst[b] = persistent SBUF tile [128, R]; slab written at
    # [:, img*NPIX + s*SLAB : +SLAB]. After each slab, one queued
    # interleave callback is drained (used to overlap fc2 with conv2).
    FUSED_STT = True     # fuse DVE tap + PSUM-combine + Sp into one stt
    ACT_SQUARE = True     # Qp via ACT Square (v1) vs DVE tensor_tensor_reduce

    def emit_conv(ci, pads, P_cps, P_cacc, P_csq, P_diag, zdst, imgs=(0, 1),
                  interleave=None, blocks=None, extra_dve_tap=None):
        nblk = conv_d[ci]["nblk"]
        rows = SLAB // HH
        diag_dram = conv_d[ci]["diag"]
        pe_taps = [t for t in PE_TAPS if t != extra_dve_tap]
        for b in (range(nblk) if blocks is None else blocks):
            dg = P_diag.tile([128, 9 * 128], f16, name="dg", tag="dg")
            nc.sync.dma_start(
                dg[:], diag_dram[:, b * 9 * 128:(b + 1) * 9 * 128])
            for img in imgs:
                p3 = pad3(pads[b][img])
                for s in range(NSLAB):
                    r0 = s * rows
                    ps = P_cps.tile([128, SLAB], f32, name=f"cps{ci}", tag="cps")
                    for ti, t in enumerate(pe_taps):
                        di, dj = TAPS[t]
                        rhs = p3[:, 1 + di + r0:1 + di + r0 + rows,
                                 1 + dj:1 + dj + HH]
                        for nn in range(SLAB // 512):
                            rr = nn * (512 // HH)
                            nc.tensor.matmul(
                                ps[:, nn * 512:(nn + 1) * 512],
                                dg[:, t * 128:(t + 1) * 128],
                                rhs[:, rr:rr + (512 // HH), :],
                                start=(ti == 0), stop=(ti == len(pe_taps) - 1))
                    slot = img * NSLAB + s
                    col = img * NPIX + s * SLAB
                    zsl = zdst[b][:, col:col + SLAB]
                    psum_in = ps[:].rearrange("p (a b) -> p a b", a=rows)
                    if extra_dve_tap is not None:
                        di, dj = TAPS[extra_dve_tap]
                        acc = P_cacc.tile([128, SLAB], f16, name="cacc", tag="cacc")
                        nc.vector.scalar_tensor_tensor(
                            acc[:].rearrange("p (a b) -> p a b", a=rows),
                            p3[:, 1 + di + r0:1 + di + r0 + rows,
                               1 + dj:1 + dj + HH],
                            wv_t[ci][b][:, extra_dve_tap:extra_dve_tap + 1],
                            psum_in, op0=AL.mult, op1=AL.add)
                        psum_in = acc[:].rearrange("p (a b) -> p a b", a=rows)
                    di, dj = TAPS[DVE_TAP]
                    tap_ap = p3[:, 1 + di + r0:1 + di + r0 + rows,
                                1 + dj:1 + dj + HH]
                    if FUSED_STT:
                        nc.vector.scalar_tensor_tensor(
                            zsl.rearrange("p (a b) -> p a b", a=rows),
                            tap_ap,
                            wv_t[ci][b][:, DVE_TAP:DVE_TAP + 1],
                            psum_in,
                            op0=AL.mult, op1=AL.add,
                            accum_out=Sp[ci][b][:, slot:slot + 1])
                    else:
                        acc = P_cacc.tile([128, SLAB], f16, name="cacc", tag="cacc")
                        nc.vector.tensor_scalar(
                            acc[:].rearrange("p (a b) -> p a b", a=rows),
                            tap_ap, wv_t[ci][b][:, DVE_TAP:DVE_TAP + 1], None,
                            op0=AL.mult)
                        nc.vector.scalar_tensor_tensor(
                            zsl, acc[:], 0.0, ps[:], op0=AL.bypass, op1=AL.add,
                            accum_out=Sp[ci][b][:, slot:slot + 1])
                    sq = P_csq.tile([128, SLAB], f16, name="sqs", tag="sqs")
                    if ACT_SQUARE:
                        nc.scalar.activation(sq[:], zsl, AF.Square,
                                             accum_out=Qp[ci][b][:, slot:slot + 1])
                    else:
                        nc.vector.tensor_tensor_reduce(
                            sq[:], zsl, zsl, 1.0, 0.0, op0=AL.mult, op1=AL.add,
                            accum_out=Qp[ci][b][:, slot:slot + 1])
                    if interleave:
                        interleave.pop(0)()

    def open_conv_pools(sfx):
        return (pools.open(f"cps{sfx}", bufs=2, space="PSUM"),
                pools.open(f"cacc{sfx}", bufs=2),
                pools.open(f"csq{sfx}", bufs=2),
                pools.open(f"diag{sfx}", bufs=2))

    def close_conv_pools(sfx):
        pools.close(f"diag{sfx}", f"csq{sfx}", f"cacc{sfx}", f"cps{sfx}")

    def bn_math(ci, b, S_ap, Q_ap):
        e = bn[ci][b]
        tt = lambda tag: P_tmpv.tile([128, 1], f32, name=tag, tag=tag)
        m = tt("bnm"); e2 = tt("bne"); m2 = tt("bnm2"); v = tt("bnv")
        sq = tt("bnsq"); iv = tt("bniv"); mb = tt("bnmb"); ab = tt("bnab")
        nc.vector.tensor_scalar(m[:], S_ap, inv_n, None, op0=AL.mult)
        nc.vector.tensor_scalar(e2[:], Q_ap, inv_n, None, op0=AL.mult)
        nc.vector.tensor_tensor(m2[:], m[:], m[:], op=AL.mult)
        nc.vector.tensor_tensor(v[:], e2[:], m2[:], op=AL.subtract)
        nc.vector.tensor_scalar(v[:], v[:], EPS, None, op0=AL.add)
        nc.scalar.activation(sq[:], v[:], AF.Sqrt)
        nc.vector.reciprocal(iv[:], sq[:])
        nc.vector.tensor_tensor(e["a"][:], e["g"], iv[:], op=AL.mult)
        nc.vector.tensor_tensor(mb[:], m[:], e["bb"], op=AL.add)
        nc.vector.tensor_tensor(ab[:], e["a"][:], mb[:], op=AL.mult)
        nc.vector.tensor_tensor(e["b"][:], e["be"], ab[:], op=AL.subtract)

    def allreduce(pack, gst, ncols, tag):
        if n_cores == 1:
            nc.vector.tensor_copy(gst[:], pack[:])
            return
        ib = P_dram.tile([128, ncols], f32, name=f"cc_in{tag}", tag=f"cc_in{tag}")
        ob = P_dram.tile([128, ncols], f32, name=f"cc_out{tag}", tag=f"cc_out{tag}")
        nc.gpsimd.dma_start(ib[:], pack[:])
        nc.gpsimd.collective_compute(
            "AllReduce", AL.add,
            replica_groups=[list(range(n_cores))],
            ins=[ib.opt()], outs=[ob.opt()])
        nc.gpsimd.dma_start(gst[:], ob[:])

    def reduce_stats(pack, cols):
        for i, (ci, b) in enumerate(cols):
            nc.vector.tensor_reduce(pack[:, 2 * i:2 * i + 1], Sp[ci][b][:],
                                    axis=mybir.AxisListType.X, op=AL.add)
            nc.vector.tensor_reduce(pack[:, 2 * i + 1:2 * i + 2], Qp[ci][b][:],
                                    axis=mybir.AxisListType.X, op=AL.add)

    # persistent SBUF activations (fp16)
    P_hf = pools.open("hfp", bufs=1)
    hf1a = P_hf.tile([128, R], f16, name="hf1a", tag="hf1a")
    hf1b = P_hf.tile([64, R], f16, name="hf1b", tag="hf1b")

    # =================== stage A: fc1_low + fc1_full ==================
    P_z12 = pools.open("z12p", bufs=1)
    z1_sb = [P_z12.tile([128, R], f16, name="z1sb", tag="z1sb")]
    z2_sb = [P_z12.tile([128, R], f16, name=f"z2sb{b}", tag=f"z2sb{b}")
             for b in range(2)]
    yl_sb = P_z12.tile([128, R], f16, name="ylsb", tag="ylsb")

    P_pad2 = pools.open("pads2", bufs=1)
    P_pad1 = pools.open("pads1", bufs=1)
    y1p = [P_pad1.tile([128, PAREA], f16, name=f"y1p{i}", tag=f"y1p{i}")
           for i in range(2)]
    y2p = [[P_pad2.tile([128, PAREA], f16, name=f"y2p{b}{i}", tag=f"y2p{b}{i}")
            for i in range(2)] for b in range(2)]
    for t in y1p:
        nc.vector.memset(t[:], 0.0)
    for i in range(2):            # image-major: img0 pads ready first
        for b in range(2):
            nc.gpsimd.memset(y2p[b][i][:], 0.0)

    P_wA = pools.open("wAp", bufs=1)
    P_xk = pools.open("xk", bufs=2)
    P_tmpA = pools.open("tmpA", bufs=2)
    P_psA = pools.open("psA", bufs=2, space="PSUM")
    wA_sb = P_wA.tile([128, 24 * 128], f16, name="wAsb", tag="wAsb")
    wAt = lambda k, m: wA_sb[:, (k * 6 + m) * 128:(k * 6 + m + 1) * 128]
    pairs = [(0, 1, lambda img: y1p[img]),
             (2, 4, lambda img: y2p[0][img]),
             (3, 5, lambda img: y2p[1][img])]
    for ch in range(8):
        img, lrow = ch // 4, (ch % 4) * 16
        xs = []
        for k in range(4):
            xt = P_xk.tile([128, 1024], f16, name=f"xk{k}", tag=f"xk{k}")
            eng = nc.sync if k % 2 == 0 else nc.scalar
            eng.dma_start(
                xt[:], x_d[k * 128:(k + 1) * 128, ch * 1024:(ch + 1) * 1024])
            xs.append(xt)
        if ch == 0:
            nc.sync.dma_start(wA_sb[:], wA_d[:])
        for bm, sm, dest in pairs:
            psB = P_psA.tile([128, 1024], f32, name="psB", tag="psB")
            psS = P_psA.tile([128, 1024], f32, name="psS", tag="psS")
            for k in range(4):
                for nn in range(2):
                    sl = slice(nn * 512, (nn + 1) * 512)
                    nc.tensor.matmul(psB[:, sl], wAt(k, bm), xs[k][:, sl],
                                     start=(k == 0), stop=(k == 3))
                    nc.tensor.matmul(psS[:, sl], wAt(k, sm), xs[k][:, sl],
                                     start=(k == 0), stop=(k == 3))
            tmp = P_tmpA.tile([128, 1024], f16, name="siluA", tag="siluA")
            nc.scalar.activation(tmp[:], psB[:], AF.Silu)
            outap = pad3(dest(img))[:, 1 + lrow:1 + lrow + 16, 1:65]
            nc.vector.scalar_tensor_tensor(
                outap,
                psS[:].rearrange("p (a b) -> p a b", a=16),
                0.0,
                tmp[:].rearrange("p (a b) -> p a b", a=16),
                op0=AL.bypass, op1=AL.add)
    pools.close("psA", "tmpA", "xk", "wAp")
    nc.scalar.dma_start(smalls[:], smalls_d[:])
    nc.scalar.dma_start(w2_sb[:], wfc2_d[:])
    nc.scalar.dma_start(wf1_sb[:], wfus1_d[:])
    nc.scalar.dma_start(w3k[0][:], wfc3_d[0:128, :])
    nc.scalar.dma_start(w3k[1][:], wfc3_d[128:192, :])

    # ============ conv1 -> AR1 (overlapped with conv2+fc2) =============
    cpools12 = open_conv_pools("c12")
    emit_conv(0, [y1p], *cpools12, zdst=z1_sb, extra_dve_tap=8)
    reduce_stats(pack1, [(0, 0)])
    allreduce(pack1, gst1, 2, "a1")

    # fc2 work units, interleaved into conv2's 16 slab iterations
    P_t2 = pools.open("fc2t", bufs=2)
    P_ps2 = pools.open("psF2", bufs=1, space="PSUM")

    def fc2_chunk(ch):
        sl = slice(ch * 1024, (ch + 1) * 1024)
        z1b = P_t2.tile([128, 1024], f16, name="z1b", tag="z1b")
        nc.scalar.activation(z1b[:], z1_sb[0][:, sl], AF.Relu,
                             bias=bn[0][0]["b"][:], scale=bn[0][0]["a"][:])
        psB = P_ps2.tile([128, 1024], f32, name="ps2B", tag="ps2B")
        psS = P_ps2.tile([128, 1024], f32, name="ps2S", tag="ps2S")
        for nn in range(2):
            s2 = slice(nn * 512, (nn + 1) * 512)
            nc.tensor.matmul(psB[:, s2], w2_sb[:, 0:128], z1b[:, s2],
                             start=True, stop=True)
            nc.tensor.matmul(psS[:, s2], w2_sb[:, 128:256], z1b[:, s2],
                             start=True, stop=True)
        tmp = P_t2.tile([128, 1024], f16, name="silu2", tag="silu2")
        nc.scalar.activation(tmp[:], psB[:], AF.Silu)
        nc.vector.scalar_tensor_tensor(yl_sb[:, sl], psS[:], 0.0, tmp[:],
                                       op0=AL.bypass, op1=AL.add)

    INTERLEAVE_FC2 = True
    if INTERLEAVE_FC2:
        todo = [lambda: bn_math(0, 0, gst1[:, 0:1], gst1[:, 1:2])]
        todo += [lambda ch=ch: fc2_chunk(ch) for ch in range(8)]
        ilv = [lambda: None] * 5 + todo
        ilv += [lambda: None] * (16 - len(ilv))
    else:
        ilv = None

    emit_conv(1, y2p, *cpools12, zdst=z2_sb, interleave=ilv)
    if not INTERLEAVE_FC2:
        bn_math(0, 0, gst1[:, 0:1], gst1[:, 1:2])
        for ch in range(8):
            fc2_chunk(ch)
    pools.close("psF2", "fc2t")
    close_conv_pools("c12")
    pools.close("pads1", "pads2")
    reduce_stats(pack2, [(1, 0), (1, 1)])
    allreduce(pack2, gst2, 4, "a2")
    bn_math(1, 0, gst2[:, 0:1], gst2[:, 1:2])
    bn_math(1, 1, gst2[:, 2:3], gst2[:, 3:4])

    # keep the PE array busy through the AR2 collective so the HAM clock
    # gate stays at 8/8 into fusion/fc3 (a >3.4us PE-idle window would
    # re-throttle to 1.2 GHz for tens of us). Garbage-in, never-read-out.
    P_warm = pools.open("pswarm", bufs=1, space="PSUM")
    wps = P_warm.tile([128, 512], f32, name="wps", tag="wps")
    NWARM = 130
    for i in range(NWARM):
        nc.tensor.matmul(wps[:], dummy_w[:], dummy_rhs[:],
                         start=(i == 0), stop=(i == NWARM - 1),
                         skip_group_check=True)
    pools.close("pswarm")

    # =================== fusion linear 1 -> hf1 (SBUF) ==================
    P_tf1 = pools.open("fu1t", bufs=3)
    P_psf1 = pools.open("psFu1", bufs=2, space="PSUM")
    wf1t = lambda k, m: wf1_sb[:, k * FUSH + m * 128:k * FUSH + m * 128 + (64 if m else 128)]
    for ch in range(8):
        sl = slice(ch * 1024, (ch + 1) * 1024)
        z2b0 = P_tf1.tile([128, 1024], f16, name="z2b0", tag="z2b0")
        z2b1 = P_tf1.tile([128, 1024], f16, name="z2b1", tag="z2b1")
        nc.scalar.activation(z2b0[:], z2_sb[0][:, sl], AF.Relu,
                             bias=bn[1][0]["b"][:], scale=bn[1][0]["a"][:])
        nc.vector.tensor_scalar(z2b1[:], z2_sb[1][:, sl], bn[1][1]["a"][:],
                                bn[1][1]["b"][:], op0=AL.mult, op1=AL.add)
        nc.vector.tensor_scalar(z2b1[:], z2b1[:], 0.0, None, op0=AL.max)
        rhs = [yl_sb[:, sl], z2b0[:], z2b1[:]]
        ps0 = P_psf1.tile([128, 1024], f32, name="psf1a", tag="psf1a")
        ps1 = P_psf1.tile([64, 1024], f32, name="psf1b", tag="psf1b")
        for k in range(3):
            for nn in range(2):
                s2 = slice(nn * 512, (nn + 1) * 512)
                nc.tensor.matmul(ps0[:, s2], wf1t(k, 0), rhs[k][:, s2],
                                 start=(k == 0), stop=(k == 2))
                nc.tensor.matmul(ps1[:, s2], wf1t(k, 1), rhs[k][:, s2],
                                 start=(k == 0), stop=(k == 2))
        nc.scalar.activation(hf1a[:, sl], ps0[:], AF.Relu, bias=bf1a)
        nc.vector.scalar_tensor_tensor(hf1b[:, sl], ps1[:], bf1b, zero64[:],
                                       op0=AL.add, op1=AL.max)
    pools.close("psFu1", "fu1t")
    pools.close("z12p")

    # ============= fc3' + conv3 (block-split stats) + finals ============
    P_z3 = pools.open("z3p", bufs=1)
    z3_sb = [P_z3.tile([128, R], f16, name=f"z3sb{b}", tag=f"z3sb{b}")
             for b in range(4)]
    P_h3 = pools.open("h3p", bufs=1)
    P_t3 = pools.open("fc3t", bufs=3)
    P_ps3 = pools.open("psF3", bufs=2, space="PSUM")
    P_xc = pools.open("xcp", bufs=3)
    P_fin = pools.open("fint", bufs=3)
    cpools3 = open_conv_pools("c3")

    def fin_chunk(b, ch, dve_bn=True):
        rows = slice(b * 128, (b + 1) * 128)
        sl = slice(ch * 2048, (ch + 1) * 2048)
        xc = P_xc.tile([128, 2048], f16, name="xc", tag="xc")
        nc.scalar.dma_start(xc[:], xs_d[rows, sl])
        t = P_fin.tile([128, 2048], f16, name="trelu", tag="trelu")
        if dve_bn and ch == 3:   # balance: every 4th BN+ReLU on the DVE
            nc.vector.tensor_scalar(t[:], z3_sb[b][:, sl],
                                    bn[2][b]["a"][:], bn[2][b]["b"][:],
                                    op0=AL.mult, op1=AL.add)
            nc.vector.tensor_scalar(t[:], t[:], 0.0, None, op0=AL.max)
        else:
            nc.scalar.activation(t[:], z3_sb[b][:, sl], AF.Relu,
                                 bias=bn[2][b]["b"][:], scale=bn[2][b]["a"][:])
        ob = P_fin.tile([128, 2048], f16, name="ob", tag="ob")
        nc.vector.tensor_tensor(ob[:], xc[:], t[:], op=AL.add)
        nc.sync.dma_start(out_d[rows, sl], ob[:])

    h3sets = []
    for img in range(2):
        h3 = [P_h3.tile([128, PAREA], f16, name=f"h3p{b}", tag=f"h3p{b}")
              for b in range(4)]
        h3sets.append(h3)
        for t in h3:
            nc.gpsimd.memset(t[:], 0.0)
        for ch in range(8):           # 512-px chunks within image
            r0 = ch * 8
            sl = slice(img * NPIX + ch * 512, img * NPIX + (ch + 1) * 512)
            rhs = [hf1a[:, sl], hf1b[:, sl]]
            for mp in range(4):
                psB = P_ps3.tile([128, 512], f32, name="ps3B", tag="ps3B")
                psS = P_ps3.tile([128, 512], f32, name="ps3S", tag="ps3S")
                for kk in range(2):
                    nc.tensor.matmul(psB[:], w3k[kk][:, mp * 128:(mp + 1) * 128],
                                     rhs[kk], start=(kk == 0), stop=(kk == 1))
                    nc.tensor.matmul(psS[:], w3k[kk][:, (4 + mp) * 128:(5 + mp) * 128],
                                     rhs[kk], start=(kk == 0), stop=(kk == 1))
                tmp = P_t3.tile([128, 512], f16, name="silu3", tag="silu3")
                nc.scalar.activation(tmp[:], psB[:], AF.Silu, bias=b3bt[mp])
                outap = pad3(h3[mp])[:, 1 + r0:1 + r0 + 8, 1:65]
                nc.vector.scalar_tensor_tensor(
                    outap,
                    psS[:].rearrange("p (a b) -> p a b", a=8),
                    b3st[mp],
                    tmp[:].rearrange("p (a b) -> p a b", a=8),
                    op0=AL.add, op1=AL.add)
        if img == 0:
            emit_conv(2, [{0: h3[b]} for b in range(4)], *cpools3,
                      zdst=z3_sb, imgs=(0,), extra_dve_tap=8)
    # conv3 img1: blocks 0-1, then AR3a fires while blocks 2-3 conv and
    # the finals for blocks 0-1 interleave into their slab loop.
    emit_conv(2, [{1: h3sets[1][b]} for b in range(4)], *cpools3,
              zdst=z3_sb, imgs=(1,), blocks=(0, 1), extra_dve_tap=8)
    reduce_stats(pack3a, [(2, 0), (2, 1)])
    allreduce(pack3a, gst3a, 4, "a3a")
    # bn_math + the first finals are staggered into conv3 blocks 2-3 via the
    # interleave hooks so their AR3a-gated ops never head-of-line-block the
    # DVE/ACT queues ahead of conv3's own slab work.
    ilv3 = [lambda: None] * 3
    ilv3.append(lambda: bn_math(2, 0, gst3a[:, 0:1], gst3a[:, 1:2]))
    ilv3.append(lambda: bn_math(2, 1, gst3a[:, 2:3], gst3a[:, 3:4]))
    ilv3 += [lambda ch=ch: fin_chunk(0, ch, dve_bn=False) for ch in range(3)]
    emit_conv(2, [{1: h3sets[1][b]} for b in range(4)], *cpools3,
              zdst=z3_sb, imgs=(1,), blocks=(2, 3), interleave=ilv3)
    fin_chunk(0, 3, dve_bn=False)
    for ch in range(4):
        fin_chunk(1, ch, dve_bn=False)
    reduce_stats(pack3b, [(2, 2), (2, 3)])
    allreduce(pack3b, gst3b, 4, "a3b")
    bn_math(2, 2, gst3b[:, 0:1], gst3b[:, 1:2])
    bn_math(2, 3, gst3b[:, 2:3], gst3b[:, 3:4])
    for b in (2, 3):
        for ch in range(4):
            fin_chunk(b, ch)
    pools.close_all()


def _get_built(n_cores):
    if n_cores not in _CACHE:
        _CACHE[n_cores] = _build(n_cores)
    return _CACHE[n_cores]


def make_in_maps(inputs, n_cores):
    shared = _prep_shared(inputs)
    xt = _prep_x(inputs["x"], n_cores)
    rsv = float(np.asarray(inputs["res_scale"]).reshape(-1)[0])
    xst = _prep_x(inputs["x"], n_cores, scale=rsv)
    return [dict(shared, x_t=xt[c], xs_t=xst[c]) for c in range(n_cores)]


def kernel(**inputs):
    from concourse.bass_utils import run_bass_kernel_spmd

    assert int(np.asarray(inputs["H"])) == HH and int(np.asarray(inputs["W"])) == HH
    n_cores = 8
    nc = _get_built(n_cores)
    in_maps = make_in_maps(inputs, n_cores)
    res = run_bass_kernel_spmd(nc, in_maps, core_ids=list(range(n_cores)))
    B = np.asarray(inputs["x"]).shape[0]
    per = B // n_cores
    out = np.empty((B, NPIX, CIN), np.float32)
    for c in range(n_cores):
        out[c * per:(c + 1) * per] = \
            res.results[c]["out_t"].astype(np.float32).T.reshape(per, NPIX, CIN)
    return out


# ------------------------------------------------------------- profiling

def _install_ntff_hook():
    """The agent image's antenv lacks axon_hooks; recreate the NTFF profile
    hook via ctypes on the axon PJRT .so (same ABI as trn_boot's)."""
    import contextlib, ctypes, sys, types
    so = "/opt/axon/libaxon_pjrt.so"
    try:
        import antenv.axon_hooks  # noqa: F401
        return True
    except ImportError:
        pass
    try:
        lib = ctypes.CDLL(so)
    except OSError:
        return False
    if not hasattr(lib, "axon_start_nrt_profile"):
        return False
    lib.axon_start_nrt_profile.argtypes = [
        ctypes.POINTER(ctypes.c_int64), ctypes.c_size_t]
    lib.axon_start_nrt_profile.restype = ctypes.c_int64
    lib.axon_stop_nrt_profile.argtypes = [ctypes.c_char_p]
    lib.axon_stop_nrt_profile.restype = ctypes.c_int64

    @contextlib.contextmanager
    def _hook(output_dir, device_ids):
        import jax
        jax.devices()
        if device_ids:
            ids = (ctypes.c_int64 * len(device_ids))(*device_ids)
            rc = lib.axon_start_nrt_profile(ids, len(device_ids))
        else:
            rc = lib.axon_start_nrt_profile(None, 0)
        if rc != 0:
            raise RuntimeError(f"axon_start_nrt_profile rc={rc}")
        try:
            yield
        finally:
            n = lib.axon_stop_nrt_profile(str(output_dir).encode())
            print(f"profile: {n} ntff file(s) -> {output_dir}", file=sys.stderr)

    mod = types.ModuleType("antenv.axon_hooks")
    mod.get_axon_ntff_profile_hook = lambda: _hook
    mod.set_axon_ntff_profile_hook = lambda h: None
    sys.modules["antenv.axon_hooks"] = mod
    import concourse.bass_utils as bu
    bu.upload_artifacts = lambda tmpdir: f"local:{tmpdir}"
    return True


def benchmark(inputs, iters=2, tmpdir=None):
    """Device-only HW execution time (ns) via neuron-profile NTFF trace."""
    import os, tempfile
    from concourse.bass_utils import run_bass_kernel_spmd

    if not _install_ntff_hook():
        raise RuntimeError("NTFF profiling unavailable")
    if tmpdir:
        os.makedirs(tmpdir, exist_ok=True)
    n_cores = 8
    nc = _get_built(n_cores)
    in_maps = make_in_maps(inputs, n_cores)
    times = []
    for i in range(max(1, min(iters, 3))):
        td = tempfile.mkdtemp(dir=tmpdir) if tmpdir else None
        res = run_bass_kernel_spmd(nc, in_maps, core_ids=list(range(n_cores)),
                                   trace=True, tmpdir=td)
        if res.exec_time_ns is not None:
            times.append(res.exec_time_ns)
    if not times:
        raise RuntimeError("no exec_time_ns from traced runs")
    return min(times)


# revision 39
# speedup vs baseline: 1.0580x; 1.0047x over previous
"""Trainium2 Bass kernel for nn_ConvLinearLayer (KAN-style conv-linear block).

Strategy
--------
Data-parallel over batch: 16 images -> 8 cores x 2 images. All activations
live on-chip in transposed layout [channels(partitions), pixels(free)], so
GEMMs (PE, fp16), depthwise 3x3 convs (8 PE diag-matmul taps + 1 fused DVE
tap), BN stats (accumulator outputs) and BN-apply+ReLU (ACT, per-partition
scale/bias) all hit their natural axes. Train-mode BN needs global batch
stats -> three tiny AllReduces (per-channel sum/sumsq), each overlapped with
surrounding compute (fc2 is interleaved into conv2's slab loop).

All matmul operands are fp16 (fp32 PSUM accumulation, fp32 BN statistics,
fp32 output): fp32 moving operands stream at ~half rate through the PE
array, fp16 streams at full rate with ample mantissa for this tolerance.
All conv outputs stay SBUF-resident; weights are host-pre-tiled so every
DMA is a single contiguous 2D transfer.

Host-side precompute: input/weight transposes + fp16 casts, spline-weight
sum (sum_k sw[:,:,k]/K == one GEMM), channel_scale folded into fus_w1,
fus_w2+b2 folded into fc3 (W3_eff = W3 @ W2, b3_eff = W3 @ b2), conv-bias
folded into the BN affine.
"""

import numpy as np

F16 = np.dtype(np.float16)

K_SPLINE = 10
EPS = 1e-5
HH = 64
PW = 66           # padded row stride (64 + 2 zero border)
PAREA = PW * PW   # 4356
NPIX = HH * HH    # 4096 pixels per image
R = 2 * NPIX      # rows per core (2 images)
CIN = 512
LOW = 128
FULL = 256
CAT = 384
FUSH = 192
COUT = 512

TAPS = [(di, dj) for di in (-1, 0, 1) for dj in (-1, 0, 1)]
DVE_TAP = 0                           # fused into the PSUM-combine stt
PE_TAPS = [t for t in range(9) if t != DVE_TAP]
NBLKS = [1, 2, 4]

_CACHE = {}


def _smalls_layout():
    """Column layout of the packed [128, N] fp32 constants tensor."""
    col = 0
    lay = {}
    lay["rs"] = col; col += 1
    for ci, nblk in enumerate(NBLKS):
        for b in range(nblk):
            for nm in ("g", "be", "bb"):
                lay[f"bn{ci}{nm}{b}"] = col; col += 1
    for ci, nblk in enumerate(NBLKS):
        for b in range(nblk):
            lay[f"wv{ci}{b}"] = col; col += 9
    lay["bf1a"] = col; col += 1
    lay["bf1b"] = col; col += 1
    for m in range(4):
        lay[f"b3b{m}"] = col; col += 1
    for m in range(4):
        lay[f"b3s{m}"] = col; col += 1
    return lay, col


# ---------------------------------------------------------------- host prep

def _prep_shared(inp):
    """All non-x device tensors (replicated across cores), as numpy 2D."""
    f = lambda a: np.ascontiguousarray(np.asarray(a, dtype=np.float32))
    h = lambda a: np.ascontiguousarray(
        np.asarray(a, dtype=np.float32).astype(F16))
    sws = lambda sw: np.asarray(sw, np.float64).sum(-1) / K_SPLINE

    fc1_low_bw = f(inp["fc1_low_bw"]); s1l = f(sws(inp["fc1_low_sw"]))
    fc1_full_bw = f(inp["fc1_full_bw"]); s1f = f(sws(inp["fc1_full_sw"]))
    fc2_bw = f(inp["fc2_low_bw"]); s2 = f(sws(inp["fc2_low_sw"]))
    fc3_bw = f(inp["fc3_bw"]); s3 = f(sws(inp["fc3_sw"]))
    w1 = f(inp["fus_w1"]); b1 = f(inp["fus_b1"])
    w2 = f(inp["fus_w2"]); b2 = f(inp["fus_b2"])
    cs = f(inp["channel_scale"])

    d = {}
    # stage A lhsT tiles (k,m) of [512, 768] packed as [128, 24*128]
    # m-blocks: [lowb, lows, fullb0, fullb1, fulls0, fulls1]
    wA = np.concatenate([fc1_low_bw.T, s1l.T, fc1_full_bw.T, s1f.T], axis=1)
    wAt = np.empty((128, 24 * 128), np.float32)
    for k in range(4):
        for m in range(6):
            wAt[:, (k * 6 + m) * 128:(k * 6 + m + 1) * 128] = \
                wA[k * 128:(k + 1) * 128, m * 128:(m + 1) * 128]
    d["wA"] = h(wAt)
    d["wfc2"] = h(np.concatenate([fc2_bw.T, s2.T], axis=1))      # [128, 256]
    wfus1 = (w1 * cs[None, :]).T                                 # [384, 192]
    wf1t = np.empty((128, 3 * FUSH), np.float32)
    for k in range(3):
        wf1t[:, k * FUSH:(k + 1) * FUSH] = wfus1[k * 128:(k + 1) * 128, :]
    d["wfus1"] = h(wf1t)                                         # [128, 576]
    w3b = fc3_bw @ w2                                            # [512, 192]
    w3s = s3 @ w2
    d["wfc3"] = h(np.concatenate([w3b.T, w3s.T], axis=1))        # [192, 1024]
    b3b = (fc3_bw @ b2).reshape(-1)
    b3s = (s3 @ b2).reshape(-1)

    lay, ncols = _smalls_layout()
    sm = np.zeros((128, ncols), np.float32)
    sm[:, lay["rs"]] = float(np.asarray(inp["res_scale"]).reshape(-1)[0])
    sm[0:128, lay["bf1a"]] = b1[0:128]
    sm[0:64, lay["bf1b"]] = b1[128:192]
    for m in range(4):
        sm[:, lay[f"b3b{m}"]] = b3b[m * 128:(m + 1) * 128]
        sm[:, lay[f"b3s{m}"]] = b3s[m * 128:(m + 1) * 128]

    # depthwise convs: diag tiles (b,t) packed as [128, nblk*9*128]
    for ci, (wname, gname, bname, bbname, Cc) in enumerate([
            ("dw1_w", "dw1_g", "dw1_beta", "dw1_b", LOW),
            ("dw2_w", "dw2_g", "dw2_beta", "dw2_b", FULL),
            ("dw3_w", "dw3_g", "dw3_beta", "dw3_b", COUT)]):
        w = f(inp[wname]).reshape(Cc, 9)                          # [C, taps]
        g = f(inp[gname]).reshape(-1)
        be = f(inp[bname]).reshape(-1)
        bb = f(inp[bbname]).reshape(-1)
        nblk = Cc // 128
        diag = np.zeros((128, nblk * 9 * 128), np.float32)
        for b in range(nblk):
            rows = slice(b * 128, (b + 1) * 128)
            for t in range(9):
                c0 = (b * 9 + t) * 128
                diag[:, c0:c0 + 128] = np.diag(w[rows, t])
            c = lay[f"wv{ci}{b}"]
            sm[:, c:c + 9] = w[rows]
            sm[:, lay[f"bn{ci}g{b}"]] = g[rows]
            sm[:, lay[f"bn{ci}be{b}"]] = be[rows]
            sm[:, lay[f"bn{ci}bb{b}"]] = bb[rows]
        d[f"diag{ci+1}"] = h(diag)
    d["smalls"] = np.ascontiguousarray(sm)
    return d


def _prep_x(x, n_cores, scale=None):
    """Per-core transposed fp16 shards [512, 8192]."""
    x = np.asarray(x, np.float32)
    if scale is not None:
        x = x * scale
    x = x.astype(F16)
    B = x.shape[0]
    per = B // n_cores
    return [np.ascontiguousarray(
        x[c * per:(c + 1) * per].reshape(per * NPIX, CIN).T)
        for c in range(n_cores)]


# ---------------------------------------------------------------- builder

def _build(n_cores):
    import concourse.bacc as bacc
    import concourse.mybir as mybir
    import concourse.tile as tile

    f32 = mybir.dt.float32
    f16 = mybir.dt.float16

    nc = bacc.Bacc("TRN2", target_bir_lowering=False, debug=False,
                   num_devices=n_cores)

    def din(name, shape, dt=f16):
        return nc.dram_tensor(name, list(shape), dt, kind="ExternalInput").ap()

    x_d = din("x_t", (CIN, R))
    xs_d = din("xs_t", (CIN, R))
    wA_d = din("wA", (128, 24 * 128))
    wfc2_d = din("wfc2", (128, 256))
    wfus1_d = din("wfus1", (128, 3 * FUSH))
    wfc3_d = din("wfc3", (FUSH, 1024))
    lay, ncols = _smalls_layout()
    smalls_d = din("smalls", (128, ncols), f32)
    conv_d = []
    for ci, Cc in [(1, LOW), (2, FULL), (3, COUT)]:
        nblk = Cc // 128
        conv_d.append(dict(
            diag=din(f"diag{ci}", (128, nblk * 9 * 128)),
            nblk=nblk))
    out_d = nc.dram_tensor("out_t", [COUT, R], f16, kind="ExternalOutput").ap()

    with tile.TileContext(nc) as tc:
        _emit(nc, tc, mybir, n_cores, x_d, xs_d, wA_d, wfc2_d, wfus1_d,
              wfc3_d, conv_d, smalls_d, lay, ncols, out_d)
    nc.compile()
    return nc


def _emit(nc, tc, mybir, n_cores, x_d, xs_d, wA_d, wfc2_d, wfus1_d,
          wfc3_d, conv_d, smalls_d, lay, ncols, out_d):
    f32 = mybir.dt.float32
    f16 = mybir.dt.float16
    AL = mybir.AluOpType
    AF = mybir.ActivationFunctionType
    inv_n = 1.0 / (n_cores * R)

    class _Pools:
        def __init__(self, tc):
            self.tc = tc
            self.cms = {}
            self.order = []
        def open(self, name, **kw):
            cm = self.tc.tile_pool(name=name, **kw)
            pool = cm.__enter__()
            self.cms[name] = cm
            self.order.append(name)
            return pool
        def close(self, *names):
            names = sorted(names, key=self.order.index, reverse=True)
            for n in names:
                assert n == self.order[-1], (n, self.order)
                self.order.pop()
                self.cms.pop(n).__exit__(None, None, None)
        def close_all(self):
            self.close(*self.order)

    pools = _Pools(tc)

    def pad3(t):
        return t[:].rearrange("p (a b) -> p a b", a=PW)

    # ---------------- persistent small tiles ----------------
    P_pers = pools.open("pers", bufs=1)
    P_tmpv = pools.open("tmpv", bufs=4)
    P_dram = pools.open("dramp", bufs=1, space="DRAM")

    # one DMA for every small constant; everything below is a column slice
    smalls = P_pers.tile([128, ncols], f32, name="smalls", tag="smalls")
    sc = lambda key: smalls[:, lay[key]:lay[key] + 1]

    rs_t = sc("rs")
    bf1a = sc("bf1a")
    bf1b = smalls[0:64, lay["bf1b"]:lay["bf1b"] + 1]
    b3bt = [sc(f"b3b{m}") for m in range(4)]
    b3st = [sc(f"b3s{m}") for m in range(4)]

    dg12 = [P_pers.tile([128, conv_d[ci]["nblk"] * 9 * 128], f16,
                        name=f"dg12_{ci}", tag=f"dg12_{ci}") for ci in range(2)]
    w2_sb = P_pers.tile([128, 256], f16, name="w2sb", tag="w2sb")
    wf1_sb = P_pers.tile([128, 3 * FUSH], f16, name="wf1sb", tag="wf1sb")
    w3k = [P_pers.tile([128, 1024], f16, name="w3k0", tag="w3k0"),
           P_pers.tile([64, 1024], f16, name="w3k1", tag="w3k1")]
    zero64 = P_pers.tile([64, 1024], f16, name="zero64", tag="zero64")
    nc.gpsimd.memset(zero64[:], 0.0)
    dummy_w = P_pers.tile([128, 128], f16, name="dummy_w", tag="dummy_w")
    dummy_rhs = P_pers.tile([128, 512], f16, name="dummy_rhs", tag="dummy_rhs")
    nc.gpsimd.memset(dummy_w[:], 0.0)
    nc.gpsimd.memset(dummy_rhs[:], 0.0)

    bn = []  # bn[ci][blk] = dict(g, be, bb(slices), a, b(tiles))
    for ci in range(3):
        blks = []
        for b in range(conv_d[ci]["nblk"]):
            e = {nm: sc(f"bn{ci}{nm}{b}") for nm in ("g", "be", "bb")}
            e["a"] = P_pers.tile([128, 1], f32, name=f"bn{ci}a{b}", tag=f"bn{ci}a{b}")
            e["b"] = P_pers.tile([128, 1], f32, name=f"bn{ci}b{b}", tag=f"bn{ci}b{b}")
            blks.append(e)
        bn.append(blks)

    wv_t = [[smalls[:, lay[f"wv{ci}{b}"]:lay[f"wv{ci}{b}"] + 9]
             for b in range(conv_d[ci]["nblk"])] for ci in range(3)]

    SLAB = 1024                      # conv slab (PSUM-resident px per step)
    NSLAB = NPIX // SLAB             # 4 slabs per image
    Sp, Qp = [], []
    for ci in range(3):
        Sp.append([P_pers.tile([128, 2 * NSLAB], f32, name=f"Sp{ci}{b}",
                               tag=f"Sp{ci}{b}") for b in range(conv_d[ci]["nblk"])])
        Qp.append([P_pers.tile([128, 2 * NSLAB], f32, name=f"Qp{ci}{b}",
                               tag=f"Qp{ci}{b}") for b in range(conv_d[ci]["nblk"])])
    pack1 = P_pers.tile([128, 2], f32, name="pack1", tag="pack1")
    pack2 = P_pers.tile([128, 4], f32, name="pack2", tag="pack2")
    pack3a = P_pers.tile([128, 4], f32, name="pack3a", tag="pack3a")
    pack3b = P_pers.tile([128, 4], f32, name="pack3b", tag="pack3b")
    gst1 = P_pers.tile([128, 2], f32, name="gst1", tag="gst1")
    gst2 = P_pers.tile([128, 4], f32, name="gst2", tag="gst2")
    gst3a = P_pers.tile([128, 4], f32, name="gst3a", tag="gst3a")
    gst3b = P_pers.tile([128, 4], f32, name="gst3b", tag="gst3b")

    # --------- conv emitter: 8 PE taps + fused DVE tap/combine/stats -----
    # zdst[b] = persistent SBUF tile [128, R]; slab written at
    # [:, img*NPIX + s*SLAB : +SLAB]. After each slab, one queued
    # interleave callback is drained (used to overlap fc2 with conv2).
    FUSED_STT = True     # fuse DVE tap + PSUM-combine + Sp into one stt
    ACT_SQUARE = True     # Qp via ACT Square (v1) vs DVE tensor_tensor_reduce

    def emit_conv(ci, pads, P_cps, P_cacc, P_csq, P_diag, zdst, imgs=(0, 1),
                  interleave=None, blocks=None, extra_dve_tap=None):
        nblk = conv_d[ci]["nblk"]
        rows = SLAB // HH
        diag_dram = conv_d[ci]["diag"]
        pe_taps = [t for t in PE_TAPS if t != extra_dve_tap]
        for b in (range(nblk) if blocks is None else blocks):
            if ci < 2:
                dg = dg12[ci][:, b * 9 * 128:(b + 1) * 9 * 128]
            else:
                dg = P_diag.tile([128, 9 * 128], f16, name="dg", tag="dg")
                nc.sync.dma_start(
                    dg[:], diag_dram[:, b * 9 * 128:(b + 1) * 9 * 128])
            for img in imgs:
                p3 = pad3(pads[b][img])
                for s in range(NSLAB):
                    r0 = s * rows
                    ps = P_cps.tile([128, SLAB], f32, name=f"cps{ci}", tag="cps")
                    for ti, t in enumerate(pe_taps):
                        di, dj = TAPS[t]
                        rhs = p3[:, 1 + di + r0:1 + di + r0 + rows,
                                 1 + dj:1 + dj + HH]
                        for nn in range(SLAB // 512):
                            rr = nn * (512 // HH)
                            nc.tensor.matmul(
                                ps[:, nn * 512:(nn + 1) * 512],
                                dg[:, t * 128:(t + 1) * 128] if ci >= 2 else dg[:, t * 128:(t + 1) * 128],
                                rhs[:, rr:rr + (512 // HH), :],
                                start=(ti == 0), stop=(ti == len(pe_taps) - 1))
                    slot = img * NSLAB + s
                    col = img * NPIX + s * SLAB
                    zsl = zdst[b][:, col:col + SLAB]
                    psum_in = ps[:].rearrange("p (a b) -> p a b", a=rows)
                    if extra_dve_tap is not None:
                        di, dj = TAPS[extra_dve_tap]
                        acc = P_cacc.tile([128, SLAB], f16, name="cacc", tag="cacc")
                        nc.vector.scalar_tensor_tensor(
                            acc[:].rearrange("p (a b) -> p a b", a=rows),
                            p3[:, 1 + di + r0:1 + di + r0 + rows,
                               1 + dj:1 + dj + HH],
                            wv_t[ci][b][:, extra_dve_tap:extra_dve_tap + 1],
                            psum_in, op0=AL.mult, op1=AL.add)
                        psum_in = acc[:].rearrange("p (a b) -> p a b", a=rows)
                    di, dj = TAPS[DVE_TAP]
                    tap_ap = p3[:, 1 + di + r0:1 + di + r0 + rows,
                                1 + dj:1 + dj + HH]
                    if FUSED_STT:
                        nc.vector.scalar_tensor_tensor(
                            zsl.rearrange("p (a b) -> p a b", a=rows),
                            tap_ap,
                            wv_t[ci][b][:, DVE_TAP:DVE_TAP + 1],
                            psum_in,
                            op0=AL.mult, op1=AL.add,
                            accum_out=Sp[ci][b][:, slot:slot + 1])
                    else:
                        acc = P_cacc.tile([128, SLAB], f16, name="cacc", tag="cacc")
                        nc.vector.tensor_scalar(
                            acc[:].rearrange("p (a b) -> p a b", a=rows),
                            tap_ap, wv_t[ci][b][:, DVE_TAP:DVE_TAP + 1], None,
                            op0=AL.mult)
                        nc.vector.scalar_tensor_tensor(
                            zsl, acc[:], 0.0, ps[:], op0=AL.bypass, op1=AL.add,
                            accum_out=Sp[ci][b][:, slot:slot + 1])
                    sq = P_csq.tile([128, SLAB], f16, name="sqs", tag="sqs")
                    if ACT_SQUARE:
                        nc.scalar.activation(sq[:], zsl, AF.Square,
                                             accum_out=Qp[ci][b][:, slot:slot + 1])
                    else:
                        nc.vector.tensor_tensor_reduce(
                            sq[:], zsl, zsl, 1.0, 0.0, op0=AL.mult, op1=AL.add,
                            accum_out=Qp[ci][b][:, slot:slot + 1])
                    if interleave:
                        interleave.pop(0)()

    def open_conv_pools(sfx):
        return (pools.open(f"cps{sfx}", bufs=2, space="PSUM"),
                pools.open(f"cacc{sfx}", bufs=2),
                pools.open(f"csq{sfx}", bufs=2),
                pools.open(f"diag{sfx}", bufs=2))

    def close_conv_pools(sfx):
        pools.close(f"diag{sfx}", f"csq{sfx}", f"cacc{sfx}", f"cps{sfx}")

    def bn_math(ci, b, S_ap, Q_ap):
        e = bn[ci][b]
        tt = lambda tag: P_tmpv.tile([128, 1], f32, name=tag, tag=tag)
        m = tt("bnm"); e2 = tt("bne"); m2 = tt("bnm2"); v = tt("bnv")
        sq = tt("bnsq"); iv = tt("bniv"); mb = tt("bnmb"); ab = tt("bnab")
        nc.vector.tensor_scalar(m[:], S_ap, inv_n, None, op0=AL.mult)
        nc.vector.tensor_scalar(e2[:], Q_ap, inv_n, None, op0=AL.mult)
        nc.vector.tensor_tensor(m2[:], m[:], m[:], op=AL.mult)
        nc.vector.tensor_tensor(v[:], e2[:], m2[:], op=AL.subtract)
        nc.vector.tensor_scalar(v[:], v[:], EPS, None, op0=AL.add)
        nc.scalar.activation(sq[:], v[:], AF.Sqrt)
        nc.vector.reciprocal(iv[:], sq[:])
        nc.vector.tensor_tensor(e["a"][:], e["g"], iv[:], op=AL.mult)
        nc.vector.tensor_tensor(mb[:], m[:], e["bb"], op=AL.add)
        nc.vector.tensor_tensor(ab[:], e["a"][:], mb[:], op=AL.mult)
        nc.vector.tensor_tensor(e["b"][:], e["be"], ab[:], op=AL.subtract)

    def allreduce(pack, gst, ncols, tag):
        if n_cores == 1:
            nc.vector.tensor_copy(gst[:], pack[:])
            return
        ib = P_dram.tile([128, ncols], f32, name=f"cc_in{tag}", tag=f"cc_in{tag}")
        ob = P_dram.tile([128, ncols], f32, name=f"cc_out{tag}", tag=f"cc_out{tag}")
        nc.gpsimd.dma_start(ib[:], pack[:])
        nc.gpsimd.collective_compute(
            "AllReduce", AL.add,
            replica_groups=[list(range(n_cores))],
            ins=[ib.opt()], outs=[ob.opt()])
        nc.gpsimd.dma_start(gst[:], ob[:])

    def reduce_stats(pack, cols):
        for i, (ci, b) in enumerate(cols):
            nc.vector.tensor_reduce(pack[:, 2 * i:2 * i + 1], Sp[ci][b][:],
                                    axis=mybir.AxisListType.X, op=AL.add)
            nc.vector.tensor_reduce(pack[:, 2 * i + 1:2 * i + 2], Qp[ci][b][:],
                                    axis=mybir.AxisListType.X, op=AL.add)

    # persistent SBUF activations (fp16)
    P_hf = pools.open("hfp", bufs=1)
    hf1a = P_hf.tile([128, R], f16, name="hf1a", tag="hf1a")
    hf1b = P_hf.tile([64, R], f16, name="hf1b", tag="hf1b")

    # =================== stage A: fc1_low + fc1_full ==================
    P_z12 = pools.open("z12p", bufs=1)
    z1_sb = [P_z12.tile([128, R], f16, name="z1sb", tag="z1sb")]
    z2_sb = [P_z12.tile([128, R], f16, name=f"z2sb{b}", tag=f"z2sb{b}")
             for b in range(2)]
    yl_sb = P_z12.tile([128, R], f16, name="ylsb", tag="ylsb")

    P_pad2 = pools.open("pads2", bufs=1)
    P_pad1 = pools.open("pads1", bufs=1)
    y1p = [P_pad1.tile([128, PAREA], f16, name=f"y1p{i}", tag=f"y1p{i}")
           for i in range(2)]
    y2p = [[P_pad2.tile([128, PAREA], f16, name=f"y2p{b}{i}", tag=f"y2p{b}{i}")
            for i in range(2)] for b in range(2)]
    for t in y1p:
        nc.vector.memset(t[:], 0.0)
    for i in range(2):            # image-major: img0 pads ready first
        for b in range(2):
            nc.gpsimd.memset(y2p[b][i][:], 0.0)

    P_wA = pools.open("wAp", bufs=1)
    P_xk = pools.open("xk", bufs=2)
    P_tmpA = pools.open("tmpA", bufs=2)
    P_psA = pools.open("psA", bufs=2, space="PSUM")
    wA_sb = P_wA.tile([128, 24 * 128], f16, name="wAsb", tag="wAsb")
    wAt = lambda k, m: wA_sb[:, (k * 6 + m) * 128:(k * 6 + m + 1) * 128]
    pairs = [(0, 1, lambda img: y1p[img]),
             (2, 4, lambda img: y2p[0][img]),
             (3, 5, lambda img: y2p[1][img])]
    for ch in range(8):
        img, lrow = ch // 4, (ch % 4) * 16
        xs = []
        for k in range(4):
            xt = P_xk.tile([128, 1024], f16, name=f"xk{k}", tag=f"xk{k}")
            eng = nc.sync if k % 2 == 0 else nc.scalar
            eng.dma_start(
                xt[:], x_d[k * 128:(k + 1) * 128, ch * 1024:(ch + 1) * 1024])
            xs.append(xt)
        if ch == 0:
            nc.sync.dma_start(wA_sb[:], wA_d[:])
        for bm, sm, dest in pairs:
            psB = P_psA.tile([128, 1024], f32, name="psB", tag="psB")
            psS = P_psA.tile([128, 1024], f32, name="psS", tag="psS")
            for k in range(4):
                for nn in range(2):
                    sl = slice(nn * 512, (nn + 1) * 512)
                    nc.tensor.matmul(psB[:, sl], wAt(k, bm), xs[k][:, sl],
                                     start=(k == 0), stop=(k == 3))
                    nc.tensor.matmul(psS[:, sl], wAt(k, sm), xs[k][:, sl],
                                     start=(k == 0), stop=(k == 3))
            tmp = P_tmpA.tile([128, 1024], f16, name="siluA", tag="siluA")
            nc.scalar.activation(tmp[:], psB[:], AF.Silu)
            outap = pad3(dest(img))[:, 1 + lrow:1 + lrow + 16, 1:65]
            nc.vector.scalar_tensor_tensor(
                outap,
                psS[:].rearrange("p (a b) -> p a b", a=16),
                0.0,
                tmp[:].rearrange("p (a b) -> p a b", a=16),
                op0=AL.bypass, op1=AL.add)
    pools.close("psA", "tmpA", "xk", "wAp")
    nc.scalar.dma_start(smalls[:], smalls_d[:])
    nc.scalar.dma_start(w2_sb[:], wfc2_d[:])
    nc.scalar.dma_start(wf1_sb[:], wfus1_d[:])
    nc.scalar.dma_start(w3k[0][:], wfc3_d[0:128, :])
    nc.scalar.dma_start(w3k[1][:], wfc3_d[128:192, :])
    nc.scalar.dma_start(dg12[0][:], conv_d[0]["diag"][:])
    nc.scalar.dma_start(dg12[1][:], conv_d[1]["diag"][:])

    # ============ conv1 -> AR1 (overlapped with conv2+fc2) =============
    cpools12 = open_conv_pools("c12")
    emit_conv(0, [y1p], *cpools12, zdst=z1_sb, extra_dve_tap=8)
    reduce_stats(pack1, [(0, 0)])
    allreduce(pack1, gst1, 2, "a1")

    # fc2 work units, interleaved into conv2's 16 slab iterations
    P_t2 = pools.open("fc2t", bufs=2)
    P_ps2 = pools.open("psF2", bufs=1, space="PSUM")

    def fc2_chunk(ch):
        sl = slice(ch * 1024, (ch + 1) * 1024)
        z1b = P_t2.tile([128, 1024], f16, name="z1b", tag="z1b")
        nc.scalar.activation(z1b[:], z1_sb[0][:, sl], AF.Relu,
                             bias=bn[0][0]["b"][:], scale=bn[0][0]["a"][:])
        psB = P_ps2.tile([128, 1024], f32, name="ps2B", tag="ps2B")
        psS = P_ps2.tile([128, 1024], f32, name="ps2S", tag="ps2S")
        for nn in range(2):
            s2 = slice(nn * 512, (nn + 1) * 512)
            nc.tensor.matmul(psB[:, s2], w2_sb[:, 0:128], z1b[:, s2],
                             start=True, stop=True)
            nc.tensor.matmul(psS[:, s2], w2_sb[:, 128:256], z1b[:, s2],
                             start=True, stop=True)
        tmp = P_t2.tile([128, 1024], f16, name="silu2", tag="silu2")
        nc.scalar.activation(tmp[:], psB[:], AF.Silu)
        nc.vector.scalar_tensor_tensor(yl_sb[:, sl], psS[:], 0.0, tmp[:],
                                       op0=AL.bypass, op1=AL.add)

    INTERLEAVE_FC2 = True
    if INTERLEAVE_FC2:
        todo = [lambda: bn_math(0, 0, gst1[:, 0:1], gst1[:, 1:2])]
        todo += [lambda ch=ch: fc2_chunk(ch) for ch in range(8)]
        ilv = [lambda: None] * 5 + todo
        ilv += [lambda: None] * (16 - len(ilv))
    else:
        ilv = None

    emit_conv(1, y2p, *cpools12, zdst=z2_sb, interleave=ilv)
    if not INTERLEAVE_FC2:
        bn_math(0, 0, gst1[:, 0:1], gst1[:, 1:2])
        for ch in range(8):
            fc2_chunk(ch)
    pools.close("psF2", "fc2t")
    close_conv_pools("c12")
    pools.close("pads1", "pads2")
    reduce_stats(pack2, [(1, 0), (1, 1)])
    allreduce(pack2, gst2, 4, "a2")
    bn_math(1, 0, gst2[:, 0:1], gst2[:, 1:2])
    bn_math(1, 1, gst2[:, 2:3], gst2[:, 3:4])

    # keep the PE array busy through the AR2 collective so the HAM clock
    # gate stays at 8/8 into fusion/fc3 (a >3.4us PE-idle window would
    # re-throttle to 1.2 GHz for tens of us). Garbage-in, never-read-out.
    P_warm = pools.open("pswarm", bufs=1, space="PSUM")
    wps = P_warm.tile([128, 512], f32, name="wps", tag="wps")
    NWARM = 130
    for i in range(NWARM):
        nc.tensor.matmul(wps[:], dummy_w[:], dummy_rhs[:],
                         start=(i == 0), stop=(i == NWARM - 1),
                         skip_group_check=True)
    pools.close("pswarm")

    # =================== fusion linear 1 -> hf1 (SBUF) ==================
    P_tf1 = pools.open("fu1t", bufs=3)
    P_psf1 = pools.open("psFu1", bufs=2, space="PSUM")
    wf1t = lambda k, m: wf1_sb[:, k * FUSH + m * 128:k * FUSH + m * 128 + (64 if m else 128)]
    for ch in range(8):
        sl = slice(ch * 1024, (ch + 1) * 1024)
        z2b0 = P_tf1.tile([128, 1024], f16, name="z2b0", tag="z2b0")
        z2b1 = P_tf1.tile([128, 1024], f16, name="z2b1", tag="z2b1")
        nc.scalar.activation(z2b0[:], z2_sb[0][:, sl], AF.Relu,
                             bias=bn[1][0]["b"][:], scale=bn[1][0]["a"][:])
        nc.vector.tensor_scalar(z2b1[:], z2_sb[1][:, sl], bn[1][1]["a"][:],
                                bn[1][1]["b"][:], op0=AL.mult, op1=AL.add)
        nc.vector.tensor_scalar(z2b1[:], z2b1[:], 0.0, None, op0=AL.max)
        rhs = [yl_sb[:, sl], z2b0[:], z2b1[:]]
        ps0 = P_psf1.tile([128, 1024], f32, name="psf1a", tag="psf1a")
        ps1 = P_psf1.tile([64, 1024], f32, name="psf1b", tag="psf1b")
        for k in range(3):
            for nn in range(2):
                s2 = slice(nn * 512, (nn + 1) * 512)
                nc.tensor.matmul(ps0[:, s2], wf1t(k, 0), rhs[k][:, s2],
                                 start=(k == 0), stop=(k == 2))
                nc.tensor.matmul(ps1[:, s2], wf1t(k, 1), rhs[k][:, s2],
                                 start=(k == 0), stop=(k == 2))
        nc.scalar.activation(hf1a[:, sl], ps0[:], AF.Relu, bias=bf1a)
        nc.vector.scalar_tensor_tensor(hf1b[:, sl], ps1[:], bf1b, zero64[:],
                                       op0=AL.add, op1=AL.max)
    pools.close("psFu1", "fu1t")
    pools.close("z12p")

    # ============= fc3' + conv3 (block-split stats) + finals ============
    P_z3 = pools.open("z3p", bufs=1)
    z3_sb = [P_z3.tile([128, R], f16, name=f"z3sb{b}", tag=f"z3sb{b}")
             for b in range(4)]
    P_h3 = pools.open("h3p", bufs=1)
    P_t3 = pools.open("fc3t", bufs=3)
    P_ps3 = pools.open("psF3", bufs=2, space="PSUM")
    P_xc = pools.open("xcp", bufs=3)
    P_fin = pools.open("fint", bufs=3)
    cpools3 = open_conv_pools("c3")

    def fin_chunk(b, ch, dve_bn=True):
        rows = slice(b * 128, (b + 1) * 128)
        sl = slice(ch * 2048, (ch + 1) * 2048)
        xc = P_xc.tile([128, 2048], f16, name="xc", tag="xc")
        nc.scalar.dma_start(xc[:], xs_d[rows, sl])
        t = P_fin.tile([128, 2048], f16, name="trelu", tag="trelu")
        if dve_bn and ch == 3:   # balance: every 4th BN+ReLU on the DVE
            nc.vector.tensor_scalar(t[:], z3_sb[b][:, sl],
                                    bn[2][b]["a"][:], bn[2][b]["b"][:],
                                    op0=AL.mult, op1=AL.add)
            nc.vector.tensor_scalar(t[:], t[:], 0.0, None, op0=AL.max)
        else:
            nc.scalar.activation(t[:], z3_sb[b][:, sl], AF.Relu,
                                 bias=bn[2][b]["b"][:], scale=bn[2][b]["a"][:])
        ob = P_fin.tile([128, 2048], f16, name="ob", tag="ob")
        nc.vector.tensor_tensor(ob[:], xc[:], t[:], op=AL.add)
        nc.sync.dma_start(out_d[rows, sl], ob[:])

    h3sets = []
    for img in range(2):
        h3 = [P_h3.tile([128, PAREA], f16, name=f"h3p{b}", tag=f"h3p{b}")
              for b in range(4)]
        h3sets.append(h3)
        for t in h3:
            nc.gpsimd.memset(t[:], 0.0)
        for ch in range(8):           # 512-px chunks within image
            r0 = ch * 8
            sl = slice(img * NPIX + ch * 512, img * NPIX + (ch + 1) * 512)
            rhs = [hf1a[:, sl], hf1b[:, sl]]
            for mp in range(4):
                psB = P_ps3.tile([128, 512], f32, name="ps3B", tag="ps3B")
                psS = P_ps3.tile([128, 512], f32, name="ps3S", tag="ps3S")
                for kk in range(2):
                    nc.tensor.matmul(psB[:], w3k[kk][:, mp * 128:(mp + 1) * 128],
                                     rhs[kk], start=(kk == 0), stop=(kk == 1))
                    nc.tensor.matmul(psS[:], w3k[kk][:, (4 + mp) * 128:(5 + mp) * 128],
                                     rhs[kk], start=(kk == 0), stop=(kk == 1))
                tmp = P_t3.tile([128, 512], f16, name="silu3", tag="silu3")
                nc.scalar.activation(tmp[:], psB[:], AF.Silu, bias=b3bt[mp])
                outap = pad3(h3[mp])[:, 1 + r0:1 + r0 + 8, 1:65]
                nc.vector.scalar_tensor_tensor(
                    outap,
                    psS[:].rearrange("p (a b) -> p a b", a=8),
                    b3st[mp],
                    tmp[:].rearrange("p (a b) -> p a b", a=8),
                    op0=AL.add, op1=AL.add)
        if img == 0:
            emit_conv(2, [{0: h3[b]} for b in range(4)], *cpools3,
                      zdst=z3_sb, imgs=(0,), extra_dve_tap=8)
    # conv3 img1: blocks 0-1, then AR3a fires while blocks 2-3 conv and
    # the finals for blocks 0-1 interleave into their slab loop.
    emit_conv(2, [{1: h3sets[1][b]} for b in range(4)], *cpools3,
              zdst=z3_sb, imgs=(1,), blocks=(0, 1), extra_dve_tap=8)
    reduce_stats(pack3a, [(2, 0), (2, 1)])
    allreduce(pack3a, gst3a, 4, "a3a")
    # bn_math + the first finals are staggered into conv3 blocks 2-3 via the
    # interleave hooks so their AR3a-gated ops never head-of-line-block the
    # DVE/ACT queues ahead of conv3's own slab work.
    ilv3 = [lambda: None] * 3
    ilv3.append(lambda: bn_math(2, 0, gst3a[:, 0:1], gst3a[:, 1:2]))
    ilv3.append(lambda: bn_math(2, 1, gst3a[:, 2:3], gst3a[:, 3:4]))
    ilv3 += [lambda ch=ch: fin_chunk(0, ch, dve_bn=False) for ch in range(3)]
    emit_conv(2, [{1: h3sets[1][b]} for b in range(4)], *cpools3,
              zdst=z3_sb, imgs=(1,), blocks=(2, 3), interleave=ilv3)
    fin_chunk(0, 3, dve_bn=False)
    for ch in range(4):
        fin_chunk(1, ch, dve_bn=False)
    reduce_stats(pack3b, [(2, 2), (2, 3)])
    allreduce(pack3b, gst3b, 4, "a3b")
    bn_math(2, 2, gst3b[:, 0:1], gst3b[:, 1:2])
    bn_math(2, 3, gst3b[:, 2:3], gst3b[:, 3:4])
    for b in (2, 3):
        for ch in range(4):
            fin_chunk(b, ch)
    pools.close_all()


def _get_built(n_cores):
    if n_cores not in _CACHE:
        _CACHE[n_cores] = _build(n_cores)
    return _CACHE[n_cores]


def make_in_maps(inputs, n_cores):
    shared = _prep_shared(inputs)
    xt = _prep_x(inputs["x"], n_cores)
    rsv = float(np.asarray(inputs["res_scale"]).reshape(-1)[0])
    xst = _prep_x(inputs["x"], n_cores, scale=rsv)
    return [dict(shared, x_t=xt[c], xs_t=xst[c]) for c in range(n_cores)]


def kernel(**inputs):
    from concourse.bass_utils import run_bass_kernel_spmd

    assert int(np.asarray(inputs["H"])) == HH and int(np.asarray(inputs["W"])) == HH
    n_cores = 8
    nc = _get_built(n_cores)
    in_maps = make_in_maps(inputs, n_cores)
    res = run_bass_kernel_spmd(nc, in_maps, core_ids=list(range(n_cores)))
    B = np.asarray(inputs["x"]).shape[0]
    per = B // n_cores
    out = np.empty((B, NPIX, CIN), np.float32)
    for c in range(n_cores):
        out[c * per:(c + 1) * per] = \
            res.results[c]["out_t"].astype(np.float32).T.reshape(per, NPIX, CIN)
    return out


# ------------------------------------------------------------- profiling

def _install_ntff_hook():
    """The agent image's antenv lacks axon_hooks; recreate the NTFF profile
    hook via ctypes on the axon PJRT .so (same ABI as trn_boot's)."""
    import contextlib, ctypes, sys, types
    so = "/opt/axon/libaxon_pjrt.so"
    try:
        import antenv.axon_hooks  # noqa: F401
        return True
    except ImportError:
        pass
    try:
        lib = ctypes.CDLL(so)
    except OSError:
        return False
    if not hasattr(lib, "axon_start_nrt_profile"):
        return False
    lib.axon_start_nrt_profile.argtypes = [
        ctypes.POINTER(ctypes.c_int64), ctypes.c_size_t]
    lib.axon_start_nrt_profile.restype = ctypes.c_int64
    lib.axon_stop_nrt_profile.argtypes = [ctypes.c_char_p]
    lib.axon_stop_nrt_profile.restype = ctypes.c_int64

    @contextlib.contextmanager
    def _hook(output_dir, device_ids):
        import jax
        jax.devices()
        if device_ids:
            ids = (ctypes.c_int64 * len(device_ids))(*device_ids)
            rc = lib.axon_start_nrt_profile(ids, len(device_ids))
        else:
            rc = lib.axon_start_nrt_profile(None, 0)
        if rc != 0:
            raise RuntimeError(f"axon_start_nrt_profile rc={rc}")
        try:
            yield
        finally:
            n = lib.axon_stop_nrt_profile(str(output_dir).encode())
            print(f"profile: {n} ntff file(s) -> {output_dir}", file=sys.stderr)

    mod = types.ModuleType("antenv.axon_hooks")
    mod.get_axon_ntff_profile_hook = lambda: _hook
    mod.set_axon_ntff_profile_hook = lambda h: None
    sys.modules["antenv.axon_hooks"] = mod
    import concourse.bass_utils as bu
    bu.upload_artifacts = lambda tmpdir: f"local:{tmpdir}"
    return True


def benchmark(inputs, iters=2, tmpdir=None):
    """Device-only HW execution time (ns) via neuron-profile NTFF trace."""
    import os, tempfile
    from concourse.bass_utils import run_bass_kernel_spmd

    if not _install_ntff_hook():
        raise RuntimeError("NTFF profiling unavailable")
    if tmpdir:
        os.makedirs(tmpdir, exist_ok=True)
    n_cores = 8
    nc = _get_built(n_cores)
    in_maps = make_in_maps(inputs, n_cores)
    times = []
    for i in range(max(1, min(iters, 3))):
        td = tempfile.mkdtemp(dir=tmpdir) if tmpdir else None
        res = run_bass_kernel_spmd(nc, in_maps, core_ids=list(range(n_cores)),
                                   trace=True, tmpdir=td)
        if res.exec_time_ns is not None:
            times.append(res.exec_time_ns)
    if not times:
        raise RuntimeError("no exec_time_ns from traced runs")
    return min(times)


# revision 40
# speedup vs baseline: 1.0692x; 1.0107x over previous
"""Trainium2 Bass kernel for nn_ConvLinearLayer (KAN-style conv-linear block).

Strategy
--------
Data-parallel over batch: 16 images -> 8 cores x 2 images. All activations
live on-chip in transposed layout [channels(partitions), pixels(free)], so
GEMMs (PE, fp16), depthwise 3x3 convs (8 PE diag-matmul taps + 1 fused DVE
tap), BN stats (accumulator outputs) and BN-apply+ReLU (ACT, per-partition
scale/bias) all hit their natural axes. Train-mode BN needs global batch
stats -> three tiny AllReduces (per-channel sum/sumsq), each overlapped with
surrounding compute (fc2 is interleaved into conv2's slab loop).

All matmul operands are fp16 (fp32 PSUM accumulation, fp32 BN statistics,
fp32 output): fp32 moving operands stream at ~half rate through the PE
array, fp16 streams at full rate with ample mantissa for this tolerance.
All conv outputs stay SBUF-resident; weights are host-pre-tiled so every
DMA is a single contiguous 2D transfer.

Host-side precompute: input/weight transposes + fp16 casts, spline-weight
sum (sum_k sw[:,:,k]/K == one GEMM), channel_scale folded into fus_w1,
fus_w2+b2 folded into fc3 (W3_eff = W3 @ W2, b3_eff = W3 @ b2), conv-bias
folded into the BN affine.
"""

import numpy as np

F16 = np.dtype(np.float16)

K_SPLINE = 10
EPS = 1e-5
HH = 64
PW = 66           # padded row stride (64 + 2 zero border)
PAREA = PW * PW   # 4356
NPIX = HH * HH    # 4096 pixels per image
R = 2 * NPIX      # rows per core (2 images)
CIN = 512
LOW = 128
FULL = 256
CAT = 384
FUSH = 192
COUT = 512

TAPS = [(di, dj) for di in (-1, 0, 1) for dj in (-1, 0, 1)]
DVE_TAP = 0                           # fused into the PSUM-combine stt
PE_TAPS = [t for t in range(9) if t != DVE_TAP]
NBLKS = [1, 2, 4]

_CACHE = {}


def _smalls_layout():
    """Column layout of the packed [128, N] fp32 constants tensor."""
    col = 0
    lay = {}
    lay["rs"] = col; col += 1
    for ci, nblk in enumerate(NBLKS):
        for b in range(nblk):
            for nm in ("g", "be", "bb"):
                lay[f"bn{ci}{nm}{b}"] = col; col += 1
    for ci, nblk in enumerate(NBLKS):
        for b in range(nblk):
            lay[f"wv{ci}{b}"] = col; col += 9
    lay["bf1a"] = col; col += 1
    lay["bf1b"] = col; col += 1
    for m in range(4):
        lay[f"b3b{m}"] = col; col += 1
    for m in range(4):
        lay[f"b3s{m}"] = col; col += 1
    return lay, col


# ---------------------------------------------------------------- host prep

def _prep_shared(inp):
    """All non-x device tensors (replicated across cores), as numpy 2D."""
    f = lambda a: np.ascontiguousarray(np.asarray(a, dtype=np.float32))
    h = lambda a: np.ascontiguousarray(
        np.asarray(a, dtype=np.float32).astype(F16))
    sws = lambda sw: np.asarray(sw, np.float64).sum(-1) / K_SPLINE

    fc1_low_bw = f(inp["fc1_low_bw"]); s1l = f(sws(inp["fc1_low_sw"]))
    fc1_full_bw = f(inp["fc1_full_bw"]); s1f = f(sws(inp["fc1_full_sw"]))
    fc2_bw = f(inp["fc2_low_bw"]); s2 = f(sws(inp["fc2_low_sw"]))
    fc3_bw = f(inp["fc3_bw"]); s3 = f(sws(inp["fc3_sw"]))
    w1 = f(inp["fus_w1"]); b1 = f(inp["fus_b1"])
    w2 = f(inp["fus_w2"]); b2 = f(inp["fus_b2"])
    cs = f(inp["channel_scale"])

    d = {}
    # stage A lhsT tiles (k,m) of [512, 768] packed as [128, 24*128]
    # m-blocks: [lowb, lows, fullb0, fullb1, fulls0, fulls1]
    wA = np.concatenate([fc1_low_bw.T, s1l.T, fc1_full_bw.T, s1f.T], axis=1)
    wAt = np.empty((128, 24 * 128), np.float32)
    for k in range(4):
        for m in range(6):
            wAt[:, (k * 6 + m) * 128:(k * 6 + m + 1) * 128] = \
                wA[k * 128:(k + 1) * 128, m * 128:(m + 1) * 128]
    d["wA"] = h(wAt)
    d["wfc2"] = h(np.concatenate([fc2_bw.T, s2.T], axis=1))      # [128, 256]
    wfus1 = (w1 * cs[None, :]).T                                 # [384, 192]
    wf1t = np.empty((128, 3 * FUSH), np.float32)
    for k in range(3):
        wf1t[:, k * FUSH:(k + 1) * FUSH] = wfus1[k * 128:(k + 1) * 128, :]
    d["wfus1"] = h(wf1t)                                         # [128, 576]
    w3b = fc3_bw @ w2                                            # [512, 192]
    w3s = s3 @ w2
    d["wfc3"] = h(np.concatenate([w3b.T, w3s.T], axis=1))        # [192, 1024]
    b3b = (fc3_bw @ b2).reshape(-1)
    b3s = (s3 @ b2).reshape(-1)

    lay, ncols = _smalls_layout()
    sm = np.zeros((128, ncols), np.float32)
    sm[:, lay["rs"]] = float(np.asarray(inp["res_scale"]).reshape(-1)[0])
    sm[0:128, lay["bf1a"]] = b1[0:128]
    sm[0:64, lay["bf1b"]] = b1[128:192]
    for m in range(4):
        sm[:, lay[f"b3b{m}"]] = b3b[m * 128:(m + 1) * 128]
        sm[:, lay[f"b3s{m}"]] = b3s[m * 128:(m + 1) * 128]

    # depthwise convs: diag tiles (b,t) packed as [128, nblk*9*128]
    for ci, (wname, gname, bname, bbname, Cc) in enumerate([
            ("dw1_w", "dw1_g", "dw1_beta", "dw1_b", LOW),
            ("dw2_w", "dw2_g", "dw2_beta", "dw2_b", FULL),
            ("dw3_w", "dw3_g", "dw3_beta", "dw3_b", COUT)]):
        w = f(inp[wname]).reshape(Cc, 9)                          # [C, taps]
        g = f(inp[gname]).reshape(-1)
        be = f(inp[bname]).reshape(-1)
        bb = f(inp[bbname]).reshape(-1)
        nblk = Cc // 128
        diag = np.zeros((128, nblk * 9 * 128), np.float32)
        for b in range(nblk):
            rows = slice(b * 128, (b + 1) * 128)
            for t in range(9):
                c0 = (b * 9 + t) * 128
                diag[:, c0:c0 + 128] = np.diag(w[rows, t])
            c = lay[f"wv{ci}{b}"]
            sm[:, c:c + 9] = w[rows]
            sm[:, lay[f"bn{ci}g{b}"]] = g[rows]
            sm[:, lay[f"bn{ci}be{b}"]] = be[rows]
            sm[:, lay[f"bn{ci}bb{b}"]] = bb[rows]
        d[f"diag{ci+1}"] = h(diag)
    d["smalls"] = np.ascontiguousarray(sm)
    return d


def _prep_x(x, n_cores, scale=None):
    """Per-core transposed fp16 shards [512, 8192]."""
    x = np.asarray(x, np.float32)
    if scale is not None:
        x = x * scale
    x = x.astype(F16)
    B = x.shape[0]
    per = B // n_cores
    return [np.ascontiguousarray(
        x[c * per:(c + 1) * per].reshape(per * NPIX, CIN).T)
        for c in range(n_cores)]


# ---------------------------------------------------------------- builder

def _build(n_cores):
    import concourse.bacc as bacc
    import concourse.mybir as mybir
    import concourse.tile as tile

    f32 = mybir.dt.float32
    f16 = mybir.dt.float16

    nc = bacc.Bacc("TRN2", target_bir_lowering=False, debug=False,
                   num_devices=n_cores)

    def din(name, shape, dt=f16):
        return nc.dram_tensor(name, list(shape), dt, kind="ExternalInput").ap()

    x_d = din("x_t", (CIN, R))
    xs_d = din("xs_t", (CIN, R))
    wA_d = din("wA", (128, 24 * 128))
    wfc2_d = din("wfc2", (128, 256))
    wfus1_d = din("wfus1", (128, 3 * FUSH))
    wfc3_d = din("wfc3", (FUSH, 1024))
    lay, ncols = _smalls_layout()
    smalls_d = din("smalls", (128, ncols), f32)
    conv_d = []
    for ci, Cc in [(1, LOW), (2, FULL), (3, COUT)]:
        nblk = Cc // 128
        conv_d.append(dict(
            diag=din(f"diag{ci}", (128, nblk * 9 * 128)),
            nblk=nblk))
    out_d = nc.dram_tensor("out_t", [COUT, R], f16, kind="ExternalOutput").ap()

    with tile.TileContext(nc) as tc:
        _emit(nc, tc, mybir, n_cores, x_d, xs_d, wA_d, wfc2_d, wfus1_d,
              wfc3_d, conv_d, smalls_d, lay, ncols, out_d)
    nc.compile()
    return nc


def _emit(nc, tc, mybir, n_cores, x_d, xs_d, wA_d, wfc2_d, wfus1_d,
          wfc3_d, conv_d, smalls_d, lay, ncols, out_d):
    f32 = mybir.dt.float32
    f16 = mybir.dt.float16
    AL = mybir.AluOpType
    AF = mybir.ActivationFunctionType
    inv_n = 1.0 / (n_cores * R)

    class _Pools:
        def __init__(self, tc):
            self.tc = tc
            self.cms = {}
            self.order = []
        def open(self, name, **kw):
            cm = self.tc.tile_pool(name=name, **kw)
            pool = cm.__enter__()
            self.cms[name] = cm
            self.order.append(name)
            return pool
        def close(self, *names):
            names = sorted(names, key=self.order.index, reverse=True)
            for n in names:
                assert n == self.order[-1], (n, self.order)
                self.order.pop()
                self.cms.pop(n).__exit__(None, None, None)
        def close_all(self):
            self.close(*self.order)

    pools = _Pools(tc)

    def pad3(t):
        return t[:].rearrange("p (a b) -> p a b", a=PW)

    # ---------------- persistent small tiles ----------------
    P_pers = pools.open("pers", bufs=1)
    P_tmpv = pools.open("tmpv", bufs=4)
    P_dram = pools.open("dramp", bufs=1, space="DRAM")

    # one DMA for every small constant; everything below is a column slice
    smalls = P_pers.tile([128, ncols], f32, name="smalls", tag="smalls")
    sc = lambda key: smalls[:, lay[key]:lay[key] + 1]

    rs_t = sc("rs")
    bf1a = sc("bf1a")
    bf1b = smalls[0:64, lay["bf1b"]:lay["bf1b"] + 1]
    b3bt = [sc(f"b3b{m}") for m in range(4)]
    b3st = [sc(f"b3s{m}") for m in range(4)]

    dg12 = [P_pers.tile([128, conv_d[ci]["nblk"] * 9 * 128], f16,
                        name=f"dg12_{ci}", tag=f"dg12_{ci}") for ci in range(2)]
    w2_sb = P_pers.tile([128, 256], f16, name="w2sb", tag="w2sb")
    wf1_sb = P_pers.tile([128, 3 * FUSH], f16, name="wf1sb", tag="wf1sb")
    w3k = [P_pers.tile([128, 1024], f16, name="w3k0", tag="w3k0"),
           P_pers.tile([64, 1024], f16, name="w3k1", tag="w3k1")]
    zero64 = P_pers.tile([64, 1024], f16, name="zero64", tag="zero64")
    nc.gpsimd.memset(zero64[:], 0.0)
    dummy_w = P_pers.tile([128, 128], f16, name="dummy_w", tag="dummy_w")
    dummy_rhs = P_pers.tile([128, 512], f16, name="dummy_rhs", tag="dummy_rhs")
    nc.gpsimd.memset(dummy_w[:], 0.0)
    nc.gpsimd.memset(dummy_rhs[:], 0.0)

    bn = []  # bn[ci][blk] = dict(g, be, bb(slices), a, b(tiles))
    for ci in range(3):
        blks = []
        for b in range(conv_d[ci]["nblk"]):
            e = {nm: sc(f"bn{ci}{nm}{b}") for nm in ("g", "be", "bb")}
            e["a"] = P_pers.tile([128, 1], f32, name=f"bn{ci}a{b}", tag=f"bn{ci}a{b}")
            e["b"] = P_pers.tile([128, 1], f32, name=f"bn{ci}b{b}", tag=f"bn{ci}b{b}")
            blks.append(e)
        bn.append(blks)

    wv_t = [[smalls[:, lay[f"wv{ci}{b}"]:lay[f"wv{ci}{b}"] + 9]
             for b in range(conv_d[ci]["nblk"])] for ci in range(3)]

    SLAB = 1024                      # conv slab (PSUM-resident px per step)
    NSLAB = NPIX // SLAB             # 4 slabs per image
    Sp, Qp = [], []
    for ci in range(3):
        Sp.append([P_pers.tile([128, 2 * NSLAB], f32, name=f"Sp{ci}{b}",
                               tag=f"Sp{ci}{b}") for b in range(conv_d[ci]["nblk"])])
        Qp.append([P_pers.tile([128, 2 * NSLAB], f32, name=f"Qp{ci}{b}",
                               tag=f"Qp{ci}{b}") for b in range(conv_d[ci]["nblk"])])
    pack1 = P_pers.tile([128, 2], f32, name="pack1", tag="pack1")
    pack2 = P_pers.tile([128, 4], f32, name="pack2", tag="pack2")
    pack3a = P_pers.tile([128, 4], f32, name="pack3a", tag="pack3a")
    pack3b = P_pers.tile([128, 4], f32, name="pack3b", tag="pack3b")
    gst1 = P_pers.tile([128, 2], f32, name="gst1", tag="gst1")
    gst2 = P_pers.tile([128, 4], f32, name="gst2", tag="gst2")
    gst3a = P_pers.tile([128, 4], f32, name="gst3a", tag="gst3a")
    gst3b = P_pers.tile([128, 4], f32, name="gst3b", tag="gst3b")

    # --------- conv emitter: 8 PE taps + fused DVE tap/combine/stats -----
    # zdst[b] = persistent SBUF tile [128, R]; slab written at
    # [:, img*NPIX + s*SLAB : +SLAB]. After each slab, one queued
    # interleave callback is drained (used to overlap fc2 with conv2).
    FUSED_STT = True     # fuse DVE tap + PSUM-combine + Sp into one stt
    ACT_SQUARE = True     # Qp via ACT Square (v1) vs DVE tensor_tensor_reduce

    def emit_conv(ci, pads, P_cps, P_cacc, P_csq, P_diag, zdst, imgs=(0, 1),
                  interleave=None, blocks=None, extra_dve_tap=None):
        nblk = conv_d[ci]["nblk"]
        rows = SLAB // HH
        diag_dram = conv_d[ci]["diag"]
        pe_taps = [t for t in PE_TAPS if t != extra_dve_tap]
        for b in (range(nblk) if blocks is None else blocks):
            if ci < 2:
                dg = dg12[ci][:, b * 9 * 128:(b + 1) * 9 * 128]
            else:
                dg = P_diag.tile([128, 9 * 128], f16, name="dg", tag="dg")
                nc.sync.dma_start(
                    dg[:], diag_dram[:, b * 9 * 128:(b + 1) * 9 * 128])
            for img in imgs:
                p3 = pad3(pads[b][img])
                for s in range(NSLAB):
                    r0 = s * rows
                    ps = P_cps.tile([128, SLAB], f32, name=f"cps{ci}", tag="cps")
                    for ti, t in enumerate(pe_taps):
                        di, dj = TAPS[t]
                        rhs = p3[:, 1 + di + r0:1 + di + r0 + rows,
                                 1 + dj:1 + dj + HH]
                        for nn in range(SLAB // 512):
                            rr = nn * (512 // HH)
                            nc.tensor.matmul(
                                ps[:, nn * 512:(nn + 1) * 512],
                                dg[:, t * 128:(t + 1) * 128] if ci >= 2 else dg[:, t * 128:(t + 1) * 128],
                                rhs[:, rr:rr + (512 // HH), :],
                                start=(ti == 0), stop=(ti == len(pe_taps) - 1))
                    slot = img * NSLAB + s
                    col = img * NPIX + s * SLAB
                    zsl = zdst[b][:, col:col + SLAB]
                    psum_in = ps[:].rearrange("p (a b) -> p a b", a=rows)
                    if extra_dve_tap is not None:
                        di, dj = TAPS[extra_dve_tap]
                        acc = P_cacc.tile([128, SLAB], f16, name="cacc", tag="cacc")
                        nc.vector.scalar_tensor_tensor(
                            acc[:].rearrange("p (a b) -> p a b", a=rows),
                            p3[:, 1 + di + r0:1 + di + r0 + rows,
                               1 + dj:1 + dj + HH],
                            wv_t[ci][b][:, extra_dve_tap:extra_dve_tap + 1],
                            psum_in, op0=AL.mult, op1=AL.add)
                        psum_in = acc[:].rearrange("p (a b) -> p a b", a=rows)
                    di, dj = TAPS[DVE_TAP]
                    tap_ap = p3[:, 1 + di + r0:1 + di + r0 + rows,
                                1 + dj:1 + dj + HH]
                    if FUSED_STT:
                        nc.vector.scalar_tensor_tensor(
                            zsl.rearrange("p (a b) -> p a b", a=rows),
                            tap_ap,
                            wv_t[ci][b][:, DVE_TAP:DVE_TAP + 1],
                            psum_in,
                            op0=AL.mult, op1=AL.add,
                            accum_out=Sp[ci][b][:, slot:slot + 1])
                    else:
                        acc = P_cacc.tile([128, SLAB], f16, name="cacc", tag="cacc")
                        nc.vector.tensor_scalar(
                            acc[:].rearrange("p (a b) -> p a b", a=rows),
                            tap_ap, wv_t[ci][b][:, DVE_TAP:DVE_TAP + 1], None,
                            op0=AL.mult)
                        nc.vector.scalar_tensor_tensor(
                            zsl, acc[:], 0.0, ps[:], op0=AL.bypass, op1=AL.add,
                            accum_out=Sp[ci][b][:, slot:slot + 1])
                    sq = P_csq.tile([128, SLAB], f16, name="sqs", tag="sqs")
                    if ACT_SQUARE:
                        nc.scalar.activation(sq[:], zsl, AF.Square,
                                             accum_out=Qp[ci][b][:, slot:slot + 1])
                    else:
                        nc.vector.tensor_tensor_reduce(
                            sq[:], zsl, zsl, 1.0, 0.0, op0=AL.mult, op1=AL.add,
                            accum_out=Qp[ci][b][:, slot:slot + 1])
                    if interleave:
                        interleave.pop(0)()

    def open_conv_pools(sfx):
        return (pools.open(f"cps{sfx}", bufs=2, space="PSUM"),
                pools.open(f"cacc{sfx}", bufs=2),
                pools.open(f"csq{sfx}", bufs=2),
                pools.open(f"diag{sfx}", bufs=2))

    def close_conv_pools(sfx):
        pools.close(f"diag{sfx}", f"csq{sfx}", f"cacc{sfx}", f"cps{sfx}")

    def bn_math(ci, b, S_ap, Q_ap):
        e = bn[ci][b]
        tt = lambda tag: P_tmpv.tile([128, 1], f32, name=tag, tag=tag)
        m = tt("bnm"); e2 = tt("bne"); m2 = tt("bnm2"); v = tt("bnv")
        sq = tt("bnsq"); iv = tt("bniv"); mb = tt("bnmb"); ab = tt("bnab")
        nc.vector.tensor_scalar(m[:], S_ap, inv_n, None, op0=AL.mult)
        nc.vector.tensor_scalar(e2[:], Q_ap, inv_n, None, op0=AL.mult)
        nc.vector.tensor_tensor(m2[:], m[:], m[:], op=AL.mult)
        nc.vector.tensor_tensor(v[:], e2[:], m2[:], op=AL.subtract)
        nc.vector.tensor_scalar(v[:], v[:], EPS, None, op0=AL.add)
        nc.scalar.activation(sq[:], v[:], AF.Sqrt)
        nc.vector.reciprocal(iv[:], sq[:])
        nc.vector.tensor_tensor(e["a"][:], e["g"], iv[:], op=AL.mult)
        nc.vector.tensor_tensor(mb[:], m[:], e["bb"], op=AL.add)
        nc.vector.tensor_tensor(ab[:], e["a"][:], mb[:], op=AL.mult)
        nc.vector.tensor_tensor(e["b"][:], e["be"], ab[:], op=AL.subtract)

    def allreduce(pack, gst, ncols, tag):
        if n_cores == 1:
            nc.vector.tensor_copy(gst[:], pack[:])
            return
        ib = P_dram.tile([128, ncols], f32, name=f"cc_in{tag}", tag=f"cc_in{tag}")
        ob = P_dram.tile([128, ncols], f32, name=f"cc_out{tag}", tag=f"cc_out{tag}")
        nc.gpsimd.dma_start(ib[:], pack[:])
        nc.gpsimd.collective_compute(
            "AllReduce", AL.add,
            replica_groups=[list(range(n_cores))],
            ins=[ib.opt()], outs=[ob.opt()])
        nc.gpsimd.dma_start(gst[:], ob[:])

    def reduce_stats(pack, cols):
        for i, (ci, b) in enumerate(cols):
            nc.vector.tensor_reduce(pack[:, 2 * i:2 * i + 1], Sp[ci][b][:],
                                    axis=mybir.AxisListType.X, op=AL.add)
            nc.vector.tensor_reduce(pack[:, 2 * i + 1:2 * i + 2], Qp[ci][b][:],
                                    axis=mybir.AxisListType.X, op=AL.add)

    # persistent SBUF activations (fp16)
    P_hf = pools.open("hfp", bufs=1)
    hf1a = P_hf.tile([128, R], f16, name="hf1a", tag="hf1a")
    hf1b = P_hf.tile([64, R], f16, name="hf1b", tag="hf1b")

    # =================== stage A: fc1_low + fc1_full ==================
    P_z12 = pools.open("z12p", bufs=1)
    z1_sb = [P_z12.tile([128, R], f16, name="z1sb", tag="z1sb")]
    z2_sb = [P_z12.tile([128, R], f16, name=f"z2sb{b}", tag=f"z2sb{b}")
             for b in range(2)]
    yl_sb = P_z12.tile([128, R], f16, name="ylsb", tag="ylsb")

    P_pad2 = pools.open("pads2", bufs=1)
    P_pad1 = pools.open("pads1", bufs=1)
    y1p = [P_pad1.tile([128, PAREA], f16, name=f"y1p{i}", tag=f"y1p{i}")
           for i in range(2)]
    y2p = [[P_pad2.tile([128, PAREA], f16, name=f"y2p{b}{i}", tag=f"y2p{b}{i}")
            for i in range(2)] for b in range(2)]
    for t in y1p:
        nc.vector.memset(t[:], 0.0)
    for i in range(2):            # image-major: img0 pads ready first
        for b in range(2):
            nc.gpsimd.memset(y2p[b][i][:], 0.0)

    P_wA = pools.open("wAp", bufs=1)
    P_xk = pools.open("xk", bufs=2)
    P_tmpA = pools.open("tmpA", bufs=2)
    P_psA = pools.open("psA", bufs=2, space="PSUM")
    wA_sb = P_wA.tile([128, 24 * 128], f16, name="wAsb", tag="wAsb")
    wAt = lambda k, m: wA_sb[:, (k * 6 + m) * 128:(k * 6 + m + 1) * 128]
    pairs = [(0, 1, lambda img: y1p[img]),
             (2, 4, lambda img: y2p[0][img]),
             (3, 5, lambda img: y2p[1][img])]
    for ch in range(8):
        img, lrow = ch // 4, (ch % 4) * 16
        xs = []
        for k in range(4):
            xt = P_xk.tile([128, 1024], f16, name=f"xk{k}", tag=f"xk{k}")
            eng = nc.sync if k % 2 == 0 else nc.scalar
            eng.dma_start(
                xt[:], x_d[k * 128:(k + 1) * 128, ch * 1024:(ch + 1) * 1024])
            xs.append(xt)
        if ch == 0:
            nc.sync.dma_start(wA_sb[:], wA_d[:])
        for bm, sm, dest in pairs:
            psB = P_psA.tile([128, 1024], f32, name="psB", tag="psB")
            psS = P_psA.tile([128, 1024], f32, name="psS", tag="psS")
            for k in range(4):
                for w, pst in ((wAt(k, bm), psB), (wAt(k, sm), psS)):
                    for nn in range(2):
                        sl = slice(nn * 512, (nn + 1) * 512)
                        nc.tensor.matmul(pst[:, sl], w, xs[k][:, sl],
                                         start=(k == 0), stop=(k == 3))
            tmp = P_tmpA.tile([128, 1024], f16, name="siluA", tag="siluA")
            nc.scalar.activation(tmp[:], psB[:], AF.Silu)
            outap = pad3(dest(img))[:, 1 + lrow:1 + lrow + 16, 1:65]
            nc.vector.scalar_tensor_tensor(
                outap,
                psS[:].rearrange("p (a b) -> p a b", a=16),
                0.0,
                tmp[:].rearrange("p (a b) -> p a b", a=16),
                op0=AL.bypass, op1=AL.add)
    pools.close("psA", "tmpA", "xk", "wAp")
    nc.scalar.dma_start(smalls[:], smalls_d[:])
    nc.scalar.dma_start(w2_sb[:], wfc2_d[:])
    nc.scalar.dma_start(wf1_sb[:], wfus1_d[:])
    nc.scalar.dma_start(w3k[0][:], wfc3_d[0:128, :])
    nc.scalar.dma_start(w3k[1][:], wfc3_d[128:192, :])
    nc.scalar.dma_start(dg12[0][:], conv_d[0]["diag"][:])
    nc.scalar.dma_start(dg12[1][:], conv_d[1]["diag"][:])

    # ============ conv1 -> AR1 (overlapped with conv2+fc2) =============
    cpools12 = open_conv_pools("c12")
    emit_conv(0, [y1p], *cpools12, zdst=z1_sb, extra_dve_tap=8)
    reduce_stats(pack1, [(0, 0)])
    allreduce(pack1, gst1, 2, "a1")

    # fc2 work units, interleaved into conv2's 16 slab iterations
    P_t2 = pools.open("fc2t", bufs=2)
    P_ps2 = pools.open("psF2", bufs=1, space="PSUM")

    def fc2_chunk(ch):
        sl = slice(ch * 1024, (ch + 1) * 1024)
        z1b = P_t2.tile([128, 1024], f16, name="z1b", tag="z1b")
        nc.scalar.activation(z1b[:], z1_sb[0][:, sl], AF.Relu,
                             bias=bn[0][0]["b"][:], scale=bn[0][0]["a"][:])
        psB = P_ps2.tile([128, 1024], f32, name="ps2B", tag="ps2B")
        psS = P_ps2.tile([128, 1024], f32, name="ps2S", tag="ps2S")
        for w, pst in ((w2_sb[:, 0:128], psB), (w2_sb[:, 128:256], psS)):
            for nn in range(2):
                s2 = slice(nn * 512, (nn + 1) * 512)
                nc.tensor.matmul(pst[:, s2], w, z1b[:, s2],
                                 start=True, stop=True)
        tmp = P_t2.tile([128, 1024], f16, name="silu2", tag="silu2")
        nc.scalar.activation(tmp[:], psB[:], AF.Silu)
        nc.vector.scalar_tensor_tensor(yl_sb[:, sl], psS[:], 0.0, tmp[:],
                                       op0=AL.bypass, op1=AL.add)

    INTERLEAVE_FC2 = True
    if INTERLEAVE_FC2:
        todo = [lambda: bn_math(0, 0, gst1[:, 0:1], gst1[:, 1:2])]
        todo += [lambda ch=ch: fc2_chunk(ch) for ch in range(8)]
        ilv = [lambda: None] * 5 + todo
        ilv += [lambda: None] * (16 - len(ilv))
    else:
        ilv = None

    emit_conv(1, y2p, *cpools12, zdst=z2_sb, interleave=ilv)
    if not INTERLEAVE_FC2:
        bn_math(0, 0, gst1[:, 0:1], gst1[:, 1:2])
        for ch in range(8):
            fc2_chunk(ch)
    pools.close("psF2", "fc2t")
    close_conv_pools("c12")
    pools.close("pads1", "pads2")
    reduce_stats(pack2, [(1, 0), (1, 1)])
    allreduce(pack2, gst2, 4, "a2")
    bn_math(1, 0, gst2[:, 0:1], gst2[:, 1:2])
    bn_math(1, 1, gst2[:, 2:3], gst2[:, 3:4])

    # keep the PE array busy through the AR2 collective so the HAM clock
    # gate stays at 8/8 into fusion/fc3 (a >3.4us PE-idle window would
    # re-throttle to 1.2 GHz for tens of us). Garbage-in, never-read-out.
    P_warm = pools.open("pswarm", bufs=1, space="PSUM")
    wps = P_warm.tile([128, 512], f32, name="wps", tag="wps")
    NWARM = 130
    for i in range(NWARM):
        nc.tensor.matmul(wps[:], dummy_w[:], dummy_rhs[:],
                         start=(i == 0), stop=(i == NWARM - 1),
                         skip_group_check=True)
    pools.close("pswarm")

    # =================== fusion linear 1 -> hf1 (SBUF) ==================
    P_tf1 = pools.open("fu1t", bufs=3)
    P_psf1 = pools.open("psFu1", bufs=2, space="PSUM")
    wf1t = lambda k, m: wf1_sb[:, k * FUSH + m * 128:k * FUSH + m * 128 + (64 if m else 128)]
    for ch in range(8):
        sl = slice(ch * 1024, (ch + 1) * 1024)
        z2b0 = P_tf1.tile([128, 1024], f16, name="z2b0", tag="z2b0")
        z2b1 = P_tf1.tile([128, 1024], f16, name="z2b1", tag="z2b1")
        nc.scalar.activation(z2b0[:], z2_sb[0][:, sl], AF.Relu,
                             bias=bn[1][0]["b"][:], scale=bn[1][0]["a"][:])
        nc.vector.tensor_scalar(z2b1[:], z2_sb[1][:, sl], bn[1][1]["a"][:],
                                bn[1][1]["b"][:], op0=AL.mult, op1=AL.add)
        nc.vector.tensor_scalar(z2b1[:], z2b1[:], 0.0, None, op0=AL.max)
        rhs = [yl_sb[:, sl], z2b0[:], z2b1[:]]
        ps0 = P_psf1.tile([128, 1024], f32, name="psf1a", tag="psf1a")
        ps1 = P_psf1.tile([64, 1024], f32, name="psf1b", tag="psf1b")
        for k in range(3):
            for w, pst in ((wf1t(k, 0), ps0), (wf1t(k, 1), ps1)):
                for nn in range(2):
                    s2 = slice(nn * 512, (nn + 1) * 512)
                    nc.tensor.matmul(pst[:, s2], w, rhs[k][:, s2],
                                     start=(k == 0), stop=(k == 2))
        nc.scalar.activation(hf1a[:, sl], ps0[:], AF.Relu, bias=bf1a)
        nc.vector.scalar_tensor_tensor(hf1b[:, sl], ps1[:], bf1b, zero64[:],
                                       op0=AL.add, op1=AL.max)
    pools.close("psFu1", "fu1t")
    pools.close("z12p")

    # ============= fc3' + conv3 (block-split stats) + finals ============
    P_z3 = pools.open("z3p", bufs=1)
    z3_sb = [P_z3.tile([128, R], f16, name=f"z3sb{b}", tag=f"z3sb{b}")
             for b in range(4)]
    P_h3 = pools.open("h3p", bufs=1)
    P_t3 = pools.open("fc3t", bufs=3)
    P_ps3 = pools.open("psF3", bufs=2, space="PSUM")
    P_xc = pools.open("xcp", bufs=3)
    P_fin = pools.open("fint", bufs=3)
    cpools3 = open_conv_pools("c3")

    def fin_chunk(b, ch, dve_bn=True):
        rows = slice(b * 128, (b + 1) * 128)
        sl = slice(ch * 2048, (ch + 1) * 2048)
        xc = P_xc.tile([128, 2048], f16, name="xc", tag="xc")
        nc.scalar.dma_start(xc[:], xs_d[rows, sl])
        t = P_fin.tile([128, 2048], f16, name="trelu", tag="trelu")
        if dve_bn and ch == 3:   # balance: every 4th BN+ReLU on the DVE
            nc.vector.tensor_scalar(t[:], z3_sb[b][:, sl],
                                    bn[2][b]["a"][:], bn[2][b]["b"][:],
                                    op0=AL.mult, op1=AL.add)
            nc.vector.tensor_scalar(t[:], t[:], 0.0, None, op0=AL.max)
        else:
            nc.scalar.activation(t[:], z3_sb[b][:, sl], AF.Relu,
                                 bias=bn[2][b]["b"][:], scale=bn[2][b]["a"][:])
        ob = P_fin.tile([128, 2048], f16, name="ob", tag="ob")
        nc.vector.tensor_tensor(ob[:], xc[:], t[:], op=AL.add)
        nc.sync.dma_start(out_d[rows, sl], ob[:])

    h3sets = []
    for img in range(2):
        h3 = [P_h3.tile([128, PAREA], f16, name=f"h3p{b}", tag=f"h3p{b}")
              for b in range(4)]
        h3sets.append(h3)
        for t in h3:
            nc.gpsimd.memset(t[:], 0.0)
        for ch in range(8):           # 512-px chunks within image
            r0 = ch * 8
            sl = slice(img * NPIX + ch * 512, img * NPIX + (ch + 1) * 512)
            rhs = [hf1a[:, sl], hf1b[:, sl]]
            for mp in range(4):
                psB = P_ps3.tile([128, 512], f32, name="ps3B", tag="ps3B")
                psS = P_ps3.tile([128, 512], f32, name="ps3S", tag="ps3S")
                for kk in range(2):
                    nc.tensor.matmul(psB[:], w3k[kk][:, mp * 128:(mp + 1) * 128],
                                     rhs[kk], start=(kk == 0), stop=(kk == 1))
                    nc.tensor.matmul(psS[:], w3k[kk][:, (4 + mp) * 128:(5 + mp) * 128],
                                     rhs[kk], start=(kk == 0), stop=(kk == 1))
                tmp = P_t3.tile([128, 512], f16, name="silu3", tag="silu3")
                nc.scalar.activation(tmp[:], psB[:], AF.Silu, bias=b3bt[mp])
                outap = pad3(h3[mp])[:, 1 + r0:1 + r0 + 8, 1:65]
                nc.vector.scalar_tensor_tensor(
                    outap,
                    psS[:].rearrange("p (a b) -> p a b", a=8),
                    b3st[mp],
                    tmp[:].rearrange("p (a b) -> p a b", a=8),
                    op0=AL.add, op1=AL.add)
        if img == 0:
            emit_conv(2, [{0: h3[b]} for b in range(4)], *cpools3,
                      zdst=z3_sb, imgs=(0,), extra_dve_tap=8)
    # conv3 img1: blocks 0-1, then AR3a fires while blocks 2-3 conv and
    # the finals for blocks 0-1 interleave into their slab loop.
    emit_conv(2, [{1: h3sets[1][b]} for b in range(4)], *cpools3,
              zdst=z3_sb, imgs=(1,), blocks=(0, 1), extra_dve_tap=8)
    reduce_stats(pack3a, [(2, 0), (2, 1)])
    allreduce(pack3a, gst3a, 4, "a3a")
    # bn_math + the first finals are staggered into conv3 blocks 2-3 via the
    # interleave hooks so their AR3a-gated ops never head-of-line-block the
    # DVE/ACT queues ahead of conv3's own slab work.
    ilv3 = [lambda: None] * 3
    ilv3.append(lambda: bn_math(2, 0, gst3a[:, 0:1], gst3a[:, 1:2]))
    ilv3.append(lambda: bn_math(2, 1, gst3a[:, 2:3], gst3a[:, 3:4]))
    ilv3 += [lambda ch=ch: fin_chunk(0, ch, dve_bn=False) for ch in range(3)]
    emit_conv(2, [{1: h3sets[1][b]} for b in range(4)], *cpools3,
              zdst=z3_sb, imgs=(1,), blocks=(2, 3), interleave=ilv3)
    fin_chunk(0, 3, dve_bn=False)
    for ch in range(4):
        fin_chunk(1, ch, dve_bn=False)
    reduce_stats(pack3b, [(2, 2), (2, 3)])
    allreduce(pack3b, gst3b, 4, "a3b")
    bn_math(2, 2, gst3b[:, 0:1], gst3b[:, 1:2])
    bn_math(2, 3, gst3b[:, 2:3], gst3b[:, 3:4])
    for b in (2, 3):
        for ch in range(4):
            fin_chunk(b, ch)
    pools.close_all()


def _get_built(n_cores):
    if n_cores not in _CACHE:
        _CACHE[n_cores] = _build(n_cores)
    return _CACHE[n_cores]


def make_in_maps(inputs, n_cores):
    shared = _prep_shared(inputs)
    xt = _prep_x(inputs["x"], n_cores)
    rsv = float(np.asarray(inputs["res_scale"]).reshape(-1)[0])
    xst = _prep_x(inputs["x"], n_cores, scale=rsv)
    return [dict(shared, x_t=xt[c], xs_t=xst[c]) for c in range(n_cores)]


def kernel(**inputs):
    from concourse.bass_utils import run_bass_kernel_spmd

    assert int(np.asarray(inputs["H"])) == HH and int(np.asarray(inputs["W"])) == HH
    n_cores = 8
    nc = _get_built(n_cores)
    in_maps = make_in_maps(inputs, n_cores)
    res = run_bass_kernel_spmd(nc, in_maps, core_ids=list(range(n_cores)))
    B = np.asarray(inputs["x"]).shape[0]
    per = B // n_cores
    out = np.empty((B, NPIX, CIN), np.float32)
    for c in range(n_cores):
        out[c * per:(c + 1) * per] = \
            res.results[c]["out_t"].astype(np.float32).T.reshape(per, NPIX, CIN)
    return out


# ------------------------------------------------------------- profiling

def _install_ntff_hook():
    """The agent image's antenv lacks axon_hooks; recreate the NTFF profile
    hook via ctypes on the axon PJRT .so (same ABI as trn_boot's)."""
    import contextlib, ctypes, sys, types
    so = "/opt/axon/libaxon_pjrt.so"
    try:
        import antenv.axon_hooks  # noqa: F401
        return True
    except ImportError:
        pass
    try:
        lib = ctypes.CDLL(so)
    except OSError:
        return False
    if not hasattr(lib, "axon_start_nrt_profile"):
        return False
    lib.axon_start_nrt_profile.argtypes = [
        ctypes.POINTER(ctypes.c_int64), ctypes.c_size_t]
    lib.axon_start_nrt_profile.restype = ctypes.c_int64
    lib.axon_stop_nrt_profile.argtypes = [ctypes.c_char_p]
    lib.axon_stop_nrt_profile.restype = ctypes.c_int64

    @contextlib.contextmanager
    def _hook(output_dir, device_ids):
        import jax
        jax.devices()
        if device_ids:
            ids = (ctypes.c_int64 * len(device_ids))(*device_ids)
            rc = lib.axon_start_nrt_profile(ids, len(device_ids))
        else:
            rc = lib.axon_start_nrt_profile(None, 0)
        if rc != 0:
            raise RuntimeError(f"axon_start_nrt_profile rc={rc}")
        try:
            yield
        finally:
            n = lib.axon_stop_nrt_profile(str(output_dir).encode())
            print(f"profile: {n} ntff file(s) -> {output_dir}", file=sys.stderr)

    mod = types.ModuleType("antenv.axon_hooks")
    mod.get_axon_ntff_profile_hook = lambda: _hook
    mod.set_axon_ntff_profile_hook = lambda h: None
    sys.modules["antenv.axon_hooks"] = mod
    import concourse.bass_utils as bu
    bu.upload_artifacts = lambda tmpdir: f"local:{tmpdir}"
    return True


def benchmark(inputs, iters=2, tmpdir=None):
    """Device-only HW execution time (ns) via neuron-profile NTFF trace."""
    import os, tempfile
    from concourse.bass_utils import run_bass_kernel_spmd

    if not _install_ntff_hook():
        raise RuntimeError("NTFF profiling unavailable")
    if tmpdir:
        os.makedirs(tmpdir, exist_ok=True)
    n_cores = 8
    nc = _get_built(n_cores)
    in_maps = make_in_maps(inputs, n_cores)
    times = []
    for i in range(max(1, min(iters, 3))):
        td = tempfile.mkdtemp(dir=tmpdir) if tmpdir else None
        res = run_bass_kernel_spmd(nc, in_maps, core_ids=list(range(n_cores)),
                                   trace=True, tmpdir=td)
        if res.exec_time_ns is not None:
            times.append(res.exec_time_ns)
    if not times:
        raise RuntimeError("no exec_time_ns from traced runs")
    return min(times)
